# revision 54
# baseline (speedup 1.0000x reference)
"""GCN layer kernel for trn2: host prep + bass kernel builder + runner.

v2: fp16 tables (p-major row numbering), batched DMAs, fp8 sexp fed
directly to PE, fp16 DVE fast modes, grouped multi-tile gathers.
"""
import sys
sys.path.insert(0, '/opt/trn_rl_repo')
import numpy as np
import ml_dtypes
from dataclasses import dataclass

import concourse.bacc as bacc
import concourse.mybir as mybir
import concourse.tile as tile
from concourse.bass_utils import run_bass_kernel_spmd

F32 = mybir.dt.float32
F16 = mybir.dt.float16
I16 = mybir.dt.int16
U32 = mybir.dt.uint32
FP8 = mybir.dt.float8e4
AF = mybir.ActivationFunctionType
OP = mybir.AluOpType

NT = 392          # total node tiles (50176/128)


@dataclass
class Cfg:
    nodes: int = 50176
    lsplit: int = 32536       # 83*392; p-major row split (<=32768 for int16)
    cores: int = 8
    d: int = 128
    kcut: int = 0
    nlk: tuple = ()           # L chunks per tile-rank (len tpc)
    nhk: tuple = ()           # H chunks per tile-rank
    perms: tuple = ()         # per-core tile permutation (host side only)
    G: int = 5                # tiles per gather group
    slab: int = 8             # y chunks per PSUM slab
    bblk: int = 16            # build tiles per DMA block
    cand_cols: int = 256
    nrounds_local: int = 2
    phases: str = "full"      # "build", "A", "T", "full"
    coll: bool = True         # False: stub collectives (tlsim)

    @property
    def tpc(self):
        return self.nodes // 128 // self.cores

    @property
    def own(self):
        return self.nodes // self.cores

    @property
    def nchunk(self):
        return int(sum(self.nlk) + sum(self.nhk))

    @property
    def slots(self):
        return self.nchunk * 128

    @property
    def groups(self):
        gs = []
        t = 0
        while t < self.tpc:
            gs.append(list(range(t, min(t + self.G, self.tpc))))
            t += self.G
        return gs


def group_meta(cfg):
    """Per group: (gbase_chunk, sbase_slot, ltot, htot, loffs, hoffs)."""
    out = []
    gb = sb = 0
    for grp in cfg.groups:
        lo, ho = [], []
        lt = ht = 0
        for k in grp:
            lo.append(lt); lt += cfg.nlk[k]
        for k in grp:
            ho.append(ht); ht += cfg.nhk[k]
        out.append((gb, sb, lt, ht, lo, ho))
        gb += lt + ht
        sb += (lt + ht) * 128
    return out


def host_prep(cfg: Cfg, src, dst):
    """Per-core gidx/dstloc/sexp in grouped-slot order, p-major row ids."""
    rowid = (src.astype(np.int64) % 128) * NT + (src.astype(np.int64) // 128)
    is_h_all = rowid >= cfg.lsplit
    rloc_all = rowid - np.where(is_h_all, cfg.lsplit, 0)
    core_of = dst // cfg.own
    gm = group_meta(cfg)
    out = []
    for c in range(cfg.cores):
        sel = np.nonzero(core_of == c)[0]
        rl = rloc_all[sel]
        ih = is_h_all[sel]
        dloc = dst[sel].astype(np.int64) - c * cfg.own
        t_all = dloc // 128
        loc = dloc % 128
        gidx = np.zeros(cfg.slots, np.int64)
        dstloc = np.full(cfg.slots, -1.0, np.float32)
        order = np.lexsort((loc, ih, t_all))
        rl, t_all, loc, ih = rl[order], t_all[order], loc[order], ih[order]
        perm = cfg.perms[c]
        for gi_, grp in enumerate(cfg.groups):
            _, sbase, ltot, htot, lo, ho = gm[gi_]
            for i, k in enumerate(grp):
                tt = perm[k]
                for hs, budget, coff in ((0, cfg.nlk[k], lo[i]),
                                         (1, cfg.nhk[k], ltot + ho[i])):
                    mm = (t_all == tt) & (ih == hs)
                    n = int(mm.sum())
                    assert n <= budget * 128, \
                        f"c{c} k{k} hs{hs}: {n}>{budget*128}"
                    gi = np.zeros(budget * 128, np.int64)
                    gi[:n] = rl[mm]
                    dl = np.full(budget * 128, -1.0, np.float32)
                    dl[:n] = loc[mm].astype(np.float32)
                    s0 = sbase + coff * 128
                    gidx[s0:s0 + budget * 128] = gi
                    dstloc[s0:s0 + budget * 128] = dl
        gidx_w = np.ascontiguousarray(
            np.tile(gidx.astype(np.int16).reshape(-1, 16).T, (8, 1)))
        dstloc_pc = np.ascontiguousarray(
            dstloc.reshape(cfg.nchunk, 128).T)
        out.append(dict(gidx=gidx_w, dstloc=dstloc_pc))
    return out


def build_nc(cfg: Cfg):
    nc = bacc.Bacc(None)
    d = cfg.d
    TPC, NCH = cfg.tpc, cfg.nchunk
    GM = group_meta(cfg)
    MAXL = max(m[2] for m in GM)
    MAXH = max(m[3] for m in GM)
    KCUT = float(cfg.kcut)
    LROWS, HROWS = cfg.lsplit, cfg.nodes - cfg.lsplit

    feat = nc.dram_tensor("feat", [128, NT * d], F32, kind="ExternalInput")
    wT = nc.dram_tensor("wT", [d, d], F32, kind="ExternalInput")
    hpre = nc.dram_tensor("hpre", [128, TPC * d], F32, kind="ExternalInput")
    degp = nc.dram_tensor("degp", [128, TPC], F32, kind="ExternalInput")
    degall = nc.dram_tensor("degall", [128, NT], F32, kind="ExternalInput")
    gidx_e = nc.dram_tensor("gidx", [128, cfg.slots // 16], I16,
                            kind="ExternalInput")
    dstloc_e = nc.dram_tensor("dstloc", [128, NCH], F32, kind="ExternalInput")
    h_ext = nc.dram_tensor("h", [128, TPC * d], F32, kind="ExternalOutput")
    ts_dbg = nc.dram_tensor("ts_dbg", [1, 4], F32, kind="ExternalOutput")

    cc_in = nc.dram_tensor("cc_in", [1, 32], F32)
    cc_out = nc.dram_tensor("cc_out", [1, 32], F32, addr_space="Shared")
    ag_in = nc.dram_tensor("ag_in", [16, cfg.cand_cols], F32)
    ag_out = nc.dram_tensor("ag_out", [cfg.cores, 16, cfg.cand_cols], F32,
                            addr_space="Shared")
    groups_rep = [list(range(cfg.cores))]

    with tile.TileContext(nc) as tc:
        with (tc.tile_pool(name="const", bufs=1) as cpool,
              tc.tile_pool(name="state", bufs=1) as spool,
              tc.tile_pool(name="dram", bufs=1, space="DRAM") as dpool,
              tc.tile_pool(name="bfeat", bufs=2) as bfpool,
              tc.tile_pool(name="bstage", bufs=2) as bspool,
              tc.tile_pool(name="gath", bufs=2) as gpool,
              tc.tile_pool(name="ysb", bufs=4) as ypool_sb,
              tc.tile_pool(name="hstage", bufs=2) as hpool,
              tc.tile_pool(name="ypsum", bufs=2, space="PSUM") as ypool,
              tc.tile_pool(name="apsum", bufs=2, space="PSUM") as apool,
              tc.tile_pool(name="tpsum", bufs=2, space="PSUM") as tppool,
              tc.tile_pool(name="misc", bufs=3) as mpool,
              tc.tile_pool(name="y16p", bufs=2) as y16pool,
              tc.tile_pool(name="thr", bufs=1) as tpool):

            # ---------- DRAM tables (fp16, contiguous p-major rows;
            # pad slots gather garbage row 0 -- harmless, sexp col is 0) ----
            nhT = dpool.tile([cfg.nodes, d], F16, tag="nhT")
            ftT = dpool.tile([cfg.nodes, d], F16, tag="ftT")

            # ---------- constants / inputs ----------
            iota_row = cpool.tile([128, 128], F32, tag="iota_row")
            nc.gpsimd.iota(iota_row[:], pattern=[[1, 128]], base=0,
                           channel_multiplier=0,
                           allow_small_or_imprecise_dtypes=True)
            iota16 = cpool.tile([128, 128], F16, tag="iota16")
            nc.vector.tensor_copy(iota16[:], iota_row[:])
            iota_col = cpool.tile([128, 1], F32, tag="iota_col")
            nc.gpsimd.iota(iota_col[:], pattern=[[1, 1]], base=0,
                           channel_multiplier=1,
                           allow_small_or_imprecise_dtypes=True)
            ones_col = cpool.tile([128, 1], F32, tag="ones_col")
            nc.vector.memset(ones_col[:], 1.0)
            ones_row = cpool.tile([1, 128], F32, tag="ones_row")
            nc.vector.memset(ones_row[:], 1.0)
            wT_sb = cpool.tile([d, d], F32, tag="wT")
            nc.sync.dma_start(wT_sb[:], wT[:])
            wT16 = cpool.tile([d, d], F16, tag="wT16")
            nc.vector.tensor_copy(wT16[:], wT_sb[:])
            hp_sb = spool.tile([128, TPC * d], F32, tag="hp")
            nc.sync.dma_start(hp_sb[:], hpre[:])
            degp_sb = spool.tile([128, TPC], F32, tag="degp")
            nc.sync.dma_start(degp_sb[:], degp[:])
            degall_sb = spool.tile([128, NT], F32, tag="degall")
            nc.sync.dma_start(degall_sb[:], degall[:])
            gidx_sb = spool.tile([128, cfg.slots // 16], I16, tag="gidx")
            nc.sync.dma_start(gidx_sb[:], gidx_e[:])
            dstloc_sb = spool.tile([128, NCH], F32, tag="dstloc")
            nc.sync.dma_start(dstloc_sb[:], dstloc_e[:])

            # norms
            norm_own = spool.tile([128, TPC], F32, tag="norm_own")
            nc.vector.tensor_scalar_max(norm_own[:], degp_sb[:], 1.0)
            nc.scalar.activation(norm_own[:], norm_own[:], AF.Sqrt)
            nc.vector.reciprocal(norm_own[:], norm_own[:])
            norm_all = spool.tile([128, NT], F32, tag="norm_all")
            nc.vector.tensor_scalar_max(norm_all[:], degall_sb[:], 1.0)
            nc.scalar.activation(norm_all[:], norm_all[:], AF.Sqrt)
            nc.vector.reciprocal(norm_all[:], norm_all[:])

            # nhi_all: normalized own-tile rows (fp16), for phase-A rhs
            ssq = spool.tile([128, TPC], F32, tag="ssq")
            for t in range(TPC):
                scr = mpool.tile([128, d], F32, tag="sqscr")
                nc.scalar.activation(scr[:], hp_sb[:, t * d:(t + 1) * d],
                                     AF.Square, accum_out=ssq[:, t:t + 1])
            invl_own = spool.tile([128, TPC], F32, tag="invl_own")
            # floor 1e-8 (not 1e-24): invl <= 1e4 stays fp16-finite in dmat;
            # zero-feature pad rows still give nhi = 0 * 1e4 = 0 exactly
            nc.vector.tensor_scalar_max(ssq[:], ssq[:], 1e-8)
            nc.scalar.activation(invl_own[:], ssq[:], AF.Sqrt)
            nc.vector.reciprocal(invl_own[:], invl_own[:])
            # nhiT_all[:, t*d+j] = nhi_t[j, :] transposed, via hp^T @ D
            ident32 = cpool.tile([128, 128], F32, tag="ident32")
            nc.vector.tensor_scalar(ident32[:], iota_row[:], iota_col[:],
                                    None, op0=OP.is_equal)
            nhiT_all = spool.tile([128, TPC * d], F16, tag="nhiT_all")
            for t in range(TPC):
                dmat = mpool.tile([128, d], F32, tag="dmat")
                nc.vector.tensor_scalar_mul(dmat[:], ident32[:],
                                            invl_own[:, t:t + 1])
                ntp = apool.tile([128, d], F32, tag="ab")
                nc.tensor.matmul(ntp[:], hp_sb[:, t * d:(t + 1) * d],
                                 dmat[:], start=True, stop=True)
                nc.scalar.copy(nhiT_all[:, t * d:(t + 1) * d], ntp[:])

            # ---------- table build (blocked) ----------
            bscope = nc.named_scope("pbuild")
            bscope.__enter__()
            nb = cfg.bblk
            for b0 in range(0, NT, nb):
                bt = min(nb, NT - b0)
                fblk = bfpool.tile([128, nb * d], F32, tag="fblk")
                nc.sync.dma_start(fblk[:, :bt * d],
                                  feat[:, b0 * d:(b0 + bt) * d])
                bss = mpool.tile([128, nb], F32, tag="bss")
                for ti in range(bt):
                    scr = mpool.tile([128, d], F32, tag="bsq")
                    nc.vector.scalar_tensor_tensor(
                        scr[:], fblk[:, ti * d:(ti + 1) * d], 1.0,
                        fblk[:, ti * d:(ti + 1) * d],
                        op0=OP.mult, op1=OP.mult,
                        accum_out=bss[:, ti:ti + 1])
                nc.vector.tensor_scalar_max(bss[:, :bt], bss[:, :bt], 1e-24)
                nc.scalar.activation(bss[:, :bt], bss[:, :bt], AF.Sqrt)
                nc.vector.reciprocal(bss[:, :bt], bss[:, :bt])
                nh_st = bspool.tile([128, nb * d], F16, tag="nh_st")
                ft_st = bspool.tile([128, nb * d], F16, tag="ft_st")
                for ti in range(bt):
                    nc.vector.tensor_scalar_mul(
                        nh_st[:, ti * d:(ti + 1) * d],
                        fblk[:, ti * d:(ti + 1) * d], bss[:, ti:ti + 1])
                    nc.scalar.activation(
                        ft_st[:, ti * d:(ti + 1) * d],
                        fblk[:, ti * d:(ti + 1) * d], AF.Copy,
                        scale=norm_all[:, b0 + ti:b0 + ti + 1])
                for tab, st in ((nhT, nh_st), (ftT, ft_st)):
                    nc.sync.dma_start(
                        tab[:]
                        .rearrange("(p t) x -> p t x", t=NT)[:, b0:b0 + bt, :],
                        st[:, :bt * d]
                        .rearrange("p (t x) -> p t x", x=d))
            bscope.__exit__(None, None, None)

            def emit_b_gather(gi_):
                _, sbase, ltot, htot, _, _ = GM[gi_]
                xfl = gpool.tile([128, MAXL, d], F16, tag="xgl")
                xfh = gpool.tile([128, MAXH, d], F16, tag="xgh")
                i0 = sbase // 16
                nc.gpsimd.dma_gather(
                    out_ap=xfl[:, :ltot, :], in_ap=ftT[0:LROWS, :],
                    idxs_ap=gidx_sb[:, i0:i0 + ltot * 8],
                    num_idxs=ltot * 128, num_idxs_reg=ltot * 128,
                    elem_size=d, single_packet=False)
                nc.gpsimd.dma_gather(
                    out_ap=xfh[:, :htot, :], in_ap=ftT[LROWS:cfg.nodes, :],
                    idxs_ap=gidx_sb[:, i0 + ltot * 8:i0 + (ltot + htot) * 8],
                    num_idxs=htot * 128, num_idxs_reg=htot * 128,
                    elem_size=d, single_packet=False)
                return xfl, xfh

            run_a = cfg.phases in ("A", "T", "full")
            run_t = cfg.phases in ("T", "full")
            run_b = cfg.phases == "full"
            cos_sb = spool.tile([128, NCH], F32, tag="cos")
            if not run_a:
                nc.vector.memset(cos_sb[:, :1], 0.0)

            # ---------- Phase A: cos ----------
            if run_a:
                ascope = nc.named_scope("pcos")
                ascope.__enter__()
                for gi_, grp in enumerate(cfg.groups):
                    gbase, sbase, ltot, htot, lo, ho = GM[gi_]
                    xgl = gpool.tile([128, MAXL * 128], F16, tag="xgl")
                    xgh = gpool.tile([128, MAXH * 128], F16, tag="xgh")
                    i0 = sbase // 16
                    nc.gpsimd.dma_gather(
                        out_ap=xgl[:, :ltot * 128]
                        .rearrange("p (o n) -> p o n", o=1),
                        in_ap=nhT[0:LROWS, :],
                        idxs_ap=gidx_sb[:, i0:i0 + ltot * 8],
                        num_idxs=ltot * 128, num_idxs_reg=ltot * 128,
                        elem_size=d, transpose=True, single_packet=False)
                    nc.gpsimd.dma_gather(
                        out_ap=xgh[:, :htot * 128]
                        .rearrange("p (o n) -> p o n", o=1),
                        in_ap=nhT[LROWS:cfg.nodes, :],
                        idxs_ap=gidx_sb[:, i0 + ltot * 8:
                                        i0 + (ltot + htot) * 8],
                        num_idxs=htot * 128, num_idxs_reg=htot * 128,
                        elem_size=d, transpose=True, single_packet=False)
                    for i, t in enumerate(grp):
                        nlt, nht = cfg.nlk[t], cfg.nhk[t]
                        nct_k = nlt + nht
                        for s0 in range(0, nct_k, cfg.slab):
                            sn = min(cfg.slab, nct_k - s0)
                            c2_ps = ypool.tile([128, cfg.slab * d], F32,
                                               tag="y")
                            for j in range(sn):
                                k = s0 + j
                                if k < nlt:
                                    xT_ap = xgl[:, (lo[i] + k) * 128:
                                                (lo[i] + k) * 128 + 128]
                                else:
                                    hc = ho[i] + (k - nlt)
                                    xT_ap = xgh[:, hc * 128:hc * 128 + 128]
                                nc.tensor.matmul(
                                    c2_ps[:, j * d:(j + 1) * d],
                                    xT_ap,
                                    nhiT_all[:, t * d:(t + 1) * d],
                                    start=True, stop=True)
                            c2_sb = ypool_sb.tile([128, cfg.slab * d], F16,
                                                  tag="ysb")
                            nc.scalar.copy(c2_sb[:, :sn * d],
                                           c2_ps[:, :sn * d])
                            for j in range(sn):
                                k = s0 + j
                                ccg = (lo[i] + k if k < nlt
                                       else ltot + ho[i] + (k - nlt))
                                cc = gbase + ccg
                                scr = mpool.tile([128, d], F16, tag="cscr")
                                nc.vector.scalar_tensor_tensor(
                                    scr[:], iota16[:],
                                    dstloc_sb[:, cc:cc + 1],
                                    c2_sb[:, j * d:(j + 1) * d],
                                    op0=OP.is_equal, op1=OP.mult,
                                    accum_out=cos_sb[:, cc:cc + 1])
                ascope.__exit__(None, None, None)

            # prefetch first B gather groups (overlap with threshold phase)
            pf = []
            if run_b:
                for gi in range(min(2, len(cfg.groups))):
                    pf.append(emit_b_gather(gi))

            # ---------- Phase T: threshold ----------
            if run_t:
                tscope = nc.named_scope("pthr")
                tscope.__enter__()
                lo_t = tpool.tile([1, 1], F32, tag="lo")
                th_row = tpool.tile([1, 32], F32, tag="throw")
                th_bc = tpool.tile([128, 32], F32, tag="thbc")
                cnt128 = tpool.tile([128, 32], F32, tag="cnt128")
                gcnt = tpool.tile([1, 32], F32, tag="gcnt")
                srow = tpool.tile([1, 1], F32, tag="srow")
                cbase = tpool.tile([1, 1], F32, tag="cbase")
                iota32 = tpool.tile([1, 32], F32, tag="iota32")
                nc.vector.tensor_copy(iota32[:], iota_row[:1, :32])
                msk = tpool.tile([1, 32], F32, tag="msk")
                msct = tpool.tile([1, 32], F32, tag="msct")
                cscr2 = tpool.tile([128, NCH], F32, tag="cscr2")
                cand = tpool.tile([128, cfg.cand_cols], F32, tag="cand")
                nc.vector.memset(cbase[:], 0.0)
                nc.vector.memset(lo_t[:], -0.75)

                def emit_round(vals_ap, ncols, w_bin, mode, shift4):
                    nc.vector.tensor_scalar_mul(th_row[:], iota32[:], w_bin)
                    nc.vector.tensor_scalar(th_row[:], th_row[:], lo_t[:],
                                            None, op0=OP.add)
                    if shift4:
                        nc.vector.tensor_scalar_add(th_row[:], th_row[:], 4.0)
                    ps = tppool.tile([128, 32], F32, tag="tiny")
                    nc.tensor.matmul(ps[:], ones_row[:], th_row[:],
                                     start=True, stop=True)
                    nc.vector.tensor_copy(th_bc[:], ps[:])
                    for j in range(32):
                        nc.vector.tensor_scalar(
                            cscr2[:, :ncols], vals_ap, th_bc[:, j:j + 1],
                            None, op0=OP.is_lt, op1=OP.add,
                            accum_out=cnt128[:, j:j + 1])
                    cps = tppool.tile([1, 32], F32, tag="tiny")
                    nc.tensor.matmul(cps[:], ones_col[:], cnt128[:],
                                     start=True, stop=True)
                    nc.vector.tensor_copy(gcnt[:], cps[:])
                    if mode.startswith("global"):
                        if cfg.coll:
                            nc.sync.dma_start(cc_in[:], gcnt[:])
                            nc.gpsimd.collective_compute(
                                "AllReduce", OP.add,
                                replica_groups=groups_rep,
                                ins=[cc_in[:]], outs=[cc_out[:]])
                            nc.sync.dma_start(gcnt[:], cc_out[:])
                        else:
                            nc.sync.dma_start(cc_in[:], gcnt[:])
                            nc.sync.dma_start(gcnt[:], cc_in[:])
                    nc.vector.tensor_scalar(
                        msct[:], gcnt[:], cbase[:], KCUT - 0.5,
                        op0=OP.add, op1=OP.is_lt)
                    nc.vector.tensor_scalar(
                        msct[:], msct[:], 0.0, None,
                        op0=OP.add, op1=OP.add, accum_out=srow[:])
                    nc.vector.tensor_scalar(srow[:], srow[:], -1.0, 0.0,
                                            op0=OP.add, op1=OP.max)
                    if mode == "global2":
                        nc.vector.tensor_scalar(msk[:], iota32[:], srow[:],
                                                None, op0=OP.is_equal)
                        nc.vector.scalar_tensor_tensor(
                            msct[:], gcnt[:], 1.0, msk[:],
                            op0=OP.mult, op1=OP.mult, accum_out=cbase[:])
                    nc.vector.scalar_tensor_tensor(
                        lo_t[:], srow[:], w_bin, lo_t[:],
                        op0=OP.mult, op1=OP.add)

                W1 = 1.5 / 32
                W2 = 1.5 / 32 ** 2
                emit_round(cos_sb[:], NCH, W1, "global1", False)
                emit_round(cos_sb[:], NCH, W2, "global2", False)

                # compact in-bracket values, remapped to cos+4
                lo_bc = tpool.tile([128, 1], F32, tag="lobc")
                psb = tppool.tile([128, 1], F32, tag="tiny")
                nc.tensor.matmul(psb[:], ones_row[:], lo_t[:],
                                 start=True, stop=True)
                nc.vector.tensor_copy(lo_bc[:], psb[:])
                m1 = tpool.tile([128, NCH], F32, tag="m1")
                nc.vector.tensor_scalar(m1[:], cos_sb[:], lo_bc[:], None,
                                        op0=OP.is_ge)
                hi_bc = tpool.tile([128, 1], F32, tag="hibc")
                nc.vector.tensor_scalar_add(hi_bc[:], lo_bc[:], W2)
                m2 = tpool.tile([128, NCH], F32, tag="m2")
                nc.vector.tensor_scalar(m2[:], cos_sb[:], hi_bc[:], None,
                                        op0=OP.is_lt)
                nc.vector.tensor_mul(m1[:], m1[:], m2[:])
                c4 = tpool.tile([128, NCH], F32, tag="c4")
                nc.vector.tensor_scalar(c4[:], cos_sb[:], 5.0, None,
                                        op0=OP.add)
                nc.vector.tensor_mul(c4[:], c4[:], m1[:])
                nc.vector.tensor_scalar_add(c4[:], c4[:], -1.0)
                NSG = 8
                sg_out_cols = cfg.cand_cols // NSG
                sgc = tpool.tile([16, cfg.cand_cols], F32, tag="sgc")
                posi = tpool.tile([16, sg_out_cols], F32, tag="posi")
                nc.gpsimd.iota(posi[:], pattern=[[16, sg_out_cols]], base=0,
                               channel_multiplier=1,
                               allow_small_or_imprecise_dtypes=True)
                for sg_i in range(NSG):
                    y16s = y16pool.tile([16, NCH], F32, tag="y16s")
                    nc.sync.dma_start(y16s[:], c4[16 * sg_i:16 * (sg_i + 1), :])
                    sg_out = y16pool.tile([16, sg_out_cols], F32,
                                          tag="sgout")
                    nfound = y16pool.tile([1, 1], U32, tag="nfound")
                    nc.gpsimd.sparse_gather(sg_out[:], y16s[:],
                                            num_found=nfound[:])
                    nf_f = y16pool.tile([1, 1], F32, tag="nff")
                    nc.vector.tensor_copy(nf_f[:], nfound[:])
                    nf16 = y16pool.tile([16, 1], F32, tag="nf16")
                    ps16 = tppool.tile([16, 1], F32, tag="tiny")
                    nc.tensor.matmul(ps16[:], ones_row[:, :16], nf_f[:],
                                     start=True, stop=True)
                    nc.vector.tensor_copy(nf16[:], ps16[:])
                    mtail = y16pool.tile([16, sg_out_cols], F32,
                                         tag="mtail")
                    nc.vector.tensor_scalar(mtail[:], posi[:], nf16[:], None,
                                            op0=OP.is_lt)
                    big = y16pool.tile([16, sg_out_cols], F32, tag="big")
                    nc.vector.tensor_scalar(big[:], mtail[:], 0.5, 1e30,
                                            op0=OP.is_lt, op1=OP.mult)
                    nc.vector.tensor_mul(sg_out[:], sg_out[:], mtail[:])
                    nc.vector.tensor_add(
                        sgc[:, sg_i * sg_out_cols:(sg_i + 1) * sg_out_cols],
                        sg_out[:], big[:])
                nc.sync.dma_start(ag_in[:], sgc[:])
                if cfg.coll:
                    nc.gpsimd.collective_compute(
                        "AllGather", OP.bypass, replica_groups=groups_rep,
                        ins=[ag_in[:]], outs=[ag_out[:]])
                    for r in range(cfg.cores):
                        nc.sync.dma_start(cand[16 * r:16 * (r + 1), :],
                                          ag_out[r, :, :])
                else:
                    for r in range(cfg.cores):
                        nc.sync.dma_start(cand[16 * r:16 * (r + 1), :],
                                          ag_in[:])

                wr = W2
                for r in range(cfg.nrounds_local):
                    wr = wr / 32
                    emit_round(cand[:], cfg.cand_cols, wr, "local", True)
                nc.vector.tensor_scalar_add(lo_t[:], lo_t[:], wr)
                tstar = tpool.tile([128, 1], F32, tag="tstar")
                pst = tppool.tile([128, 1], F32, tag="tiny")
                nc.tensor.matmul(pst[:], ones_row[:], lo_t[:],
                                 start=True, stop=True)
                nc.vector.tensor_copy(tstar[:], pst[:])

                nc.sync.dma_start(ts_dbg[:, 0:1], lo_t[:])
                nc.sync.dma_start(ts_dbg[:, 1:2], cbase[:])
                nc.sync.dma_start(ts_dbg[:, 2:3], srow[:])
                nc.sync.dma_start(ts_dbg[:, 3:4], nf_f[:])
                # dm = keep*(dstloc+1) - 1  (keep = cos >= t*)
                keep = tpool.tile([128, NCH], F32, tag="m1")
                nc.vector.tensor_scalar(keep[:], cos_sb[:], tstar[:], None,
                                        op0=OP.is_ge)
                dm = tpool.tile([128, NCH], F32, tag="c4")
                nc.vector.tensor_scalar_add(dm[:], dstloc_sb[:], 1.0)
                nc.vector.tensor_mul(dm[:], dm[:], keep[:])
                nc.vector.tensor_scalar_add(dm[:], dm[:], -1.0)
                tscope.__exit__(None, None, None)

            # ---------- Phase B: aggregate + linear + tail ----------
            if run_b:
                bscope2 = nc.named_scope("pagg")
                bscope2.__enter__()
                hblk = 8
                hout_st = None
                for gi, grp in enumerate(cfg.groups):
                    gbase, sbase, ltot, htot, lo, ho = GM[gi]
                    if gi < len(pf):
                        xfl, xfh = pf[gi]
                    else:
                        xfl, xfh = emit_b_gather(gi)
                    for i, t in enumerate(grp):
                        nlt, nht = cfg.nlk[t], cfg.nhk[t]
                        nct_k = nlt + nht
                        if t % hblk == 0:
                            hout_st = hpool.tile([128, hblk * d], F32,
                                                 tag="hout")
                        at_ps = apool.tile([128, 128], F32, tag="ab")
                        for k in range(nct_k):
                            if k < nlt:
                                cc = gbase + lo[i] + k
                                x_ap = xfl[:, lo[i] + k, :]
                            else:
                                cc = gbase + ltot + ho[i] + (k - nlt)
                                x_ap = xfh[:, ho[i] + (k - nlt), :]
                            sa = mpool.tile([128, 128], F16, tag="sa")
                            nc.vector.tensor_scalar(sa[:], iota16[:],
                                                    dm[:, cc:cc + 1], None,
                                                    op0=OP.is_equal)
                            nc.tensor.matmul(at_ps[:], x_ap, sa[:],
                                             start=(k == 0),
                                             stop=(k == nct_k - 1))
                        at_sb = mpool.tile([128, 128], F16, tag="aggTsb")
                        nc.scalar.copy(at_sb[:], at_ps[:])
                        h_ps = apool.tile([128, d], F32, tag="ab")
                        nc.tensor.matmul(h_ps[:], at_sb[:], wT16[:],
                                         start=True, stop=True)
                        hre = mpool.tile([128, d], F32, tag="hre")
                        nc.scalar.activation(hre[:], h_ps[:], AF.Relu,
                                             scale=norm_own[:, t:t + 1])
                        nc.vector.tensor_add(
                            hout_st[:, (t % hblk) * d:(t % hblk + 1) * d],
                            hre[:], hp_sb[:, t * d:(t + 1) * d])
                        if t % hblk == hblk - 1 or t == TPC - 1:
                            hb0 = (t // hblk) * hblk
                            nbt = t - hb0 + 1
                            nc.sync.dma_start(
                                h_ext[:, hb0 * d:(hb0 + nbt) * d],
                                hout_st[:, :nbt * d])
                bscope2.__exit__(None, None, None)

    nc.finalize()
    return nc


def make_cfg(src, dst, kcut, cores=8):
    cfg = Cfg(kcut=kcut, cores=cores)
    rowid = (src.astype(np.int64) % 128) * NT + (src.astype(np.int64) // 128)
    is_h = rowid >= cfg.lsplit
    core_of = dst // cfg.own
    tpc = cfg.tpc
    cntL = np.zeros((cores, tpc), np.int64)
    cntH = np.zeros((cores, tpc), np.int64)
    for c in range(cores):
        sel = core_of == c
        ih, dd = is_h[sel], dst[sel]
        t_all = (dd.astype(np.int64) - c * cfg.own) // 128
        cntL[c] = np.bincount(t_all[~ih], minlength=tpc)
        cntH[c] = np.bincount(t_all[ih], minlength=tpc)
    perms = [np.argsort(-(cntL[c] + cntH[c]), kind="stable")
             for c in range(cores)]
    sL = np.stack([cntL[c][perms[c]] for c in range(cores)])
    sH = np.stack([cntH[c][perms[c]] for c in range(cores)])
    cfg.nlk = tuple(max(1, int(np.ceil(sL[:, k].max() / 128)))
                    for k in range(tpc))
    cfg.nhk = tuple(max(1, int(np.ceil(sH[:, k].max() / 128)))
                    for k in range(tpc))
    cfg.perms = tuple(perms)
    NSG = 8
    sg_in = (cfg.nchunk * 8 + NSG - 1) // NSG
    cfg.cand_cols = NSG * min(64, sg_in)
    return cfg


def make_inputs(cfg: Cfg, features, W, src, dst):
    nreal = features.shape[0]
    featp = np.zeros((cfg.nodes, cfg.d), np.float32)
    featp[:nreal] = features
    feat_t = np.ascontiguousarray(
        featp.reshape(NT, 128, cfg.d).transpose(1, 0, 2).reshape(128, -1))
    deg = np.bincount(dst, minlength=cfg.nodes).astype(np.float32)
    degall = np.ascontiguousarray(deg.reshape(NT, 128).T)
    wTc = np.ascontiguousarray(W.T).astype(np.float32)
    percore = host_prep(cfg, src, dst)
    in_maps = []
    for c in range(cfg.cores):
        base = c * cfg.own
        perm = np.asarray(cfg.perms[c])
        degp = np.ascontiguousarray(
            deg[base:base + cfg.own].reshape(cfg.tpc, 128)[perm].T)
        hpre_t = np.ascontiguousarray(
            featp[base:base + cfg.own].reshape(cfg.tpc, 128, cfg.d)[perm]
            .transpose(1, 0, 2).reshape(128, -1))
        pc = percore[c]
        in_maps.append(dict(
            feat=feat_t, wT=wTc, hpre=hpre_t, degp=degp, degall=degall,
            gidx=pc["gidx"], dstloc=pc["dstloc"]))
    return in_maps


def run(cfg: Cfg, features, W, src, dst):
    in_maps = make_inputs(cfg, features, W, src, dst)
    nc = build_nc(cfg)
    r = run_bass_kernel_spmd(nc, in_maps, core_ids=list(range(cfg.cores)))
    hs = []
    for c in range(cfg.cores):
        hp = r.results[c]["h"]
        tkp = hp.reshape(128, cfg.tpc, cfg.d).transpose(1, 0, 2)
        unp = np.empty_like(tkp)
        unp[np.asarray(cfg.perms[c])] = tkp
        hs.append(unp.reshape(cfg.own, cfg.d))
    h = np.concatenate(hs, axis=0)
    return h[:features.shape[0]]


# ---------------- harness entry point ----------------
def kernel(features, W, src, dst):
    """Full inputs in, full output out. Edges sharded by dst range across
    8 NeuronCores; cosine cut threshold found exactly on-device via
    multi-round counting + candidate compaction + allgather."""
    src = np.asarray(src).astype(np.int32)
    dst = np.asarray(dst).astype(np.int32)
    features = np.asarray(features, dtype=np.float32)
    W = np.asarray(W, dtype=np.float32)
    kcut = int(src.shape[0] * 0.1)
    cfg = make_cfg(src, dst, kcut)
    return run(cfg, features, W, src, dst).astype(np.float32)


# revision 55
# speedup vs baseline: 1.0452x; 1.0452x over previous
"""GCN layer kernel for trn2: host prep + bass kernel builder + runner.

v2: fp16 tables (p-major row numbering), batched DMAs, fp8 sexp fed
directly to PE, fp16 DVE fast modes, grouped multi-tile gathers.
"""
import sys
sys.path.insert(0, '/opt/trn_rl_repo')
import numpy as np
import ml_dtypes
from dataclasses import dataclass

import concourse.bacc as bacc
import concourse.mybir as mybir
import concourse.tile as tile
from concourse.bass_utils import run_bass_kernel_spmd

F32 = mybir.dt.float32
F16 = mybir.dt.float16
I16 = mybir.dt.int16
U32 = mybir.dt.uint32
FP8 = mybir.dt.float8e4
AF = mybir.ActivationFunctionType
OP = mybir.AluOpType

NT = 392          # total node tiles (50176/128)


@dataclass
class Cfg:
    nodes: int = 50176
    lsplit: int = 32536       # 83*392; p-major row split (<=32768 for int16)
    cores: int = 8
    d: int = 128
    kcut: int = 0
    nlk: tuple = ()           # L chunks per tile-rank (len tpc)
    nhk: tuple = ()           # H chunks per tile-rank
    perms: tuple = ()         # per-core tile permutation (host side only)
    G: int = 5                # tiles per gather group
    slab: int = 8             # y chunks per PSUM slab
    bblk: int = 16            # build tiles per DMA block
    cand_cols: int = 256
    nrounds_local: int = 2
    phases: str = "full"      # "build", "A", "T", "full"
    coll: bool = True         # False: stub collectives (tlsim)

    @property
    def tpc(self):
        return self.nodes // 128 // self.cores

    @property
    def own(self):
        return self.nodes // self.cores

    @property
    def nchunk(self):
        return int(sum(self.nlk) + sum(self.nhk))

    @property
    def slots(self):
        return self.nchunk * 128

    @property
    def groups(self):
        gs = []
        t = 0
        while t < self.tpc:
            gs.append(list(range(t, min(t + self.G, self.tpc))))
            t += self.G
        return gs


def group_meta(cfg):
    """Per group: (gbase_chunk, sbase_slot, ltot, htot, loffs, hoffs)."""
    out = []
    gb = sb = 0
    for grp in cfg.groups:
        lo, ho = [], []
        lt = ht = 0
        for k in grp:
            lo.append(lt); lt += cfg.nlk[k]
        for k in grp:
            ho.append(ht); ht += cfg.nhk[k]
        out.append((gb, sb, lt, ht, lo, ho))
        gb += lt + ht
        sb += (lt + ht) * 128
    return out


def host_prep(cfg: Cfg, src, dst):
    """Per-core gidx/dstloc/sexp in grouped-slot order, p-major row ids."""
    rowid = (src.astype(np.int64) % 128) * NT + (src.astype(np.int64) // 128)
    is_h_all = rowid >= cfg.lsplit
    rloc_all = rowid - np.where(is_h_all, cfg.lsplit, 0)
    core_of = dst // cfg.own
    gm = group_meta(cfg)
    out = []
    for c in range(cfg.cores):
        sel = np.nonzero(core_of == c)[0]
        rl = rloc_all[sel]
        ih = is_h_all[sel]
        dloc = dst[sel].astype(np.int64) - c * cfg.own
        t_all = dloc // 128
        loc = dloc % 128
        gidx = np.zeros(cfg.slots, np.int64)
        dstloc = np.full(cfg.slots, -1.0, np.float32)
        order = np.lexsort((loc, ih, t_all))
        rl, t_all, loc, ih = rl[order], t_all[order], loc[order], ih[order]
        perm = cfg.perms[c]
        for gi_, grp in enumerate(cfg.groups):
            _, sbase, ltot, htot, lo, ho = gm[gi_]
            for i, k in enumerate(grp):
                tt = perm[k]
                for hs, budget, coff in ((0, cfg.nlk[k], lo[i]),
                                         (1, cfg.nhk[k], ltot + ho[i])):
                    mm = (t_all == tt) & (ih == hs)
                    n = int(mm.sum())
                    assert n <= budget * 128, \
                        f"c{c} k{k} hs{hs}: {n}>{budget*128}"
                    gi = np.zeros(budget * 128, np.int64)
                    gi[:n] = rl[mm]
                    dl = np.full(budget * 128, -1.0, np.float32)
                    dl[:n] = loc[mm].astype(np.float32)
                    s0 = sbase + coff * 128
                    gidx[s0:s0 + budget * 128] = gi
                    dstloc[s0:s0 + budget * 128] = dl
        gidx_w = np.ascontiguousarray(
            np.tile(gidx.astype(np.int16).reshape(-1, 16).T, (8, 1)))
        dstloc_pc = np.ascontiguousarray(
            dstloc.reshape(cfg.nchunk, 128).T)
        out.append(dict(gidx=gidx_w, dstloc=dstloc_pc))
    return out


def build_nc(cfg: Cfg):
    nc = bacc.Bacc(None)
    d = cfg.d
    TPC, NCH = cfg.tpc, cfg.nchunk
    GM = group_meta(cfg)
    MAXL = max(m[2] for m in GM)
    MAXH = max(m[3] for m in GM)
    KCUT = float(cfg.kcut)
    LROWS, HROWS = cfg.lsplit, cfg.nodes - cfg.lsplit

    feat = nc.dram_tensor("feat", [128, NT * d], F32, kind="ExternalInput")
    wT = nc.dram_tensor("wT", [d, d], F32, kind="ExternalInput")
    hpre = nc.dram_tensor("hpre", [128, TPC * d], F32, kind="ExternalInput")
    degp = nc.dram_tensor("degp", [128, TPC], F32, kind="ExternalInput")
    degall = nc.dram_tensor("degall", [128, NT], F32, kind="ExternalInput")
    gidx_e = nc.dram_tensor("gidx", [128, cfg.slots // 16], I16,
                            kind="ExternalInput")
    dstloc_e = nc.dram_tensor("dstloc", [128, NCH], F32, kind="ExternalInput")
    h_ext = nc.dram_tensor("h", [128, TPC * d], F32, kind="ExternalOutput")
    ts_dbg = nc.dram_tensor("ts_dbg", [1, 4], F32, kind="ExternalOutput")

    cc_in = nc.dram_tensor("cc_in", [1, 32], F32)
    cc_out = nc.dram_tensor("cc_out", [1, 32], F32, addr_space="Shared")
    ag_in = nc.dram_tensor("ag_in", [16, cfg.cand_cols], F32)
    ag_out = nc.dram_tensor("ag_out", [cfg.cores, 16, cfg.cand_cols], F32,
                            addr_space="Shared")
    groups_rep = [list(range(cfg.cores))]

    with tile.TileContext(nc) as tc:
        with (tc.tile_pool(name="const", bufs=1) as cpool,
              tc.tile_pool(name="state", bufs=1) as spool,
              tc.tile_pool(name="dram", bufs=1, space="DRAM") as dpool,
              tc.tile_pool(name="bfeat", bufs=2) as bfpool,
              tc.tile_pool(name="bstage", bufs=2) as bspool,
              tc.tile_pool(name="gath", bufs=2) as gpool,
              tc.tile_pool(name="ysb", bufs=4) as ypool_sb,
              tc.tile_pool(name="hstage", bufs=2) as hpool,
              tc.tile_pool(name="ypsum", bufs=2, space="PSUM") as ypool,
              tc.tile_pool(name="apsum", bufs=2, space="PSUM") as apool,
              tc.tile_pool(name="tpsum", bufs=2, space="PSUM") as tppool,
              tc.tile_pool(name="misc", bufs=3) as mpool,
              tc.tile_pool(name="y16p", bufs=2) as y16pool,
              tc.tile_pool(name="thr", bufs=1) as tpool):

            # ---------- DRAM tables (fp16, contiguous p-major rows;
            # pad slots gather garbage row 0 -- harmless, sexp col is 0) ----
            nhT = dpool.tile([cfg.nodes, d], F16, tag="nhT")
            ftT = dpool.tile([cfg.nodes, d], F16, tag="ftT")

            # ---------- constants / inputs ----------
            iota_row = cpool.tile([128, 128], F32, tag="iota_row")
            nc.gpsimd.iota(iota_row[:], pattern=[[1, 128]], base=0,
                           channel_multiplier=0,
                           allow_small_or_imprecise_dtypes=True)
            iota16 = cpool.tile([128, 128], F16, tag="iota16")
            nc.vector.tensor_copy(iota16[:], iota_row[:])
            iota_col = cpool.tile([128, 1], F32, tag="iota_col")
            nc.gpsimd.iota(iota_col[:], pattern=[[1, 1]], base=0,
                           channel_multiplier=1,
                           allow_small_or_imprecise_dtypes=True)
            ones_col = cpool.tile([128, 1], F32, tag="ones_col")
            nc.vector.memset(ones_col[:], 1.0)
            ones_row = cpool.tile([1, 128], F32, tag="ones_row")
            nc.vector.memset(ones_row[:], 1.0)
            degall_sb = spool.tile([128, NT], F32, tag="degall")
            nc.sync.dma_start(degall_sb[:], degall[:])
            norm_all = spool.tile([128, NT], F32, tag="norm_all")
            nc.vector.tensor_scalar_max(norm_all[:], degall_sb[:], 1.0)
            nc.scalar.activation(norm_all[:], norm_all[:], AF.Sqrt)
            nc.vector.reciprocal(norm_all[:], norm_all[:])


            # ---------- table build (blocked) ----------
            bscope = nc.named_scope("pbuild")
            bscope.__enter__()
            nb = cfg.bblk
            for b0 in range(0, NT, nb):
                bt = min(nb, NT - b0)
                fblk = bfpool.tile([128, nb * d], F32, tag="fblk")
                nc.sync.dma_start(fblk[:, :bt * d],
                                  feat[:, b0 * d:(b0 + bt) * d])
                bss = mpool.tile([128, nb], F32, tag="bss")
                for ti in range(bt):
                    scr = mpool.tile([128, d], F32, tag="bsq")
                    nc.vector.scalar_tensor_tensor(
                        scr[:], fblk[:, ti * d:(ti + 1) * d], 1.0,
                        fblk[:, ti * d:(ti + 1) * d],
                        op0=OP.mult, op1=OP.mult,
                        accum_out=bss[:, ti:ti + 1])
                nc.vector.tensor_scalar_max(bss[:, :bt], bss[:, :bt], 1e-24)
                nc.scalar.activation(bss[:, :bt], bss[:, :bt], AF.Sqrt)
                nc.vector.reciprocal(bss[:, :bt], bss[:, :bt])
                nh_st = bspool.tile([128, nb * d], F16, tag="nh_st")
                ft_st = bspool.tile([128, nb * d], F16, tag="ft_st")
                for ti in range(bt):
                    nc.vector.tensor_scalar_mul(
                        nh_st[:, ti * d:(ti + 1) * d],
                        fblk[:, ti * d:(ti + 1) * d], bss[:, ti:ti + 1])
                    nc.scalar.activation(
                        ft_st[:, ti * d:(ti + 1) * d],
                        fblk[:, ti * d:(ti + 1) * d], AF.Copy,
                        scale=norm_all[:, b0 + ti:b0 + ti + 1])
                for tab, st in ((nhT, nh_st), (ftT, ft_st)):
                    nc.sync.dma_start(
                        tab[:]
                        .rearrange("(p t) x -> p t x", t=NT)[:, b0:b0 + bt, :],
                        st[:, :bt * d]
                        .rearrange("p (t x) -> p t x", x=d))
            bscope.__exit__(None, None, None)
            # ---- A/B-phase inputs + nhiT, emitted late so the build
            # loop's DMA stream starts immediately ----
            wT_sb = cpool.tile([d, d], F32, tag="wT")
            nc.sync.dma_start(wT_sb[:], wT[:])
            wT16 = cpool.tile([d, d], F16, tag="wT16")
            nc.vector.tensor_copy(wT16[:], wT_sb[:])
            hp_sb = spool.tile([128, TPC * d], F32, tag="hp")
            nc.sync.dma_start(hp_sb[:], hpre[:])
            degp_sb = spool.tile([128, TPC], F32, tag="degp")
            nc.sync.dma_start(degp_sb[:], degp[:])
            gidx_sb = spool.tile([128, cfg.slots // 16], I16, tag="gidx")
            nc.sync.dma_start(gidx_sb[:], gidx_e[:])
            dstloc_sb = spool.tile([128, NCH], F32, tag="dstloc")
            nc.sync.dma_start(dstloc_sb[:], dstloc_e[:])
            norm_own = spool.tile([128, TPC], F32, tag="norm_own")
            nc.vector.tensor_scalar_max(norm_own[:], degp_sb[:], 1.0)
            nc.scalar.activation(norm_own[:], norm_own[:], AF.Sqrt)
            nc.vector.reciprocal(norm_own[:], norm_own[:])
            # nhi_all: normalized own-tile rows (fp16), for phase-A rhs
            ssq = spool.tile([128, TPC], F32, tag="ssq")
            for t in range(TPC):
                scr = mpool.tile([128, d], F32, tag="sqscr")
                nc.scalar.activation(scr[:], hp_sb[:, t * d:(t + 1) * d],
                                     AF.Square, accum_out=ssq[:, t:t + 1])
            invl_own = spool.tile([128, TPC], F32, tag="invl_own")
            # floor 1e-8 (not 1e-24): invl <= 1e4 stays fp16-finite in dmat;
            # zero-feature pad rows still give nhi = 0 * 1e4 = 0 exactly
            nc.vector.tensor_scalar_max(ssq[:], ssq[:], 1e-8)
            nc.scalar.activation(invl_own[:], ssq[:], AF.Sqrt)
            nc.vector.reciprocal(invl_own[:], invl_own[:])
            # nhiT_all[:, t*d+j] = nhi_t[j, :] transposed, via hp^T @ D
            ident32 = cpool.tile([128, 128], F32, tag="ident32")
            nc.vector.tensor_scalar(ident32[:], iota_row[:], iota_col[:],
                                    None, op0=OP.is_equal)
            nhiT_all = spool.tile([128, TPC * d], F16, tag="nhiT_all")
            for t in range(TPC):
                dmat = mpool.tile([128, d], F32, tag="dmat")
                nc.vector.tensor_scalar_mul(dmat[:], ident32[:],
                                            invl_own[:, t:t + 1])
                ntp = apool.tile([128, d], F32, tag="ab")
                nc.tensor.matmul(ntp[:], hp_sb[:, t * d:(t + 1) * d],
                                 dmat[:], start=True, stop=True)
                nc.scalar.copy(nhiT_all[:, t * d:(t + 1) * d], ntp[:])


            def emit_b_gather(gi_):
                _, sbase, ltot, htot, _, _ = GM[gi_]
                xfl = gpool.tile([128, MAXL, d], F16, tag="xgl")
                xfh = gpool.tile([128, MAXH, d], F16, tag="xgh")
                i0 = sbase // 16
                nc.gpsimd.dma_gather(
                    out_ap=xfl[:, :ltot, :], in_ap=ftT[0:LROWS, :],
                    idxs_ap=gidx_sb[:, i0:i0 + ltot * 8],
                    num_idxs=ltot * 128, num_idxs_reg=ltot * 128,
                    elem_size=d, single_packet=False)
                nc.gpsimd.dma_gather(
                    out_ap=xfh[:, :htot, :], in_ap=ftT[LROWS:cfg.nodes, :],
                    idxs_ap=gidx_sb[:, i0 + ltot * 8:i0 + (ltot + htot) * 8],
                    num_idxs=htot * 128, num_idxs_reg=htot * 128,
                    elem_size=d, single_packet=False)
                return xfl, xfh

            run_a = cfg.phases in ("A", "T", "full")
            run_t = cfg.phases in ("T", "full")
            run_b = cfg.phases == "full"
            cos_sb = spool.tile([128, NCH], F32, tag="cos")
            if not run_a:
                nc.vector.memset(cos_sb[:, :1], 0.0)

            # ---------- Phase A: cos ----------
            if run_a:
                ascope = nc.named_scope("pcos")
                ascope.__enter__()
                for gi_, grp in enumerate(cfg.groups):
                    gbase, sbase, ltot, htot, lo, ho = GM[gi_]
                    xgl = gpool.tile([128, MAXL * 128], F16, tag="xgl")
                    xgh = gpool.tile([128, MAXH * 128], F16, tag="xgh")
                    i0 = sbase // 16
                    nc.gpsimd.dma_gather(
                        out_ap=xgl[:, :ltot * 128]
                        .rearrange("p (o n) -> p o n", o=1),
                        in_ap=nhT[0:LROWS, :],
                        idxs_ap=gidx_sb[:, i0:i0 + ltot * 8],
                        num_idxs=ltot * 128, num_idxs_reg=ltot * 128,
                        elem_size=d, transpose=True, single_packet=False)
                    nc.gpsimd.dma_gather(
                        out_ap=xgh[:, :htot * 128]
                        .rearrange("p (o n) -> p o n", o=1),
                        in_ap=nhT[LROWS:cfg.nodes, :],
                        idxs_ap=gidx_sb[:, i0 + ltot * 8:
                                        i0 + (ltot + htot) * 8],
                        num_idxs=htot * 128, num_idxs_reg=htot * 128,
                        elem_size=d, transpose=True, single_packet=False)
                    for i, t in enumerate(grp):
                        nlt, nht = cfg.nlk[t], cfg.nhk[t]
                        nct_k = nlt + nht
                        for s0 in range(0, nct_k, cfg.slab):
                            sn = min(cfg.slab, nct_k - s0)
                            c2_ps = ypool.tile([128, cfg.slab * d], F32,
                                               tag="y")
                            for j in range(sn):
                                k = s0 + j
                                if k < nlt:
                                    xT_ap = xgl[:, (lo[i] + k) * 128:
                                                (lo[i] + k) * 128 + 128]
                                else:
                                    hc = ho[i] + (k - nlt)
                                    xT_ap = xgh[:, hc * 128:hc * 128 + 128]
                                nc.tensor.matmul(
                                    c2_ps[:, j * d:(j + 1) * d],
                                    xT_ap,
                                    nhiT_all[:, t * d:(t + 1) * d],
                                    start=True, stop=True)
                            c2_sb = ypool_sb.tile([128, cfg.slab * d], F16,
                                                  tag="ysb")
                            nc.scalar.copy(c2_sb[:, :sn * d],
                                           c2_ps[:, :sn * d])
                            for j in range(sn):
                                k = s0 + j
                                ccg = (lo[i] + k if k < nlt
                                       else ltot + ho[i] + (k - nlt))
                                cc = gbase + ccg
                                scr = mpool.tile([128, d], F16, tag="cscr")
                                nc.vector.scalar_tensor_tensor(
                                    scr[:], iota16[:],
                                    dstloc_sb[:, cc:cc + 1],
                                    c2_sb[:, j * d:(j + 1) * d],
                                    op0=OP.is_equal, op1=OP.mult,
                                    accum_out=cos_sb[:, cc:cc + 1])
                ascope.__exit__(None, None, None)

            # prefetch first B gather groups (overlap with threshold phase)
            pf = []
            if run_b:
                for gi in range(min(2, len(cfg.groups))):
                    pf.append(emit_b_gather(gi))

            # ---------- Phase T: threshold ----------
            if run_t:
                tscope = nc.named_scope("pthr")
                tscope.__enter__()
                lo_t = tpool.tile([1, 1], F32, tag="lo")
                th_row = tpool.tile([1, 32], F32, tag="throw")
                th_bc = tpool.tile([128, 32], F32, tag="thbc")
                cnt128 = tpool.tile([128, 32], F32, tag="cnt128")
                gcnt = tpool.tile([1, 32], F32, tag="gcnt")
                srow = tpool.tile([1, 1], F32, tag="srow")
                cbase = tpool.tile([1, 1], F32, tag="cbase")
                iota32 = tpool.tile([1, 32], F32, tag="iota32")
                nc.vector.tensor_copy(iota32[:], iota_row[:1, :32])
                msk = tpool.tile([1, 32], F32, tag="msk")
                msct = tpool.tile([1, 32], F32, tag="msct")
                cscr2 = tpool.tile([128, NCH], F32, tag="cscr2")
                cand = tpool.tile([128, cfg.cand_cols], F32, tag="cand")
                nc.vector.memset(cbase[:], 0.0)
                nc.vector.memset(lo_t[:], -0.75)

                def emit_round(vals_ap, ncols, w_bin, mode, shift4):
                    nc.vector.tensor_scalar_mul(th_row[:], iota32[:], w_bin)
                    nc.vector.tensor_scalar(th_row[:], th_row[:], lo_t[:],
                                            None, op0=OP.add)
                    if shift4:
                        nc.vector.tensor_scalar_add(th_row[:], th_row[:], 4.0)
                    ps = tppool.tile([128, 32], F32, tag="tiny")
                    nc.tensor.matmul(ps[:], ones_row[:], th_row[:],
                                     start=True, stop=True)
                    nc.vector.tensor_copy(th_bc[:], ps[:])
                    for j in range(32):
                        nc.vector.tensor_scalar(
                            cscr2[:, :ncols], vals_ap, th_bc[:, j:j + 1],
                            None, op0=OP.is_lt, op1=OP.add,
                            accum_out=cnt128[:, j:j + 1])
                    cps = tppool.tile([1, 32], F32, tag="tiny")
                    nc.tensor.matmul(cps[:], ones_col[:], cnt128[:],
                                     start=True, stop=True)
                    nc.vector.tensor_copy(gcnt[:], cps[:])
                    if mode.startswith("global"):
                        if cfg.coll:
                            nc.sync.dma_start(cc_in[:], gcnt[:])
                            nc.gpsimd.collective_compute(
                                "AllReduce", OP.add,
                                replica_groups=groups_rep,
                                ins=[cc_in[:]], outs=[cc_out[:]])
                            nc.sync.dma_start(gcnt[:], cc_out[:])
                        else:
                            nc.sync.dma_start(cc_in[:], gcnt[:])
                            nc.sync.dma_start(gcnt[:], cc_in[:])
                    nc.vector.tensor_scalar(
                        msct[:], gcnt[:], cbase[:], KCUT - 0.5,
                        op0=OP.add, op1=OP.is_lt)
                    nc.vector.tensor_scalar(
                        msct[:], msct[:], 0.0, None,
                        op0=OP.add, op1=OP.add, accum_out=srow[:])
                    nc.vector.tensor_scalar(srow[:], srow[:], -1.0, 0.0,
                                            op0=OP.add, op1=OP.max)
                    if mode == "global2":
                        nc.vector.tensor_scalar(msk[:], iota32[:], srow[:],
                                                None, op0=OP.is_equal)
                        nc.vector.scalar_tensor_tensor(
                            msct[:], gcnt[:], 1.0, msk[:],
                            op0=OP.mult, op1=OP.mult, accum_out=cbase[:])
                    nc.vector.scalar_tensor_tensor(
                        lo_t[:], srow[:], w_bin, lo_t[:],
                        op0=OP.mult, op1=OP.add)

                W1 = 1.5 / 32
                W2 = 1.5 / 32 ** 2
                emit_round(cos_sb[:], NCH, W1, "global1", False)
                emit_round(cos_sb[:], NCH, W2, "global2", False)

                # compact in-bracket values, remapped to cos+4
                lo_bc = tpool.tile([128, 1], F32, tag="lobc")
                psb = tppool.tile([128, 1], F32, tag="tiny")
                nc.tensor.matmul(psb[:], ones_row[:], lo_t[:],
                                 start=True, stop=True)
                nc.vector.tensor_copy(lo_bc[:], psb[:])
                m1 = tpool.tile([128, NCH], F32, tag="m1")
                nc.vector.tensor_scalar(m1[:], cos_sb[:], lo_bc[:], None,
                                        op0=OP.is_ge)
                hi_bc = tpool.tile([128, 1], F32, tag="hibc")
                nc.vector.tensor_scalar_add(hi_bc[:], lo_bc[:], W2)
                m2 = tpool.tile([128, NCH], F32, tag="m2")
                nc.vector.tensor_scalar(m2[:], cos_sb[:], hi_bc[:], None,
                                        op0=OP.is_lt)
                nc.vector.tensor_mul(m1[:], m1[:], m2[:])
                c4 = tpool.tile([128, NCH], F32, tag="c4")
                nc.vector.tensor_scalar(c4[:], cos_sb[:], 5.0, None,
                                        op0=OP.add)
                nc.vector.tensor_mul(c4[:], c4[:], m1[:])
                nc.vector.tensor_scalar_add(c4[:], c4[:], -1.0)
                NSG = 8
                sg_out_cols = cfg.cand_cols // NSG
                sgc = tpool.tile([16, cfg.cand_cols], F32, tag="sgc")
                posi = tpool.tile([16, sg_out_cols], F32, tag="posi")
                nc.gpsimd.iota(posi[:], pattern=[[16, sg_out_cols]], base=0,
                               channel_multiplier=1,
                               allow_small_or_imprecise_dtypes=True)
                for sg_i in range(NSG):
                    y16s = y16pool.tile([16, NCH], F32, tag="y16s")
                    nc.sync.dma_start(y16s[:], c4[16 * sg_i:16 * (sg_i + 1), :])
                    sg_out = y16pool.tile([16, sg_out_cols], F32,
                                          tag="sgout")
                    nfound = y16pool.tile([1, 1], U32, tag="nfound")
                    nc.gpsimd.sparse_gather(sg_out[:], y16s[:],
                                            num_found=nfound[:])
                    nf_f = y16pool.tile([1, 1], F32, tag="nff")
                    nc.vector.tensor_copy(nf_f[:], nfound[:])
                    nf16 = y16pool.tile([16, 1], F32, tag="nf16")
                    ps16 = tppool.tile([16, 1], F32, tag="tiny")
                    nc.tensor.matmul(ps16[:], ones_row[:, :16], nf_f[:],
                                     start=True, stop=True)
                    nc.vector.tensor_copy(nf16[:], ps16[:])
                    mtail = y16pool.tile([16, sg_out_cols], F32,
                                         tag="mtail")
                    nc.vector.tensor_scalar(mtail[:], posi[:], nf16[:], None,
                                            op0=OP.is_lt)
                    big = y16pool.tile([16, sg_out_cols], F32, tag="big")
                    nc.vector.tensor_scalar(big[:], mtail[:], 0.5, 1e30,
                                            op0=OP.is_lt, op1=OP.mult)
                    nc.vector.tensor_mul(sg_out[:], sg_out[:], mtail[:])
                    nc.vector.tensor_add(
                        sgc[:, sg_i * sg_out_cols:(sg_i + 1) * sg_out_cols],
                        sg_out[:], big[:])
                nc.sync.dma_start(ag_in[:], sgc[:])
                if cfg.coll:
                    nc.gpsimd.collective_compute(
                        "AllGather", OP.bypass, replica_groups=groups_rep,
                        ins=[ag_in[:]], outs=[ag_out[:]])
                    for r in range(cfg.cores):
                        nc.sync.dma_start(cand[16 * r:16 * (r + 1), :],
                                          ag_out[r, :, :])
                else:
                    for r in range(cfg.cores):
                        nc.sync.dma_start(cand[16 * r:16 * (r + 1), :],
                                          ag_in[:])

                wr = W2
                for r in range(cfg.nrounds_local):
                    wr = wr / 32
                    emit_round(cand[:], cfg.cand_cols, wr, "local", True)
                nc.vector.tensor_scalar_add(lo_t[:], lo_t[:], wr)
                tstar = tpool.tile([128, 1], F32, tag="tstar")
                pst = tppool.tile([128, 1], F32, tag="tiny")
                nc.tensor.matmul(pst[:], ones_row[:], lo_t[:],
                                 start=True, stop=True)
                nc.vector.tensor_copy(tstar[:], pst[:])

                nc.sync.dma_start(ts_dbg[:, 0:1], lo_t[:])
                nc.sync.dma_start(ts_dbg[:, 1:2], cbase[:])
                nc.sync.dma_start(ts_dbg[:, 2:3], srow[:])
                nc.sync.dma_start(ts_dbg[:, 3:4], nf_f[:])
                # dm = keep*(dstloc+1) - 1  (keep = cos >= t*)
                keep = tpool.tile([128, NCH], F32, tag="m1")
                nc.vector.tensor_scalar(keep[:], cos_sb[:], tstar[:], None,
                                        op0=OP.is_ge)
                dm = tpool.tile([128, NCH], F32, tag="c4")
                nc.vector.tensor_scalar_add(dm[:], dstloc_sb[:], 1.0)
                nc.vector.tensor_mul(dm[:], dm[:], keep[:])
                nc.vector.tensor_scalar_add(dm[:], dm[:], -1.0)
                tscope.__exit__(None, None, None)

            # ---------- Phase B: aggregate + linear + tail ----------
            if run_b:
                bscope2 = nc.named_scope("pagg")
                bscope2.__enter__()
                hblk = 8
                hout_st = None
                for gi, grp in enumerate(cfg.groups):
                    gbase, sbase, ltot, htot, lo, ho = GM[gi]
                    if gi < len(pf):
                        xfl, xfh = pf[gi]
                    else:
                        xfl, xfh = emit_b_gather(gi)
                    for i, t in enumerate(grp):
                        nlt, nht = cfg.nlk[t], cfg.nhk[t]
                        nct_k = nlt + nht
                        if t % hblk == 0:
                            hout_st = hpool.tile([128, hblk * d], F32,
                                                 tag="hout")
                        at_ps = apool.tile([128, 128], F32, tag="ab")
                        for k in range(nct_k):
                            if k < nlt:
                                cc = gbase + lo[i] + k
                                x_ap = xfl[:, lo[i] + k, :]
                            else:
                                cc = gbase + ltot + ho[i] + (k - nlt)
                                x_ap = xfh[:, ho[i] + (k - nlt), :]
                            sa = mpool.tile([128, 128], F16, tag="sa")
                            nc.vector.tensor_scalar(sa[:], iota16[:],
                                                    dm[:, cc:cc + 1], None,
                                                    op0=OP.is_equal)
                            nc.tensor.matmul(at_ps[:], x_ap, sa[:],
                                             start=(k == 0),
                                             stop=(k == nct_k - 1))
                        at_sb = mpool.tile([128, 128], F16, tag="aggTsb")
                        nc.scalar.copy(at_sb[:], at_ps[:])
                        h_ps = apool.tile([128, d], F32, tag="ab")
                        nc.tensor.matmul(h_ps[:], at_sb[:], wT16[:],
                                         start=True, stop=True)
                        hre = mpool.tile([128, d], F32, tag="hre")
                        nc.scalar.activation(hre[:], h_ps[:], AF.Relu,
                                             scale=norm_own[:, t:t + 1])
                        nc.vector.tensor_add(
                            hout_st[:, (t % hblk) * d:(t % hblk + 1) * d],
                            hre[:], hp_sb[:, t * d:(t + 1) * d])
                        if t % hblk == hblk - 1 or t == TPC - 1:
                            hb0 = (t // hblk) * hblk
                            nbt = t - hb0 + 1
                            nc.sync.dma_start(
                                h_ext[:, hb0 * d:(hb0 + nbt) * d],
                                hout_st[:, :nbt * d])
                bscope2.__exit__(None, None, None)

    nc.finalize()
    return nc


def make_cfg(src, dst, kcut, cores=8):
    cfg = Cfg(kcut=kcut, cores=cores)
    rowid = (src.astype(np.int64) % 128) * NT + (src.astype(np.int64) // 128)
    is_h = rowid >= cfg.lsplit
    core_of = dst // cfg.own
    tpc = cfg.tpc
    cntL = np.zeros((cores, tpc), np.int64)
    cntH = np.zeros((cores, tpc), np.int64)
    for c in range(cores):
        sel = core_of == c
        ih, dd = is_h[sel], dst[sel]
        t_all = (dd.astype(np.int64) - c * cfg.own) // 128
        cntL[c] = np.bincount(t_all[~ih], minlength=tpc)
        cntH[c] = np.bincount(t_all[ih], minlength=tpc)
    perms = [np.argsort(-(cntL[c] + cntH[c]), kind="stable")
             for c in range(cores)]
    sL = np.stack([cntL[c][perms[c]] for c in range(cores)])
    sH = np.stack([cntH[c][perms[c]] for c in range(cores)])
    cfg.nlk = tuple(max(1, int(np.ceil(sL[:, k].max() / 128)))
                    for k in range(tpc))
    cfg.nhk = tuple(max(1, int(np.ceil(sH[:, k].max() / 128)))
                    for k in range(tpc))
    cfg.perms = tuple(perms)
    NSG = 8
    sg_in = (cfg.nchunk * 8 + NSG - 1) // NSG
    cfg.cand_cols = NSG * min(64, sg_in)
    return cfg


def make_inputs(cfg: Cfg, features, W, src, dst):
    nreal = features.shape[0]
    featp = np.zeros((cfg.nodes, cfg.d), np.float32)
    featp[:nreal] = features
    feat_t = np.ascontiguousarray(
        featp.reshape(NT, 128, cfg.d).transpose(1, 0, 2).reshape(128, -1))
    deg = np.bincount(dst, minlength=cfg.nodes).astype(np.float32)
    degall = np.ascontiguousarray(deg.reshape(NT, 128).T)
    wTc = np.ascontiguousarray(W.T).astype(np.float32)
    percore = host_prep(cfg, src, dst)
    in_maps = []
    for c in range(cfg.cores):
        base = c * cfg.own
        perm = np.asarray(cfg.perms[c])
        degp = np.ascontiguousarray(
            deg[base:base + cfg.own].reshape(cfg.tpc, 128)[perm].T)
        hpre_t = np.ascontiguousarray(
            featp[base:base + cfg.own].reshape(cfg.tpc, 128, cfg.d)[perm]
            .transpose(1, 0, 2).reshape(128, -1))
        pc = percore[c]
        in_maps.append(dict(
            feat=feat_t, wT=wTc, hpre=hpre_t, degp=degp, degall=degall,
            gidx=pc["gidx"], dstloc=pc["dstloc"]))
    return in_maps


def run(cfg: Cfg, features, W, src, dst):
    in_maps = make_inputs(cfg, features, W, src, dst)
    nc = build_nc(cfg)
    r = run_bass_kernel_spmd(nc, in_maps, core_ids=list(range(cfg.cores)))
    hs = []
    for c in range(cfg.cores):
        hp = r.results[c]["h"]
        tkp = hp.reshape(128, cfg.tpc, cfg.d).transpose(1, 0, 2)
        unp = np.empty_like(tkp)
        unp[np.asarray(cfg.perms[c])] = tkp
        hs.append(unp.reshape(cfg.own, cfg.d))
    h = np.concatenate(hs, axis=0)
    return h[:features.shape[0]]


# ---------------- harness entry point ----------------
def kernel(features, W, src, dst):
    """Full inputs in, full output out. Edges sharded by dst range across
    8 NeuronCores; cosine cut threshold found exactly on-device via
    multi-round counting + candidate compaction + allgather."""
    src = np.asarray(src).astype(np.int32)
    dst = np.asarray(dst).astype(np.int32)
    features = np.asarray(features, dtype=np.float32)
    W = np.asarray(W, dtype=np.float32)
    kcut = int(src.shape[0] * 0.1)
    cfg = make_cfg(src, dst, kcut)
    return run(cfg, features, W, src, dst).astype(np.float32)


# revision 62
# speedup vs baseline: 1.0532x; 1.0077x over previous
"""GCN layer kernel for trn2: host prep + bass kernel builder + runner.

v2: fp16 tables (p-major row numbering), batched DMAs, fp8 sexp fed
directly to PE, fp16 DVE fast modes, grouped multi-tile gathers.
"""
import sys
sys.path.insert(0, '/opt/trn_rl_repo')
import numpy as np
import ml_dtypes
from dataclasses import dataclass

import concourse.bacc as bacc
import concourse.mybir as mybir
import concourse.tile as tile
from concourse.bass_utils import run_bass_kernel_spmd

F32 = mybir.dt.float32
F16 = mybir.dt.float16
I16 = mybir.dt.int16
U32 = mybir.dt.uint32
FP8 = mybir.dt.float8e4
AF = mybir.ActivationFunctionType
OP = mybir.AluOpType

NT = 392          # total node tiles (50176/128)


@dataclass
class Cfg:
    nodes: int = 50176
    lsplit: int = 32536       # 83*392; p-major row split (<=32768 for int16)
    cores: int = 8
    d: int = 128
    kcut: int = 0
    nlk: tuple = ()           # L chunks per tile-rank (len tpc)
    nhk: tuple = ()           # H chunks per tile-rank
    perms: tuple = ()         # per-core tile permutation (host side only)
    G: int = 5                # tiles per gather group
    slab: int = 8             # y chunks per PSUM slab
    bblk: int = 20            # build tiles per DMA block
    cand_cols: int = 256
    nrounds_local: int = 2
    phases: str = "full"      # "build", "A", "T", "full"
    coll: bool = True         # False: stub collectives (tlsim)

    @property
    def tpc(self):
        return self.nodes // 128 // self.cores

    @property
    def own(self):
        return self.nodes // self.cores

    @property
    def nchunk(self):
        return int(sum(self.nlk) + sum(self.nhk))

    @property
    def slots(self):
        return self.nchunk * 128

    @property
    def groups(self):
        gs = []
        t = 0
        while t < self.tpc:
            gs.append(list(range(t, min(t + self.G, self.tpc))))
            t += self.G
        return gs


def group_meta(cfg):
    """Per group: (gbase_chunk, sbase_slot, ltot, htot, loffs, hoffs)."""
    out = []
    gb = sb = 0
    for grp in cfg.groups:
        lo, ho = [], []
        lt = ht = 0
        for k in grp:
            lo.append(lt); lt += cfg.nlk[k]
        for k in grp:
            ho.append(ht); ht += cfg.nhk[k]
        out.append((gb, sb, lt, ht, lo, ho))
        gb += lt + ht
        sb += (lt + ht) * 128
    return out


def host_prep(cfg: Cfg, src, dst):
    """Per-core gidx/dstloc/sexp in grouped-slot order, p-major row ids."""
    rowid = (src.astype(np.int64) % 128) * NT + (src.astype(np.int64) // 128)
    is_h_all = rowid >= cfg.lsplit
    rloc_all = rowid - np.where(is_h_all, cfg.lsplit, 0)
    core_of = dst // cfg.own
    gm = group_meta(cfg)
    out = []
    for c in range(cfg.cores):
        sel = np.nonzero(core_of == c)[0]
        rl = rloc_all[sel]
        ih = is_h_all[sel]
        dloc = dst[sel].astype(np.int64) - c * cfg.own
        t_all = dloc // 128
        loc = dloc % 128
        gidx = np.zeros(cfg.slots, np.int64)
        dstloc = np.full(cfg.slots, -1.0, np.float32)
        order = np.lexsort((loc, ih, t_all))
        rl, t_all, loc, ih = rl[order], t_all[order], loc[order], ih[order]
        perm = cfg.perms[c]
        for gi_, grp in enumerate(cfg.groups):
            _, sbase, ltot, htot, lo, ho = gm[gi_]
            for i, k in enumerate(grp):
                tt = perm[k]
                for hs, budget, coff in ((0, cfg.nlk[k], lo[i]),
                                         (1, cfg.nhk[k], ltot + ho[i])):
                    mm = (t_all == tt) & (ih == hs)
                    n = int(mm.sum())
                    assert n <= budget * 128, \
                        f"c{c} k{k} hs{hs}: {n}>{budget*128}"
                    gi = np.zeros(budget * 128, np.int64)
                    gi[:n] = rl[mm]
                    dl = np.full(budget * 128, -1.0, np.float32)
                    dl[:n] = loc[mm].astype(np.float32)
                    s0 = sbase + coff * 128
                    gidx[s0:s0 + budget * 128] = gi
                    dstloc[s0:s0 + budget * 128] = dl
        gidx_w = np.ascontiguousarray(
            np.tile(gidx.astype(np.int16).reshape(-1, 16).T, (8, 1)))
        dstloc_pc = np.ascontiguousarray(
            dstloc.reshape(cfg.nchunk, 128).T)
        out.append(dict(gidx=gidx_w, dstloc=dstloc_pc))
    return out


def build_nc(cfg: Cfg):
    nc = bacc.Bacc(None)
    d = cfg.d
    TPC, NCH = cfg.tpc, cfg.nchunk
    GM = group_meta(cfg)
    MAXL = max(m[2] for m in GM)
    MAXH = max(m[3] for m in GM)
    KCUT = float(cfg.kcut)
    LROWS, HROWS = cfg.lsplit, cfg.nodes - cfg.lsplit

    feat = nc.dram_tensor("feat", [128, NT * d], F32, kind="ExternalInput")
    wT = nc.dram_tensor("wT", [d, d], F32, kind="ExternalInput")
    hpre = nc.dram_tensor("hpre", [128, TPC * d], F32, kind="ExternalInput")
    degp = nc.dram_tensor("degp", [128, TPC], F32, kind="ExternalInput")
    degall = nc.dram_tensor("degall", [128, NT], F32, kind="ExternalInput")
    gidx_e = nc.dram_tensor("gidx", [128, cfg.slots // 16], I16,
                            kind="ExternalInput")
    dstloc_e = nc.dram_tensor("dstloc", [128, NCH], F32, kind="ExternalInput")
    h_ext = nc.dram_tensor("h", [128, TPC * d], F32, kind="ExternalOutput")
    ts_dbg = nc.dram_tensor("ts_dbg", [1, 4], F32, kind="ExternalOutput")

    cc_in = nc.dram_tensor("cc_in", [1, 32], F32)
    cc_out = nc.dram_tensor("cc_out", [1, 32], F32, addr_space="Shared")
    ag_in = nc.dram_tensor("ag_in", [16, cfg.cand_cols], F32)
    ag_out = nc.dram_tensor("ag_out", [cfg.cores, 16, cfg.cand_cols], F32,
                            addr_space="Shared")
    groups_rep = [list(range(cfg.cores))]

    with tile.TileContext(nc) as tc:
        with (tc.tile_pool(name="const", bufs=1) as cpool,
              tc.tile_pool(name="state", bufs=1) as spool,
              tc.tile_pool(name="dram", bufs=1, space="DRAM") as dpool,
              tc.tile_pool(name="bfeat", bufs=2) as bfpool,
              tc.tile_pool(name="bstage", bufs=2) as bspool,
              tc.tile_pool(name="gath", bufs=2) as gpool,
              tc.tile_pool(name="ysb", bufs=4) as ypool_sb,
              tc.tile_pool(name="hstage", bufs=2) as hpool,
              tc.tile_pool(name="ypsum", bufs=2, space="PSUM") as ypool,
              tc.tile_pool(name="apsum", bufs=2, space="PSUM") as apool,
              tc.tile_pool(name="tpsum", bufs=2, space="PSUM") as tppool,
              tc.tile_pool(name="misc", bufs=3) as mpool,
              tc.tile_pool(name="y16p", bufs=2) as y16pool,
              tc.tile_pool(name="thr", bufs=1) as tpool):

            # ---------- DRAM tables (fp16, contiguous p-major rows;
            # pad slots gather garbage row 0 -- harmless, sexp col is 0) ----
            nhT = dpool.tile([cfg.nodes, d], F16, tag="nhT")
            ftT = dpool.tile([cfg.nodes, d], F16, tag="ftT")

            # ---------- constants / inputs ----------
            iota_row = cpool.tile([128, 128], F32, tag="iota_row")
            nc.gpsimd.iota(iota_row[:], pattern=[[1, 128]], base=0,
                           channel_multiplier=0,
                           allow_small_or_imprecise_dtypes=True)
            iota16 = cpool.tile([128, 128], F16, tag="iota16")
            nc.vector.tensor_copy(iota16[:], iota_row[:])
            iota_col = cpool.tile([128, 1], F32, tag="iota_col")
            nc.gpsimd.iota(iota_col[:], pattern=[[1, 1]], base=0,
                           channel_multiplier=1,
                           allow_small_or_imprecise_dtypes=True)
            ones_col = cpool.tile([128, 1], F32, tag="ones_col")
            nc.vector.memset(ones_col[:], 1.0)
            ones_row = cpool.tile([1, 128], F32, tag="ones_row")
            nc.vector.memset(ones_row[:], 1.0)
            degall_sb = spool.tile([128, NT], F32, tag="degall")
            nc.sync.dma_start(degall_sb[:], degall[:])
            norm_all = spool.tile([128, NT], F32, tag="norm_all")
            nc.vector.tensor_scalar_max(norm_all[:], degall_sb[:], 1.0)
            nc.scalar.activation(norm_all[:], norm_all[:], AF.Sqrt)
            nc.vector.reciprocal(norm_all[:], norm_all[:])


            # ---------- table build (blocked) ----------
            bscope = nc.named_scope("pbuild")
            bscope.__enter__()
            nb = cfg.bblk
            for b0 in range(0, NT, nb):
                bt = min(nb, NT - b0)
                fblk = bfpool.tile([128, nb * d], F32, tag="fblk")
                nc.sync.dma_start(fblk[:, :bt * d],
                                  feat[:, b0 * d:(b0 + bt) * d])
                bss = mpool.tile([128, nb], F32, tag="bss")
                for ti in range(bt):
                    scr = mpool.tile([128, d], F32, tag="bsq")
                    nc.vector.scalar_tensor_tensor(
                        scr[:], fblk[:, ti * d:(ti + 1) * d], 1.0,
                        fblk[:, ti * d:(ti + 1) * d],
                        op0=OP.mult, op1=OP.mult,
                        accum_out=bss[:, ti:ti + 1])
                nc.vector.tensor_scalar_max(bss[:, :bt], bss[:, :bt], 1e-24)
                nc.scalar.activation(bss[:, :bt], bss[:, :bt], AF.Sqrt)
                nc.vector.reciprocal(bss[:, :bt], bss[:, :bt])
                nh_st = bspool.tile([128, nb * d], F16, tag="nh_st")
                ft_st = bspool.tile([128, nb * d], F16, tag="ft_st")
                for ti in range(bt):
                    nc.vector.tensor_scalar_mul(
                        nh_st[:, ti * d:(ti + 1) * d],
                        fblk[:, ti * d:(ti + 1) * d], bss[:, ti:ti + 1])
                    nc.scalar.activation(
                        ft_st[:, ti * d:(ti + 1) * d],
                        fblk[:, ti * d:(ti + 1) * d], AF.Copy,
                        scale=norm_all[:, b0 + ti:b0 + ti + 1])
                for tab, st in ((nhT, nh_st), (ftT, ft_st)):
                    nc.sync.dma_start(
                        tab[:]
                        .rearrange("(p t) x -> p t x", t=NT)[:, b0:b0 + bt, :],
                        st[:, :bt * d]
                        .rearrange("p (t x) -> p t x", x=d))
            bscope.__exit__(None, None, None)
            # ---- A/B-phase inputs + nhiT, emitted late so the build
            # loop's DMA stream starts immediately ----
            gidx_sb = spool.tile([128, cfg.slots // 16], I16, tag="gidx")
            nc.sync.dma_start(gidx_sb[:], gidx_e[:])
            dstloc_sb = spool.tile([128, NCH], F32, tag="dstloc")
            nc.sync.dma_start(dstloc_sb[:], dstloc_e[:])
            hp_sb = spool.tile([128, TPC * d], F32, tag="hp")
            nc.sync.dma_start(hp_sb[:], hpre[:])
            wT_sb = cpool.tile([d, d], F32, tag="wT")
            nc.sync.dma_start(wT_sb[:], wT[:])
            wT16 = cpool.tile([d, d], F16, tag="wT16")
            nc.vector.tensor_copy(wT16[:], wT_sb[:])
            degp_sb = spool.tile([128, TPC], F32, tag="degp")
            nc.sync.dma_start(degp_sb[:], degp[:])
            norm_own = spool.tile([128, TPC], F32, tag="norm_own")
            nc.vector.tensor_scalar_max(norm_own[:], degp_sb[:], 1.0)
            nc.scalar.activation(norm_own[:], norm_own[:], AF.Sqrt)
            nc.vector.reciprocal(norm_own[:], norm_own[:])
            # nhi_all: normalized own-tile rows (fp16), for phase-A rhs
            ssq = spool.tile([128, TPC], F32, tag="ssq")
            for t in range(TPC):
                scr = mpool.tile([128, d], F32, tag="sqscr")
                nc.scalar.activation(scr[:], hp_sb[:, t * d:(t + 1) * d],
                                     AF.Square, accum_out=ssq[:, t:t + 1])
            invl_own = spool.tile([128, TPC], F32, tag="invl_own")
            # floor 1e-8 (not 1e-24): invl <= 1e4 stays fp16-finite in dmat;
            # zero-feature pad rows still give nhi = 0 * 1e4 = 0 exactly
            nc.vector.tensor_scalar_max(ssq[:], ssq[:], 1e-8)
            nc.scalar.activation(invl_own[:], ssq[:], AF.Sqrt)
            nc.vector.reciprocal(invl_own[:], invl_own[:])
            # nhiT_all[:, t*d+j] = nhi_t[j, :] transposed, via hp^T @ D
            ident32 = cpool.tile([128, 128], F32, tag="ident32")
            nc.vector.tensor_scalar(ident32[:], iota_row[:], iota_col[:],
                                    None, op0=OP.is_equal)
            nhiT_all = spool.tile([128, TPC * d], F16, tag="nhiT_all")
            for t in range(TPC):
                dmat = mpool.tile([128, d], F32, tag="dmat")
                nc.vector.tensor_scalar_mul(dmat[:], ident32[:],
                                            invl_own[:, t:t + 1])
                ntp = apool.tile([128, d], F32, tag="ab")
                nc.tensor.matmul(ntp[:], hp_sb[:, t * d:(t + 1) * d],
                                 dmat[:], start=True, stop=True)
                nc.scalar.copy(nhiT_all[:, t * d:(t + 1) * d], ntp[:])


            def emit_b_gather(gi_):
                _, sbase, ltot, htot, _, _ = GM[gi_]
                xfl = gpool.tile([128, MAXL, d], F16, tag="xgl")
                xfh = gpool.tile([128, MAXH, d], F16, tag="xgh")
                i0 = sbase // 16
                nc.gpsimd.dma_gather(
                    out_ap=xfl[:, :ltot, :], in_ap=ftT[0:LROWS, :],
                    idxs_ap=gidx_sb[:, i0:i0 + ltot * 8],
                    num_idxs=ltot * 128, num_idxs_reg=ltot * 128,
                    elem_size=d, single_packet=False)
                nc.gpsimd.dma_gather(
                    out_ap=xfh[:, :htot, :], in_ap=ftT[LROWS:cfg.nodes, :],
                    idxs_ap=gidx_sb[:, i0 + ltot * 8:i0 + (ltot + htot) * 8],
                    num_idxs=htot * 128, num_idxs_reg=htot * 128,
                    elem_size=d, single_packet=False)
                return xfl, xfh

            run_a = cfg.phases in ("A", "T", "full")
            run_t = cfg.phases in ("T", "full")
            run_b = cfg.phases == "full"
            cos_sb = spool.tile([128, NCH], F32, tag="cos")
            if not run_a:
                nc.vector.memset(cos_sb[:, :1], 0.0)

            # ---------- Phase A: cos ----------
            if run_a:
                ascope = nc.named_scope("pcos")
                ascope.__enter__()
                for gi_, grp in enumerate(cfg.groups):
                    gbase, sbase, ltot, htot, lo, ho = GM[gi_]
                    xgl = gpool.tile([128, MAXL * 128], F16, tag="xgl")
                    xgh = gpool.tile([128, MAXH * 128], F16, tag="xgh")
                    i0 = sbase // 16
                    nc.gpsimd.dma_gather(
                        out_ap=xgl[:, :ltot * 128]
                        .rearrange("p (o n) -> p o n", o=1),
                        in_ap=nhT[0:LROWS, :],
                        idxs_ap=gidx_sb[:, i0:i0 + ltot * 8],
                        num_idxs=ltot * 128, num_idxs_reg=ltot * 128,
                        elem_size=d, transpose=True, single_packet=False)
                    nc.gpsimd.dma_gather(
                        out_ap=xgh[:, :htot * 128]
                        .rearrange("p (o n) -> p o n", o=1),
                        in_ap=nhT[LROWS:cfg.nodes, :],
                        idxs_ap=gidx_sb[:, i0 + ltot * 8:
                                        i0 + (ltot + htot) * 8],
                        num_idxs=htot * 128, num_idxs_reg=htot * 128,
                        elem_size=d, transpose=True, single_packet=False)
                    for i, t in enumerate(grp):
                        nlt, nht = cfg.nlk[t], cfg.nhk[t]
                        nct_k = nlt + nht
                        for s0 in range(0, nct_k, cfg.slab):
                            sn = min(cfg.slab, nct_k - s0)
                            c2_ps = ypool.tile([128, cfg.slab * d], F32,
                                               tag="y")
                            for j in range(sn):
                                k = s0 + j
                                if k < nlt:
                                    xT_ap = xgl[:, (lo[i] + k) * 128:
                                                (lo[i] + k) * 128 + 128]
                                else:
                                    hc = ho[i] + (k - nlt)
                                    xT_ap = xgh[:, hc * 128:hc * 128 + 128]
                                nc.tensor.matmul(
                                    c2_ps[:, j * d:(j + 1) * d],
                                    xT_ap,
                                    nhiT_all[:, t * d:(t + 1) * d],
                                    start=True, stop=True)
                            c2_sb = ypool_sb.tile([128, cfg.slab * d], F16,
                                                  tag="ysb")
                            nc.scalar.copy(c2_sb[:, :sn * d],
                                           c2_ps[:, :sn * d])
                            for j in range(sn):
                                k = s0 + j
                                ccg = (lo[i] + k if k < nlt
                                       else ltot + ho[i] + (k - nlt))
                                cc = gbase + ccg
                                scr = mpool.tile([128, d], F16, tag="cscr")
                                nc.vector.scalar_tensor_tensor(
                                    scr[:], iota16[:],
                                    dstloc_sb[:, cc:cc + 1],
                                    c2_sb[:, j * d:(j + 1) * d],
                                    op0=OP.is_equal, op1=OP.mult,
                                    accum_out=cos_sb[:, cc:cc + 1])
                ascope.__exit__(None, None, None)

            # prefetch first B gather groups (overlap with threshold phase)
            pf = []
            if run_b:
                for gi in range(min(2, len(cfg.groups))):
                    pf.append(emit_b_gather(gi))

            # ---------- Phase T: threshold ----------
            if run_t:
                tscope = nc.named_scope("pthr")
                tscope.__enter__()
                lo_t = tpool.tile([1, 1], F32, tag="lo")
                th_row = tpool.tile([1, 32], F32, tag="throw")
                th_bc = tpool.tile([128, 32], F32, tag="thbc")
                cnt128 = tpool.tile([128, 32], F32, tag="cnt128")
                gcnt = tpool.tile([1, 32], F32, tag="gcnt")
                srow = tpool.tile([1, 1], F32, tag="srow")
                cbase = tpool.tile([1, 1], F32, tag="cbase")
                iota32 = tpool.tile([1, 32], F32, tag="iota32")
                nc.vector.tensor_copy(iota32[:], iota_row[:1, :32])
                msk = tpool.tile([1, 32], F32, tag="msk")
                msct = tpool.tile([1, 32], F32, tag="msct")
                cscr2 = tpool.tile([128, NCH], F32, tag="cscr2")
                cand = tpool.tile([128, cfg.cand_cols], F32, tag="cand")
                nc.vector.memset(cbase[:], 0.0)
                nc.vector.memset(lo_t[:], -0.75)

                def emit_round(vals_ap, ncols, w_bin, mode, shift4):
                    nc.vector.tensor_scalar_mul(th_row[:], iota32[:], w_bin)
                    nc.vector.tensor_scalar(th_row[:], th_row[:], lo_t[:],
                                            None, op0=OP.add)
                    if shift4:
                        nc.vector.tensor_scalar_add(th_row[:], th_row[:], 4.0)
                    ps = tppool.tile([128, 32], F32, tag="tiny")
                    nc.tensor.matmul(ps[:], ones_row[:], th_row[:],
                                     start=True, stop=True)
                    nc.vector.tensor_copy(th_bc[:], ps[:])
                    for j in range(32):
                        nc.vector.tensor_scalar(
                            cscr2[:, :ncols], vals_ap, th_bc[:, j:j + 1],
                            None, op0=OP.is_lt, op1=OP.add,
                            accum_out=cnt128[:, j:j + 1])
                    cps = tppool.tile([1, 32], F32, tag="tiny")
                    nc.tensor.matmul(cps[:], ones_col[:], cnt128[:],
                                     start=True, stop=True)
                    nc.vector.tensor_copy(gcnt[:], cps[:])
                    if mode.startswith("global"):
                        if cfg.coll:
                            nc.sync.dma_start(cc_in[:], gcnt[:])
                            nc.gpsimd.collective_compute(
                                "AllReduce", OP.add,
                                replica_groups=groups_rep,
                                ins=[cc_in[:]], outs=[cc_out[:]])
                            nc.sync.dma_start(gcnt[:], cc_out[:])
                        else:
                            nc.sync.dma_start(cc_in[:], gcnt[:])
                            nc.sync.dma_start(gcnt[:], cc_in[:])
                    nc.vector.tensor_scalar(
                        msct[:], gcnt[:], cbase[:], KCUT - 0.5,
                        op0=OP.add, op1=OP.is_lt)
                    nc.vector.tensor_scalar(
                        msct[:], msct[:], 0.0, None,
                        op0=OP.add, op1=OP.add, accum_out=srow[:])
                    nc.vector.tensor_scalar(srow[:], srow[:], -1.0, 0.0,
                                            op0=OP.add, op1=OP.max)
                    if mode == "global2":
                        nc.vector.tensor_scalar(msk[:], iota32[:], srow[:],
                                                None, op0=OP.is_equal)
                        nc.vector.scalar_tensor_tensor(
                            msct[:], gcnt[:], 1.0, msk[:],
                            op0=OP.mult, op1=OP.mult, accum_out=cbase[:])
                    nc.vector.scalar_tensor_tensor(
                        lo_t[:], srow[:], w_bin, lo_t[:],
                        op0=OP.mult, op1=OP.add)

                W1 = 1.5 / 32
                W2 = 1.5 / 32 ** 2
                emit_round(cos_sb[:], NCH, W1, "global1", False)
                emit_round(cos_sb[:], NCH, W2, "global2", False)

                # compact in-bracket values, remapped to cos+4
                lo_bc = tpool.tile([128, 1], F32, tag="lobc")
                psb = tppool.tile([128, 1], F32, tag="tiny")
                nc.tensor.matmul(psb[:], ones_row[:], lo_t[:],
                                 start=True, stop=True)
                nc.vector.tensor_copy(lo_bc[:], psb[:])
                m1 = tpool.tile([128, NCH], F32, tag="m1")
                nc.vector.tensor_scalar(m1[:], cos_sb[:], lo_bc[:], None,
                                        op0=OP.is_ge)
                hi_bc = tpool.tile([128, 1], F32, tag="hibc")
                nc.vector.tensor_scalar_add(hi_bc[:], lo_bc[:], W2)
                m2 = tpool.tile([128, NCH], F32, tag="m2")
                nc.vector.tensor_scalar(m2[:], cos_sb[:], hi_bc[:], None,
                                        op0=OP.is_lt)
                nc.vector.tensor_mul(m1[:], m1[:], m2[:])
                c4 = tpool.tile([128, NCH], F32, tag="c4")
                nc.vector.tensor_scalar(c4[:], cos_sb[:], 5.0, None,
                                        op0=OP.add)
                nc.vector.tensor_mul(c4[:], c4[:], m1[:])
                nc.vector.tensor_scalar_add(c4[:], c4[:], -1.0)
                NSG = 8
                sg_out_cols = cfg.cand_cols // NSG
                sgc = tpool.tile([16, cfg.cand_cols], F32, tag="sgc")
                posi = tpool.tile([16, sg_out_cols], F32, tag="posi")
                nc.gpsimd.iota(posi[:], pattern=[[16, sg_out_cols]], base=0,
                               channel_multiplier=1,
                               allow_small_or_imprecise_dtypes=True)
                for sg_i in range(NSG):
                    y16s = y16pool.tile([16, NCH], F32, tag="y16s")
                    nc.sync.dma_start(y16s[:], c4[16 * sg_i:16 * (sg_i + 1), :])
                    sg_out = y16pool.tile([16, sg_out_cols], F32,
                                          tag="sgout")
                    nfound = y16pool.tile([1, 1], U32, tag="nfound")
                    nc.gpsimd.sparse_gather(sg_out[:], y16s[:],
                                            num_found=nfound[:])
                    nf_f = y16pool.tile([1, 1], F32, tag="nff")
                    nc.vector.tensor_copy(nf_f[:], nfound[:])
                    nf16 = y16pool.tile([16, 1], F32, tag="nf16")
                    ps16 = tppool.tile([16, 1], F32, tag="tiny")
                    nc.tensor.matmul(ps16[:], ones_row[:, :16], nf_f[:],
                                     start=True, stop=True)
                    nc.vector.tensor_copy(nf16[:], ps16[:])
                    mtail = y16pool.tile([16, sg_out_cols], F32,
                                         tag="mtail")
                    nc.vector.tensor_scalar(mtail[:], posi[:], nf16[:], None,
                                            op0=OP.is_lt)
                    big = y16pool.tile([16, sg_out_cols], F32, tag="big")
                    nc.vector.tensor_scalar(big[:], mtail[:], 0.5, 1e30,
                                            op0=OP.is_lt, op1=OP.mult)
                    nc.vector.tensor_mul(sg_out[:], sg_out[:], mtail[:])
                    nc.vector.tensor_add(
                        sgc[:, sg_i * sg_out_cols:(sg_i + 1) * sg_out_cols],
                        sg_out[:], big[:])
                nc.sync.dma_start(ag_in[:], sgc[:])
                if cfg.coll:
                    nc.gpsimd.collective_compute(
                        "AllGather", OP.bypass, replica_groups=groups_rep,
                        ins=[ag_in[:]], outs=[ag_out[:]])
                    for r in range(cfg.cores):
                        nc.sync.dma_start(cand[16 * r:16 * (r + 1), :],
                                          ag_out[r, :, :])
                else:
                    for r in range(cfg.cores):
                        nc.sync.dma_start(cand[16 * r:16 * (r + 1), :],
                                          ag_in[:])

                wr = W2
                for r in range(cfg.nrounds_local):
                    wr = wr / 32
                    emit_round(cand[:], cfg.cand_cols, wr, "local", True)
                nc.vector.tensor_scalar_add(lo_t[:], lo_t[:], wr)
                tstar = tpool.tile([128, 1], F32, tag="tstar")
                pst = tppool.tile([128, 1], F32, tag="tiny")
                nc.tensor.matmul(pst[:], ones_row[:], lo_t[:],
                                 start=True, stop=True)
                nc.vector.tensor_copy(tstar[:], pst[:])

                nc.sync.dma_start(ts_dbg[:, 0:1], lo_t[:])
                nc.sync.dma_start(ts_dbg[:, 1:2], cbase[:])
                nc.sync.dma_start(ts_dbg[:, 2:3], srow[:])
                nc.sync.dma_start(ts_dbg[:, 3:4], nf_f[:])
                # dm = keep*(dstloc+1) - 1  (keep = cos >= t*)
                keep = tpool.tile([128, NCH], F32, tag="m1")
                nc.vector.tensor_scalar(keep[:], cos_sb[:], tstar[:], None,
                                        op0=OP.is_ge)
                dm = tpool.tile([128, NCH], F32, tag="c4")
                nc.vector.tensor_scalar_add(dm[:], dstloc_sb[:], 1.0)
                nc.vector.tensor_mul(dm[:], dm[:], keep[:])
                nc.vector.tensor_scalar_add(dm[:], dm[:], -1.0)
                tscope.__exit__(None, None, None)

            # ---------- Phase B: aggregate + linear + tail ----------
            if run_b:
                bscope2 = nc.named_scope("pagg")
                bscope2.__enter__()
                hblk = 8
                hout_st = None
                for gi, grp in enumerate(cfg.groups):
                    gbase, sbase, ltot, htot, lo, ho = GM[gi]
                    if gi < len(pf):
                        xfl, xfh = pf[gi]
                    else:
                        xfl, xfh = emit_b_gather(gi)
                    for i, t in enumerate(grp):
                        nlt, nht = cfg.nlk[t], cfg.nhk[t]
                        nct_k = nlt + nht
                        if t % hblk == 0:
                            hout_st = hpool.tile([128, hblk * d], F32,
                                                 tag="hout")
                        at_ps = apool.tile([128, 128], F32, tag="ab")
                        for k in range(nct_k):
                            if k < nlt:
                                cc = gbase + lo[i] + k
                                x_ap = xfl[:, lo[i] + k, :]
                            else:
                                cc = gbase + ltot + ho[i] + (k - nlt)
                                x_ap = xfh[:, ho[i] + (k - nlt), :]
                            sa = mpool.tile([128, 128], F16, tag="sa")
                            nc.vector.tensor_scalar(sa[:], iota16[:],
                                                    dm[:, cc:cc + 1], None,
                                                    op0=OP.is_equal)
                            nc.tensor.matmul(at_ps[:], x_ap, sa[:],
                                             start=(k == 0),
                                             stop=(k == nct_k - 1))
                        at_sb = mpool.tile([128, 128], F16, tag="aggTsb")
                        nc.scalar.copy(at_sb[:], at_ps[:])
                        h_ps = apool.tile([128, d], F32, tag="ab")
                        nc.tensor.matmul(h_ps[:], at_sb[:], wT16[:],
                                         start=True, stop=True)
                        hre = mpool.tile([128, d], F32, tag="hre")
                        nc.scalar.activation(hre[:], h_ps[:], AF.Relu,
                                             scale=norm_own[:, t:t + 1])
                        nc.vector.tensor_add(
                            hout_st[:, (t % hblk) * d:(t % hblk + 1) * d],
                            hre[:], hp_sb[:, t * d:(t + 1) * d])
                        if t % hblk == hblk - 1 or t == TPC - 1:
                            hb0 = (t // hblk) * hblk
                            nbt = t - hb0 + 1
                            nc.sync.dma_start(
                                h_ext[:, hb0 * d:(hb0 + nbt) * d],
                                hout_st[:, :nbt * d])
                bscope2.__exit__(None, None, None)

    nc.finalize()
    return nc


def make_cfg(src, dst, kcut, cores=8):
    cfg = Cfg(kcut=kcut, cores=cores)
    rowid = (src.astype(np.int64) % 128) * NT + (src.astype(np.int64) // 128)
    is_h = rowid >= cfg.lsplit
    core_of = dst // cfg.own
    tpc = cfg.tpc
    cntL = np.zeros((cores, tpc), np.int64)
    cntH = np.zeros((cores, tpc), np.int64)
    for c in range(cores):
        sel = core_of == c
        ih, dd = is_h[sel], dst[sel]
        t_all = (dd.astype(np.int64) - c * cfg.own) // 128
        cntL[c] = np.bincount(t_all[~ih], minlength=tpc)
        cntH[c] = np.bincount(t_all[ih], minlength=tpc)
    perms = [np.argsort(-(cntL[c] + cntH[c]), kind="stable")
             for c in range(cores)]
    sL = np.stack([cntL[c][perms[c]] for c in range(cores)])
    sH = np.stack([cntH[c][perms[c]] for c in range(cores)])
    cfg.nlk = tuple(max(1, int(np.ceil(sL[:, k].max() / 128)))
                    for k in range(tpc))
    cfg.nhk = tuple(max(1, int(np.ceil(sH[:, k].max() / 128)))
                    for k in range(tpc))
    cfg.perms = tuple(perms)
    NSG = 8
    sg_in = (cfg.nchunk * 8 + NSG - 1) // NSG
    cfg.cand_cols = NSG * min(64, sg_in)
    return cfg


def make_inputs(cfg: Cfg, features, W, src, dst):
    nreal = features.shape[0]
    featp = np.zeros((cfg.nodes, cfg.d), np.float32)
    featp[:nreal] = features
    feat_t = np.ascontiguousarray(
        featp.reshape(NT, 128, cfg.d).transpose(1, 0, 2).reshape(128, -1))
    deg = np.bincount(dst, minlength=cfg.nodes).astype(np.float32)
    degall = np.ascontiguousarray(deg.reshape(NT, 128).T)
    wTc = np.ascontiguousarray(W.T).astype(np.float32)
    percore = host_prep(cfg, src, dst)
    in_maps = []
    for c in range(cfg.cores):
        base = c * cfg.own
        perm = np.asarray(cfg.perms[c])
        degp = np.ascontiguousarray(
            deg[base:base + cfg.own].reshape(cfg.tpc, 128)[perm].T)
        hpre_t = np.ascontiguousarray(
            featp[base:base + cfg.own].reshape(cfg.tpc, 128, cfg.d)[perm]
            .transpose(1, 0, 2).reshape(128, -1))
        pc = percore[c]
        in_maps.append(dict(
            feat=feat_t, wT=wTc, hpre=hpre_t, degp=degp, degall=degall,
            gidx=pc["gidx"], dstloc=pc["dstloc"]))
    return in_maps


def run(cfg: Cfg, features, W, src, dst):
    in_maps = make_inputs(cfg, features, W, src, dst)
    nc = build_nc(cfg)
    r = run_bass_kernel_spmd(nc, in_maps, core_ids=list(range(cfg.cores)))
    hs = []
    for c in range(cfg.cores):
        hp = r.results[c]["h"]
        tkp = hp.reshape(128, cfg.tpc, cfg.d).transpose(1, 0, 2)
        unp = np.empty_like(tkp)
        unp[np.asarray(cfg.perms[c])] = tkp
        hs.append(unp.reshape(cfg.own, cfg.d))
    h = np.concatenate(hs, axis=0)
    return h[:features.shape[0]]


# ---------------- harness entry point ----------------
def kernel(features, W, src, dst):
    """Full inputs in, full output out. Edges sharded by dst range across
    8 NeuronCores; cosine cut threshold found exactly on-device via
    multi-round counting + candidate compaction + allgather."""
    src = np.asarray(src).astype(np.int32)
    dst = np.asarray(dst).astype(np.int32)
    features = np.asarray(features, dtype=np.float32)
    W = np.asarray(W, dtype=np.float32)
    kcut = int(src.shape[0] * 0.1)
    cfg = make_cfg(src, dst, kcut)
    return run(cfg, features, W, src, dst).astype(np.float32)


# revision 66
# speedup vs baseline: 1.0751x; 1.0208x over previous
"""GCN layer kernel for trn2: host prep + bass kernel builder + runner.

v2: fp16 tables (p-major row numbering), batched DMAs, fp8 sexp fed
directly to PE, fp16 DVE fast modes, grouped multi-tile gathers.
"""
import sys
sys.path.insert(0, '/opt/trn_rl_repo')
import numpy as np
import ml_dtypes
from dataclasses import dataclass

import concourse.bacc as bacc
import concourse.mybir as mybir
import concourse.tile as tile
from concourse.bass_utils import run_bass_kernel_spmd

F32 = mybir.dt.float32
F16 = mybir.dt.float16
I16 = mybir.dt.int16
U32 = mybir.dt.uint32
FP8 = mybir.dt.float8e4
AF = mybir.ActivationFunctionType
OP = mybir.AluOpType

NT = 392          # total node tiles (50176/128)


@dataclass
class Cfg:
    nodes: int = 50176
    lsplit: int = 32536       # 83*392; p-major row split (<=32768 for int16)
    cores: int = 8
    d: int = 128
    kcut: int = 0
    nlk: tuple = ()           # L chunks per tile-rank (len tpc)
    nhk: tuple = ()           # H chunks per tile-rank
    perms: tuple = ()         # per-core tile permutation (host side only)
    G: int = 5                # tiles per gather group
    slab: int = 8             # y chunks per PSUM slab
    bblk: int = 20            # build tiles per DMA block
    cand_cols: int = 256
    nrounds_local: int = 2
    phases: str = "full"      # "build", "A", "T", "full"
    coll: bool = True         # False: stub collectives (tlsim)

    @property
    def tpc(self):
        return self.nodes // 128 // self.cores

    @property
    def own(self):
        return self.nodes // self.cores

    @property
    def nchunk(self):
        return int(sum(self.nlk) + sum(self.nhk))

    @property
    def slots(self):
        return self.nchunk * 128

    @property
    def groups(self):
        gs = []
        t = 0
        while t < self.tpc:
            gs.append(list(range(t, min(t + self.G, self.tpc))))
            t += self.G
        return gs


def group_meta(cfg):
    """Per group: (gbase_chunk, sbase_slot, ltot, htot, loffs, hoffs)."""
    out = []
    gb = sb = 0
    for grp in cfg.groups:
        lo, ho = [], []
        lt = ht = 0
        for k in grp:
            lo.append(lt); lt += cfg.nlk[k]
        for k in grp:
            ho.append(ht); ht += cfg.nhk[k]
        out.append((gb, sb, lt, ht, lo, ho))
        gb += lt + ht
        sb += (lt + ht) * 128
    return out


def host_prep(cfg: Cfg, src, dst):
    """Per-core gidx/dstloc/sexp in grouped-slot order, p-major row ids."""
    rowid = (src.astype(np.int64) % 128) * NT + (src.astype(np.int64) // 128)
    is_h_all = rowid >= cfg.lsplit
    rloc_all = rowid - np.where(is_h_all, cfg.lsplit, 0)
    core_of = dst // cfg.own
    gm = group_meta(cfg)
    out = []
    for c in range(cfg.cores):
        sel = np.nonzero(core_of == c)[0]
        rl = rloc_all[sel]
        ih = is_h_all[sel]
        dloc = dst[sel].astype(np.int64) - c * cfg.own
        t_all = dloc // 128
        loc = dloc % 128
        gidx = np.zeros(cfg.slots, np.int64)
        dstloc = np.full(cfg.slots, -1.0, np.float32)
        order = np.lexsort((loc, ih, t_all))
        rl, t_all, loc, ih = rl[order], t_all[order], loc[order], ih[order]
        perm = cfg.perms[c]
        for gi_, grp in enumerate(cfg.groups):
            _, sbase, ltot, htot, lo, ho = gm[gi_]
            for i, k in enumerate(grp):
                tt = perm[k]
                for hs, budget, coff in ((0, cfg.nlk[k], lo[i]),
                                         (1, cfg.nhk[k], ltot + ho[i])):
                    mm = (t_all == tt) & (ih == hs)
                    n = int(mm.sum())
                    assert n <= budget * 128, \
                        f"c{c} k{k} hs{hs}: {n}>{budget*128}"
                    gi = np.zeros(budget * 128, np.int64)
                    gi[:n] = rl[mm]
                    dl = np.full(budget * 128, -1.0, np.float32)
                    dl[:n] = loc[mm].astype(np.float32)
                    s0 = sbase + coff * 128
                    gidx[s0:s0 + budget * 128] = gi
                    dstloc[s0:s0 + budget * 128] = dl
        gidx_w = np.ascontiguousarray(
            np.tile(gidx.astype(np.int16).reshape(-1, 16).T, (8, 1)))
        dstloc_pc = np.ascontiguousarray(
            dstloc.reshape(cfg.nchunk, 128).T)
        out.append(dict(gidx=gidx_w, dstloc=dstloc_pc))
    return out


def build_nc(cfg: Cfg):
    nc = bacc.Bacc(None)
    d = cfg.d
    TPC, NCH = cfg.tpc, cfg.nchunk
    GM = group_meta(cfg)
    MAXL = max(m[2] for m in GM)
    MAXH = max(m[3] for m in GM)
    KCUT = float(cfg.kcut)
    LROWS, HROWS = cfg.lsplit, cfg.nodes - cfg.lsplit

    feat = nc.dram_tensor("feat", [128, NT * d], F32, kind="ExternalInput")
    wT = nc.dram_tensor("wT", [d, d], F32, kind="ExternalInput")
    hpre = nc.dram_tensor("hpre", [128, TPC * d], F32, kind="ExternalInput")
    degp = nc.dram_tensor("degp", [128, TPC], F32, kind="ExternalInput")
    degall = nc.dram_tensor("degall", [128, NT], F32, kind="ExternalInput")
    gidx_e = nc.dram_tensor("gidx", [128, cfg.slots // 16], I16,
                            kind="ExternalInput")
    dstloc_e = nc.dram_tensor("dstloc", [128, NCH], F32, kind="ExternalInput")
    h_ext = nc.dram_tensor("h", [128, TPC * d], F32, kind="ExternalOutput")
    ts_dbg = nc.dram_tensor("ts_dbg", [1, 4], F32, kind="ExternalOutput")

    cc_in = nc.dram_tensor("cc_in", [1, 32], F32)
    cc_out = nc.dram_tensor("cc_out", [1, 32], F32, addr_space="Shared")
    ag_in = nc.dram_tensor("ag_in", [16, cfg.cand_cols], F32)
    ag_out = nc.dram_tensor("ag_out", [cfg.cores, 16, cfg.cand_cols], F32,
                            addr_space="Shared")
    groups_rep = [list(range(cfg.cores))]

    with tile.TileContext(nc) as tc:
        with (tc.tile_pool(name="const", bufs=1) as cpool,
              tc.tile_pool(name="state", bufs=1) as spool,
              tc.tile_pool(name="dram", bufs=1, space="DRAM") as dpool,
              tc.tile_pool(name="bfeat", bufs=2) as bfpool,
              tc.tile_pool(name="bstage", bufs=2) as bspool,
              tc.tile_pool(name="gath", bufs=2) as gpool,
              tc.tile_pool(name="ysb", bufs=4) as ypool_sb,
              tc.tile_pool(name="hstage", bufs=2) as hpool,
              tc.tile_pool(name="ypsum", bufs=2, space="PSUM") as ypool,
              tc.tile_pool(name="apsum", bufs=2, space="PSUM") as apool,
              tc.tile_pool(name="tpsum", bufs=2, space="PSUM") as tppool,
              tc.tile_pool(name="misc", bufs=3) as mpool,
              tc.tile_pool(name="y16p", bufs=2) as y16pool,
              tc.tile_pool(name="thr", bufs=1) as tpool):

            # ---------- DRAM tables (fp16, contiguous p-major rows;
            # pad slots gather garbage row 0 -- harmless, sexp col is 0) ----
            nhT = dpool.tile([cfg.nodes, d], F16, tag="nhT")
            ftT = dpool.tile([cfg.nodes, d], F16, tag="ftT")

            # ---------- constants / inputs ----------
            iota_row = cpool.tile([128, 128], F32, tag="iota_row")
            nc.gpsimd.iota(iota_row[:], pattern=[[1, 128]], base=0,
                           channel_multiplier=0,
                           allow_small_or_imprecise_dtypes=True)
            iota16 = cpool.tile([128, 128], F16, tag="iota16")
            nc.vector.tensor_copy(iota16[:], iota_row[:])
            iota_col = cpool.tile([128, 1], F32, tag="iota_col")
            nc.gpsimd.iota(iota_col[:], pattern=[[1, 1]], base=0,
                           channel_multiplier=1,
                           allow_small_or_imprecise_dtypes=True)
            ones_col = cpool.tile([128, 1], F32, tag="ones_col")
            nc.vector.memset(ones_col[:], 1.0)
            ones_row = cpool.tile([1, 128], F32, tag="ones_row")
            nc.vector.memset(ones_row[:], 1.0)
            degall_sb = spool.tile([128, NT], F32, tag="degall")
            nc.sync.dma_start(degall_sb[:], degall[:])
            norm_all = spool.tile([128, NT], F32, tag="norm_all")
            nc.vector.tensor_scalar_max(norm_all[:], degall_sb[:], 1.0)
            nc.scalar.activation(norm_all[:], norm_all[:], AF.Sqrt)
            nc.vector.reciprocal(norm_all[:], norm_all[:])


            # ---------- table build (blocked) ----------
            bscope = nc.named_scope("pbuild")
            bscope.__enter__()
            nb = cfg.bblk
            for b0 in range(0, NT, nb):
                bt = min(nb, NT - b0)
                fblk = bfpool.tile([128, nb * d], F32, tag="fblk")
                nc.sync.dma_start(fblk[:, :bt * d],
                                  feat[:, b0 * d:(b0 + bt) * d])
                bss = mpool.tile([128, nb], F32, tag="bss")
                for ti in range(bt):
                    scr = mpool.tile([128, d], F32, tag="bsq")
                    nc.vector.scalar_tensor_tensor(
                        scr[:], fblk[:, ti * d:(ti + 1) * d], 1.0,
                        fblk[:, ti * d:(ti + 1) * d],
                        op0=OP.mult, op1=OP.mult,
                        accum_out=bss[:, ti:ti + 1])
                nc.vector.tensor_scalar_max(bss[:, :bt], bss[:, :bt], 1e-24)
                nc.scalar.activation(bss[:, :bt], bss[:, :bt], AF.Sqrt)
                nc.vector.reciprocal(bss[:, :bt], bss[:, :bt])
                nh_st = bspool.tile([128, nb * d], F16, tag="nh_st")
                ft_st = bspool.tile([128, nb * d], F16, tag="ft_st")
                for ti in range(bt):
                    nc.vector.tensor_scalar_mul(
                        nh_st[:, ti * d:(ti + 1) * d],
                        fblk[:, ti * d:(ti + 1) * d], bss[:, ti:ti + 1])
                    nc.scalar.activation(
                        ft_st[:, ti * d:(ti + 1) * d],
                        fblk[:, ti * d:(ti + 1) * d], AF.Copy,
                        scale=norm_all[:, b0 + ti:b0 + ti + 1])
                for tab, st in ((nhT, nh_st), (ftT, ft_st)):
                    nc.sync.dma_start(
                        tab[:]
                        .rearrange("(p t) x -> p t x", t=NT)[:, b0:b0 + bt, :],
                        st[:, :bt * d]
                        .rearrange("p (t x) -> p t x", x=d))
            bscope.__exit__(None, None, None)
            # ---- A/B-phase inputs + nhiT, emitted late so the build
            # loop's DMA stream starts immediately ----
            gidx_sb = spool.tile([128, cfg.slots // 16], I16, tag="gidx")
            nc.sync.dma_start(gidx_sb[:], gidx_e[:])
            dstloc_sb = spool.tile([128, NCH], F32, tag="dstloc")
            nc.sync.dma_start(dstloc_sb[:], dstloc_e[:])
            hp_sb = spool.tile([128, TPC * d], F32, tag="hp")
            nc.sync.dma_start(hp_sb[:], hpre[:])
            wT_sb = cpool.tile([d, d], F32, tag="wT")
            nc.sync.dma_start(wT_sb[:], wT[:])
            wT16 = cpool.tile([d, d], F16, tag="wT16")
            nc.vector.tensor_copy(wT16[:], wT_sb[:])
            degp_sb = spool.tile([128, TPC], F32, tag="degp")
            nc.sync.dma_start(degp_sb[:], degp[:])
            norm_own = spool.tile([128, TPC], F32, tag="norm_own")
            nc.vector.tensor_scalar_max(norm_own[:], degp_sb[:], 1.0)
            nc.scalar.activation(norm_own[:], norm_own[:], AF.Sqrt)
            nc.vector.reciprocal(norm_own[:], norm_own[:])
            # nhi_all: normalized own-tile rows (fp16), for phase-A rhs
            ssq = spool.tile([128, TPC], F32, tag="ssq")
            for t in range(TPC):
                scr = mpool.tile([128, d], F32, tag="sqscr")
                nc.scalar.activation(scr[:], hp_sb[:, t * d:(t + 1) * d],
                                     AF.Square, accum_out=ssq[:, t:t + 1])
            invl_own = spool.tile([128, TPC], F32, tag="invl_own")
            # floor 1e-8 (not 1e-24): invl <= 1e4 stays fp16-finite in dmat;
            # zero-feature pad rows still give nhi = 0 * 1e4 = 0 exactly
            nc.vector.tensor_scalar_max(ssq[:], ssq[:], 1e-8)
            nc.scalar.activation(invl_own[:], ssq[:], AF.Sqrt)
            nc.vector.reciprocal(invl_own[:], invl_own[:])
            # nhiT_all[:, t*d+j] = nhi_t[j, :] transposed, via hp^T @ D
            ident32 = cpool.tile([128, 128], F32, tag="ident32")
            nc.vector.tensor_scalar(ident32[:], iota_row[:], iota_col[:],
                                    None, op0=OP.is_equal)
            nhiT_all = spool.tile([128, TPC * d], F16, tag="nhiT_all")
            for t in range(TPC):
                dmat = mpool.tile([128, d], F32, tag="dmat")
                nc.vector.tensor_scalar_mul(dmat[:], ident32[:],
                                            invl_own[:, t:t + 1])
                ntp = apool.tile([128, d], F32, tag="ab")
                nc.tensor.matmul(ntp[:], hp_sb[:, t * d:(t + 1) * d],
                                 dmat[:], start=True, stop=True)
                nc.scalar.copy(nhiT_all[:, t * d:(t + 1) * d], ntp[:])


            def emit_b_gather(gi_):
                _, sbase, ltot, htot, _, _ = GM[gi_]
                xfl = gpool.tile([128, MAXL, d], F16, tag="xgl")
                xfh = gpool.tile([128, MAXH, d], F16, tag="xgh")
                i0 = sbase // 16
                nc.gpsimd.dma_gather(
                    out_ap=xfl[:, :ltot, :], in_ap=ftT[0:LROWS, :],
                    idxs_ap=gidx_sb[:, i0:i0 + ltot * 8],
                    num_idxs=ltot * 128, num_idxs_reg=ltot * 128,
                    elem_size=d, single_packet=False)
                nc.gpsimd.dma_gather(
                    out_ap=xfh[:, :htot, :], in_ap=ftT[LROWS:cfg.nodes, :],
                    idxs_ap=gidx_sb[:, i0 + ltot * 8:i0 + (ltot + htot) * 8],
                    num_idxs=htot * 128, num_idxs_reg=htot * 128,
                    elem_size=d, single_packet=False)
                return xfl, xfh

            run_a = cfg.phases in ("A", "T", "full")
            run_t = cfg.phases in ("T", "full")
            run_b = cfg.phases == "full"
            cos_sb = spool.tile([128, NCH], F32, tag="cos")
            if not run_a:
                nc.vector.memset(cos_sb[:, :1], 0.0)

            # ---------- Phase A: cos ----------
            if run_a:
                ascope = nc.named_scope("pcos")
                ascope.__enter__()
                for gi_, grp in enumerate(cfg.groups):
                    gbase, sbase, ltot, htot, lo, ho = GM[gi_]
                    xgl = gpool.tile([128, MAXL * 128], F16, tag="xgl")
                    xgh = gpool.tile([128, MAXH * 128], F16, tag="xgh")
                    i0 = sbase // 16
                    nc.gpsimd.dma_gather(
                        out_ap=xgl[:, :ltot * 128]
                        .rearrange("p (o n) -> p o n", o=1),
                        in_ap=nhT[0:LROWS, :],
                        idxs_ap=gidx_sb[:, i0:i0 + ltot * 8],
                        num_idxs=ltot * 128, num_idxs_reg=ltot * 128,
                        elem_size=d, transpose=True, single_packet=False)
                    nc.gpsimd.dma_gather(
                        out_ap=xgh[:, :htot * 128]
                        .rearrange("p (o n) -> p o n", o=1),
                        in_ap=nhT[LROWS:cfg.nodes, :],
                        idxs_ap=gidx_sb[:, i0 + ltot * 8:
                                        i0 + (ltot + htot) * 8],
                        num_idxs=htot * 128, num_idxs_reg=htot * 128,
                        elem_size=d, transpose=True, single_packet=False)
                    for i, t in enumerate(grp):
                        nlt, nht = cfg.nlk[t], cfg.nhk[t]
                        nct_k = nlt + nht
                        for s0 in range(0, nct_k, cfg.slab):
                            sn = min(cfg.slab, nct_k - s0)
                            c2_ps = ypool.tile([128, cfg.slab * d], F32,
                                               tag="y")
                            for j in range(sn):
                                k = s0 + j
                                if k < nlt:
                                    xT_ap = xgl[:, (lo[i] + k) * 128:
                                                (lo[i] + k) * 128 + 128]
                                else:
                                    hc = ho[i] + (k - nlt)
                                    xT_ap = xgh[:, hc * 128:hc * 128 + 128]
                                nc.tensor.matmul(
                                    c2_ps[:, j * d:(j + 1) * d],
                                    xT_ap,
                                    nhiT_all[:, t * d:(t + 1) * d],
                                    start=True, stop=True)
                            c2_sb = ypool_sb.tile([128, cfg.slab * d], F16,
                                                  tag="ysb")
                            nc.scalar.copy(c2_sb[:, :sn * d],
                                           c2_ps[:, :sn * d])
                            for j in range(sn):
                                k = s0 + j
                                ccg = (lo[i] + k if k < nlt
                                       else ltot + ho[i] + (k - nlt))
                                cc = gbase + ccg
                                scr = mpool.tile([128, d], F16, tag="cscr")
                                nc.vector.scalar_tensor_tensor(
                                    scr[:], iota16[:],
                                    dstloc_sb[:, cc:cc + 1],
                                    c2_sb[:, j * d:(j + 1) * d],
                                    op0=OP.is_equal, op1=OP.mult,
                                    accum_out=cos_sb[:, cc:cc + 1])
                ascope.__exit__(None, None, None)

            # prefetch first B gather groups (overlap with threshold phase)
            pf = []
            if run_b:
                for gi in range(min(2, len(cfg.groups))):
                    pf.append(emit_b_gather(gi))

            # ---------- Phase T: threshold ----------
            if run_t:
                tscope = nc.named_scope("pthr")
                tscope.__enter__()
                lo_t = tpool.tile([1, 1], F32, tag="lo")
                th_row = tpool.tile([1, 32], F32, tag="throw")
                th_bc = tpool.tile([128, 32], F32, tag="thbc")
                cnt128 = tpool.tile([128, 32], F32, tag="cnt128")
                gcnt = tpool.tile([1, 32], F32, tag="gcnt")
                srow = tpool.tile([1, 1], F32, tag="srow")
                cbase = tpool.tile([1, 1], F32, tag="cbase")
                iota32 = tpool.tile([1, 32], F32, tag="iota32")
                nc.vector.tensor_copy(iota32[:], iota_row[:1, :32])
                msk = tpool.tile([1, 32], F32, tag="msk")
                msct = tpool.tile([1, 32], F32, tag="msct")
                cscr2 = tpool.tile([128, NCH], F32, tag="cscr2")
                cand = tpool.tile([128, cfg.cand_cols], F32, tag="cand")
                nc.vector.memset(cbase[:], 0.0)
                nc.vector.memset(lo_t[:], -0.75)

                def emit_round(vals_ap, ncols, w_bin, mode, shift4):
                    nc.vector.tensor_scalar_mul(th_row[:], iota32[:], w_bin)
                    nc.vector.tensor_scalar(th_row[:], th_row[:], lo_t[:],
                                            None, op0=OP.add)
                    if shift4:
                        nc.vector.tensor_scalar_add(th_row[:], th_row[:], 4.0)
                    ps = tppool.tile([128, 32], F32, tag="tiny")
                    nc.tensor.matmul(ps[:], ones_row[:], th_row[:],
                                     start=True, stop=True)
                    nc.vector.tensor_copy(th_bc[:], ps[:])
                    for j in range(32):
                        nc.vector.tensor_scalar(
                            cscr2[:, :ncols], vals_ap, th_bc[:, j:j + 1],
                            None, op0=OP.is_lt, op1=OP.add,
                            accum_out=cnt128[:, j:j + 1])
                    cps = tppool.tile([1, 32], F32, tag="tiny")
                    nc.tensor.matmul(cps[:], ones_col[:], cnt128[:],
                                     start=True, stop=True)
                    nc.vector.tensor_copy(gcnt[:], cps[:])
                    if mode.startswith("global"):
                        if cfg.coll:
                            nc.sync.dma_start(cc_in[:], gcnt[:])
                            nc.gpsimd.collective_compute(
                                "AllReduce", OP.add,
                                replica_groups=groups_rep,
                                ins=[cc_in[:]], outs=[cc_out[:]])
                            nc.sync.dma_start(gcnt[:], cc_out[:])
                        else:
                            nc.sync.dma_start(cc_in[:], gcnt[:])
                            nc.sync.dma_start(gcnt[:], cc_in[:])
                    nc.vector.tensor_scalar(
                        msct[:], gcnt[:], cbase[:], KCUT - 0.5,
                        op0=OP.add, op1=OP.is_lt)
                    nc.vector.tensor_scalar(
                        msct[:], msct[:], 0.0, None,
                        op0=OP.add, op1=OP.add, accum_out=srow[:])
                    nc.vector.tensor_scalar(srow[:], srow[:], -1.0, 0.0,
                                            op0=OP.add, op1=OP.max)
                    if mode == "global2":
                        nc.vector.tensor_scalar(msk[:], iota32[:], srow[:],
                                                None, op0=OP.is_equal)
                        nc.vector.scalar_tensor_tensor(
                            msct[:], gcnt[:], 1.0, msk[:],
                            op0=OP.mult, op1=OP.mult, accum_out=cbase[:])
                    nc.vector.scalar_tensor_tensor(
                        lo_t[:], srow[:], w_bin, lo_t[:],
                        op0=OP.mult, op1=OP.add)

                W1 = 1.5 / 32
                W2 = 1.5 / 32 ** 2
                emit_round(cos_sb[:], NCH, W1, "global1", False)
                emit_round(cos_sb[:], NCH, W2, "global2", False)

                # compact in-bracket values, remapped to cos+4
                lo_bc = tpool.tile([128, 1], F32, tag="lobc")
                psb = tppool.tile([128, 1], F32, tag="tiny")
                nc.tensor.matmul(psb[:], ones_row[:], lo_t[:],
                                 start=True, stop=True)
                nc.vector.tensor_copy(lo_bc[:], psb[:])
                m1 = tpool.tile([128, NCH], F32, tag="m1")
                nc.vector.tensor_scalar(m1[:], cos_sb[:], lo_bc[:], None,
                                        op0=OP.is_ge)
                hi_bc = tpool.tile([128, 1], F32, tag="hibc")
                nc.vector.tensor_scalar_add(hi_bc[:], lo_bc[:], W2)
                m2 = tpool.tile([128, NCH], F32, tag="m2")
                nc.vector.tensor_scalar(m2[:], cos_sb[:], hi_bc[:], None,
                                        op0=OP.is_lt)
                nc.vector.tensor_mul(m1[:], m1[:], m2[:])
                c4 = tpool.tile([128, NCH], F32, tag="c4")
                nc.vector.tensor_scalar(c4[:], cos_sb[:], 5.0, None,
                                        op0=OP.add)
                nc.vector.tensor_mul(c4[:], c4[:], m1[:])
                nc.vector.tensor_scalar_add(c4[:], c4[:], -1.0)
                NSG = 8
                sg_out_cols = cfg.cand_cols // NSG
                sgc = tpool.tile([16, cfg.cand_cols], F32, tag="sgc")
                posi = tpool.tile([16, sg_out_cols], F32, tag="posi")
                nc.gpsimd.iota(posi[:], pattern=[[16, sg_out_cols]], base=0,
                               channel_multiplier=1,
                               allow_small_or_imprecise_dtypes=True)
                for sg_i in range(NSG):
                    y16s = y16pool.tile([16, NCH], F32, tag="y16s")
                    nc.sync.dma_start(y16s[:], c4[16 * sg_i:16 * (sg_i + 1), :])
                    sg_out = y16pool.tile([16, sg_out_cols], F32,
                                          tag="sgout")
                    nfound = y16pool.tile([1, 1], U32, tag="nfound")
                    nc.gpsimd.sparse_gather(sg_out[:], y16s[:],
                                            num_found=nfound[:])
                    nf_f = y16pool.tile([1, 1], F32, tag="nff")
                    nc.vector.tensor_copy(nf_f[:], nfound[:])
                    nf16 = y16pool.tile([16, 1], F32, tag="nf16")
                    ps16 = tppool.tile([16, 1], F32, tag="tiny")
                    nc.tensor.matmul(ps16[:], ones_row[:, :16], nf_f[:],
                                     start=True, stop=True)
                    nc.vector.tensor_copy(nf16[:], ps16[:])
                    mtail = y16pool.tile([16, sg_out_cols], F32,
                                         tag="mtail")
                    nc.vector.tensor_scalar(mtail[:], posi[:], nf16[:], None,
                                            op0=OP.is_lt)
                    big = y16pool.tile([16, sg_out_cols], F32, tag="big")
                    nc.vector.tensor_scalar(big[:], mtail[:], 0.5, 1e30,
                                            op0=OP.is_lt, op1=OP.mult)
                    nc.vector.tensor_mul(sg_out[:], sg_out[:], mtail[:])
                    nc.vector.tensor_add(
                        sgc[:, sg_i * sg_out_cols:(sg_i + 1) * sg_out_cols],
                        sg_out[:], big[:])
                nc.sync.dma_start(ag_in[:], sgc[:])
                if cfg.coll:
                    nc.gpsimd.collective_compute(
                        "AllGather", OP.bypass, replica_groups=groups_rep,
                        ins=[ag_in[:]], outs=[ag_out[:]])
                    for r in range(cfg.cores):
                        nc.sync.dma_start(cand[16 * r:16 * (r + 1), :],
                                          ag_out[r, :, :])
                else:
                    for r in range(cfg.cores):
                        nc.sync.dma_start(cand[16 * r:16 * (r + 1), :],
                                          ag_in[:])

                wr = W2
                for r in range(cfg.nrounds_local):
                    wr = wr / 32
                    emit_round(cand[:], cfg.cand_cols, wr, "local", True)
                nc.vector.tensor_scalar_add(lo_t[:], lo_t[:], wr)
                tstar = tpool.tile([128, 1], F32, tag="tstar")
                pst = tppool.tile([128, 1], F32, tag="tiny")
                nc.tensor.matmul(pst[:], ones_row[:], lo_t[:],
                                 start=True, stop=True)
                nc.vector.tensor_copy(tstar[:], pst[:])

                nc.sync.dma_start(ts_dbg[:, 0:1], lo_t[:])
                nc.sync.dma_start(ts_dbg[:, 1:2], cbase[:])
                nc.sync.dma_start(ts_dbg[:, 2:3], srow[:])
                nc.sync.dma_start(ts_dbg[:, 3:4], nf_f[:])
                # dm = keep*(dstloc+1) - 1  (keep = cos >= t*)
                keep = tpool.tile([128, NCH], F32, tag="m1")
                nc.vector.tensor_scalar(keep[:], cos_sb[:], tstar[:], None,
                                        op0=OP.is_ge)
                dm = tpool.tile([128, NCH], F32, tag="c4")
                nc.vector.tensor_scalar_add(dm[:], dstloc_sb[:], 1.0)
                nc.vector.tensor_mul(dm[:], dm[:], keep[:])
                nc.vector.tensor_scalar_add(dm[:], dm[:], -1.0)
                tscope.__exit__(None, None, None)

            # ---------- Phase B: aggregate + linear + tail ----------
            if run_b:
                bscope2 = nc.named_scope("pagg")
                bscope2.__enter__()
                hblk = 8
                hout_st = None
                for gi, grp in enumerate(cfg.groups):
                    gbase, sbase, ltot, htot, lo, ho = GM[gi]
                    if gi < len(pf):
                        xfl, xfh = pf[gi]
                    else:
                        xfl, xfh = emit_b_gather(gi)
                    for i, t in enumerate(grp):
                        nlt, nht = cfg.nlk[t], cfg.nhk[t]
                        nct_k = nlt + nht
                        if t % hblk == 0:
                            hout_st = hpool.tile([128, hblk * d], F32,
                                                 tag="hout")
                        at_ps = apool.tile([128, 128], F32, tag="ab")
                        for k in range(nct_k):
                            if k < nlt:
                                cc = gbase + lo[i] + k
                                x_ap = xfl[:, lo[i] + k, :]
                            else:
                                cc = gbase + ltot + ho[i] + (k - nlt)
                                x_ap = xfh[:, ho[i] + (k - nlt), :]
                            sa = mpool.tile([128, 128], F16, tag="sa")
                            nc.vector.tensor_scalar(sa[:], iota16[:],
                                                    dm[:, cc:cc + 1], None,
                                                    op0=OP.is_equal)
                            nc.tensor.matmul(at_ps[:], x_ap, sa[:],
                                             start=(k == 0),
                                             stop=(k == nct_k - 1))
                        at_sb = mpool.tile([128, 128], F16, tag="aggTsb")
                        nc.scalar.copy(at_sb[:], at_ps[:])
                        h_ps = apool.tile([128, d], F32, tag="ab")
                        nc.tensor.matmul(h_ps[:], at_sb[:], wT16[:],
                                         start=True, stop=True)
                        hre = mpool.tile([128, d], F32, tag="hre")
                        nc.scalar.activation(hre[:], h_ps[:], AF.Relu,
                                             scale=norm_own[:, t:t + 1])
                        nc.vector.tensor_add(
                            hout_st[:, (t % hblk) * d:(t % hblk + 1) * d],
                            hre[:], hp_sb[:, t * d:(t + 1) * d])
                        if t % hblk == hblk - 1 or t == TPC - 1:
                            hb0 = (t // hblk) * hblk
                            nbt = t - hb0 + 1
                            nc.sync.dma_start(
                                h_ext[:, hb0 * d:(hb0 + nbt) * d],
                                hout_st[:, :nbt * d])
                bscope2.__exit__(None, None, None)

    nc.finalize()
    return nc


def make_cfg(src, dst, kcut, cores=8):
    cfg = Cfg(kcut=kcut, cores=cores)
    rowid = (src.astype(np.int64) % 128) * NT + (src.astype(np.int64) // 128)
    is_h = rowid >= cfg.lsplit
    core_of = dst // cfg.own
    tpc = cfg.tpc
    cntL = np.zeros((cores, tpc), np.int64)
    cntH = np.zeros((cores, tpc), np.int64)
    for c in range(cores):
        sel = core_of == c
        ih, dd = is_h[sel], dst[sel]
        t_all = (dd.astype(np.int64) - c * cfg.own) // 128
        cntL[c] = np.bincount(t_all[~ih], minlength=tpc)
        cntH[c] = np.bincount(t_all[ih], minlength=tpc)
    perms = [np.argsort(-(cntL[c] + cntH[c]), kind="stable")
             for c in range(cores)]
    sL = np.stack([cntL[c][perms[c]] for c in range(cores)])
    sH = np.stack([cntH[c][perms[c]] for c in range(cores)])
    cfg.nlk = tuple(max(1, int(np.ceil(sL[:, k].max() / 128)))
                    for k in range(tpc))
    cfg.nhk = tuple(max(1, int(np.ceil(sH[:, k].max() / 128)))
                    for k in range(tpc))
    cfg.perms = tuple(perms)
    NSG = 8
    sg_in = (cfg.nchunk * 8 + NSG - 1) // NSG
    cfg.cand_cols = NSG * min(16, sg_in)
    return cfg


def make_inputs(cfg: Cfg, features, W, src, dst):
    nreal = features.shape[0]
    featp = np.zeros((cfg.nodes, cfg.d), np.float32)
    featp[:nreal] = features
    feat_t = np.ascontiguousarray(
        featp.reshape(NT, 128, cfg.d).transpose(1, 0, 2).reshape(128, -1))
    deg = np.bincount(dst, minlength=cfg.nodes).astype(np.float32)
    degall = np.ascontiguousarray(deg.reshape(NT, 128).T)
    wTc = np.ascontiguousarray(W.T).astype(np.float32)
    percore = host_prep(cfg, src, dst)
    in_maps = []
    for c in range(cfg.cores):
        base = c * cfg.own
        perm = np.asarray(cfg.perms[c])
        degp = np.ascontiguousarray(
            deg[base:base + cfg.own].reshape(cfg.tpc, 128)[perm].T)
        hpre_t = np.ascontiguousarray(
            featp[base:base + cfg.own].reshape(cfg.tpc, 128, cfg.d)[perm]
            .transpose(1, 0, 2).reshape(128, -1))
        pc = percore[c]
        in_maps.append(dict(
            feat=feat_t, wT=wTc, hpre=hpre_t, degp=degp, degall=degall,
            gidx=pc["gidx"], dstloc=pc["dstloc"]))
    return in_maps


def run(cfg: Cfg, features, W, src, dst):
    in_maps = make_inputs(cfg, features, W, src, dst)
    nc = build_nc(cfg)
    r = run_bass_kernel_spmd(nc, in_maps, core_ids=list(range(cfg.cores)))
    hs = []
    for c in range(cfg.cores):
        hp = r.results[c]["h"]
        tkp = hp.reshape(128, cfg.tpc, cfg.d).transpose(1, 0, 2)
        unp = np.empty_like(tkp)
        unp[np.asarray(cfg.perms[c])] = tkp
        hs.append(unp.reshape(cfg.own, cfg.d))
    h = np.concatenate(hs, axis=0)
    return h[:features.shape[0]]


# ---------------- harness entry point ----------------
def kernel(features, W, src, dst):
    """Full inputs in, full output out. Edges sharded by dst range across
    8 NeuronCores; cosine cut threshold found exactly on-device via
    multi-round counting + candidate compaction + allgather."""
    src = np.asarray(src).astype(np.int32)
    dst = np.asarray(dst).astype(np.int32)
    features = np.asarray(features, dtype=np.float32)
    W = np.asarray(W, dtype=np.float32)
    kcut = int(src.shape[0] * 0.1)
    cfg = make_cfg(src, dst, kcut)
    return run(cfg, features, W, src, dst).astype(np.float32)


# revision 67
# speedup vs baseline: 1.0797x; 1.0043x over previous
"""GCN layer kernel for trn2: host prep + bass kernel builder + runner.

v2: fp16 tables (p-major row numbering), batched DMAs, fp8 sexp fed
directly to PE, fp16 DVE fast modes, grouped multi-tile gathers.
"""
import sys
sys.path.insert(0, '/opt/trn_rl_repo')
import numpy as np
import ml_dtypes
from dataclasses import dataclass

import concourse.bacc as bacc
import concourse.mybir as mybir
import concourse.tile as tile
from concourse.bass_utils import run_bass_kernel_spmd

F32 = mybir.dt.float32
F16 = mybir.dt.float16
I16 = mybir.dt.int16
U32 = mybir.dt.uint32
FP8 = mybir.dt.float8e4
AF = mybir.ActivationFunctionType
OP = mybir.AluOpType

NT = 392          # total node tiles (50176/128)


@dataclass
class Cfg:
    nodes: int = 50176
    lsplit: int = 32536       # 83*392; p-major row split (<=32768 for int16)
    cores: int = 8
    d: int = 128
    kcut: int = 0
    nlk: tuple = ()           # L chunks per tile-rank (len tpc)
    nhk: tuple = ()           # H chunks per tile-rank
    perms: tuple = ()         # per-core tile permutation (host side only)
    G: int = 5                # tiles per gather group
    slab: int = 8             # y chunks per PSUM slab
    bblk: int = 20            # build tiles per DMA block
    cand_cols: int = 256
    nrounds_local: int = 2
    phases: str = "full"      # "build", "A", "T", "full"
    coll: bool = True         # False: stub collectives (tlsim)

    @property
    def tpc(self):
        return self.nodes // 128 // self.cores

    @property
    def own(self):
        return self.nodes // self.cores

    @property
    def nchunk(self):
        return int(sum(self.nlk) + sum(self.nhk))

    @property
    def slots(self):
        return self.nchunk * 128

    @property
    def groups(self):
        gs = []
        t = 0
        while t < self.tpc:
            gs.append(list(range(t, min(t + self.G, self.tpc))))
            t += self.G
        return gs


def group_meta(cfg):
    """Per group: (gbase_chunk, sbase_slot, ltot, htot, loffs, hoffs)."""
    out = []
    gb = sb = 0
    for grp in cfg.groups:
        lo, ho = [], []
        lt = ht = 0
        for k in grp:
            lo.append(lt); lt += cfg.nlk[k]
        for k in grp:
            ho.append(ht); ht += cfg.nhk[k]
        out.append((gb, sb, lt, ht, lo, ho))
        gb += lt + ht
        sb += (lt + ht) * 128
    return out


def host_prep(cfg: Cfg, src, dst):
    """Per-core gidx/dstloc/sexp in grouped-slot order, p-major row ids."""
    rowid = (src.astype(np.int64) % 128) * NT + (src.astype(np.int64) // 128)
    is_h_all = rowid >= cfg.lsplit
    rloc_all = rowid - np.where(is_h_all, cfg.lsplit, 0)
    core_of = dst // cfg.own
    gm = group_meta(cfg)
    out = []
    for c in range(cfg.cores):
        sel = np.nonzero(core_of == c)[0]
        rl = rloc_all[sel]
        ih = is_h_all[sel]
        dloc = dst[sel].astype(np.int64) - c * cfg.own
        t_all = dloc // 128
        loc = dloc % 128
        gidx = np.zeros(cfg.slots, np.int64)
        dstloc = np.full(cfg.slots, -1.0, np.float32)
        order = np.lexsort((loc, ih, t_all))
        rl, t_all, loc, ih = rl[order], t_all[order], loc[order], ih[order]
        perm = cfg.perms[c]
        for gi_, grp in enumerate(cfg.groups):
            _, sbase, ltot, htot, lo, ho = gm[gi_]
            for i, k in enumerate(grp):
                tt = perm[k]
                for hs, budget, coff in ((0, cfg.nlk[k], lo[i]),
                                         (1, cfg.nhk[k], ltot + ho[i])):
                    mm = (t_all == tt) & (ih == hs)
                    n = int(mm.sum())
                    assert n <= budget * 128, \
                        f"c{c} k{k} hs{hs}: {n}>{budget*128}"
                    gi = np.zeros(budget * 128, np.int64)
                    gi[:n] = rl[mm]
                    dl = np.full(budget * 128, -1.0, np.float32)
                    dl[:n] = loc[mm].astype(np.float32)
                    s0 = sbase + coff * 128
                    gidx[s0:s0 + budget * 128] = gi
                    dstloc[s0:s0 + budget * 128] = dl
        gidx_w = np.ascontiguousarray(
            np.tile(gidx.astype(np.int16).reshape(-1, 16).T, (8, 1)))
        dstloc_pc = np.ascontiguousarray(
            dstloc.reshape(cfg.nchunk, 128).T)
        out.append(dict(gidx=gidx_w, dstloc=dstloc_pc))
    return out


def build_nc(cfg: Cfg):
    nc = bacc.Bacc(None)
    d = cfg.d
    TPC, NCH = cfg.tpc, cfg.nchunk
    GM = group_meta(cfg)
    MAXL = max(m[2] for m in GM)
    MAXH = max(m[3] for m in GM)
    KCUT = float(cfg.kcut)
    LROWS, HROWS = cfg.lsplit, cfg.nodes - cfg.lsplit

    feat = nc.dram_tensor("feat", [128, NT * d], F32, kind="ExternalInput")
    wT = nc.dram_tensor("wT", [d, d], F32, kind="ExternalInput")
    hpre = nc.dram_tensor("hpre", [128, TPC * d], F32, kind="ExternalInput")
    degp = nc.dram_tensor("degp", [128, TPC], F32, kind="ExternalInput")
    degall = nc.dram_tensor("degall", [128, NT], F32, kind="ExternalInput")
    gidx_e = nc.dram_tensor("gidx", [128, cfg.slots // 16], I16,
                            kind="ExternalInput")
    dstloc_e = nc.dram_tensor("dstloc", [128, NCH], F32, kind="ExternalInput")
    h_ext = nc.dram_tensor("h", [128, TPC * d], F32, kind="ExternalOutput")
    ts_dbg = nc.dram_tensor("ts_dbg", [1, 4], F32, kind="ExternalOutput")

    cc_in = nc.dram_tensor("cc_in", [1, 32], F32)
    cc_out = nc.dram_tensor("cc_out", [1, 32], F32, addr_space="Shared")
    ag_in = nc.dram_tensor("ag_in", [16, cfg.cand_cols], F32)
    ag_out = nc.dram_tensor("ag_out", [cfg.cores, 16, cfg.cand_cols], F32,
                            addr_space="Shared")
    groups_rep = [list(range(cfg.cores))]

    with tile.TileContext(nc) as tc:
        with (tc.tile_pool(name="const", bufs=1) as cpool,
              tc.tile_pool(name="state", bufs=1) as spool,
              tc.tile_pool(name="dram", bufs=1, space="DRAM") as dpool,
              tc.tile_pool(name="bfeat", bufs=2) as bfpool,
              tc.tile_pool(name="bstage", bufs=2) as bspool,
              tc.tile_pool(name="gath", bufs=2) as gpool,
              tc.tile_pool(name="ysb", bufs=4) as ypool_sb,
              tc.tile_pool(name="hstage", bufs=2) as hpool,
              tc.tile_pool(name="ypsum", bufs=2, space="PSUM") as ypool,
              tc.tile_pool(name="apsum", bufs=2, space="PSUM") as apool,
              tc.tile_pool(name="tpsum", bufs=2, space="PSUM") as tppool,
              tc.tile_pool(name="misc", bufs=3) as mpool,
              tc.tile_pool(name="y16p", bufs=2) as y16pool,
              tc.tile_pool(name="thr", bufs=1) as tpool):

            # ---------- DRAM tables (fp16, contiguous p-major rows;
            # pad slots gather garbage row 0 -- harmless, sexp col is 0) ----
            nhT = dpool.tile([cfg.nodes, d], F16, tag="nhT")
            ftT = dpool.tile([cfg.nodes, d], F16, tag="ftT")

            # ---------- constants / inputs ----------
            iota_row = cpool.tile([128, 128], F32, tag="iota_row")
            nc.gpsimd.iota(iota_row[:], pattern=[[1, 128]], base=0,
                           channel_multiplier=0,
                           allow_small_or_imprecise_dtypes=True)
            iota16 = cpool.tile([128, 128], F16, tag="iota16")
            nc.vector.tensor_copy(iota16[:], iota_row[:])
            iota_col = cpool.tile([128, 1], F32, tag="iota_col")
            nc.gpsimd.iota(iota_col[:], pattern=[[1, 1]], base=0,
                           channel_multiplier=1,
                           allow_small_or_imprecise_dtypes=True)
            ones_col = cpool.tile([128, 1], F32, tag="ones_col")
            nc.vector.memset(ones_col[:], 1.0)
            ones_row = cpool.tile([1, 128], F32, tag="ones_row")
            nc.vector.memset(ones_row[:], 1.0)
            degall_sb = spool.tile([128, NT], F32, tag="degall")
            nc.sync.dma_start(degall_sb[:], degall[:])
            norm_all = spool.tile([128, NT], F32, tag="norm_all")
            nc.vector.tensor_scalar_max(norm_all[:], degall_sb[:], 1.0)
            nc.scalar.activation(norm_all[:], norm_all[:], AF.Sqrt)
            nc.vector.reciprocal(norm_all[:], norm_all[:])


            # ---------- table build (blocked) ----------
            bscope = nc.named_scope("pbuild")
            bscope.__enter__()
            nb = cfg.bblk
            for b0 in range(0, NT, nb):
                bt = min(nb, NT - b0)
                fblk = bfpool.tile([128, nb * d], F32, tag="fblk")
                nc.sync.dma_start(fblk[:, :bt * d],
                                  feat[:, b0 * d:(b0 + bt) * d])
                bss = mpool.tile([128, nb], F32, tag="bss")
                for ti in range(bt):
                    scr = mpool.tile([128, d], F32, tag="bsq")
                    nc.vector.scalar_tensor_tensor(
                        scr[:], fblk[:, ti * d:(ti + 1) * d], 1.0,
                        fblk[:, ti * d:(ti + 1) * d],
                        op0=OP.mult, op1=OP.mult,
                        accum_out=bss[:, ti:ti + 1])
                nc.vector.tensor_scalar_max(bss[:, :bt], bss[:, :bt], 1e-24)
                nc.scalar.activation(bss[:, :bt], bss[:, :bt], AF.Sqrt)
                nc.vector.reciprocal(bss[:, :bt], bss[:, :bt])
                nh_st = bspool.tile([128, nb * d], F16, tag="nh_st")
                ft_st = bspool.tile([128, nb * d], F16, tag="ft_st")
                for ti in range(bt):
                    nc.vector.tensor_scalar_mul(
                        nh_st[:, ti * d:(ti + 1) * d],
                        fblk[:, ti * d:(ti + 1) * d], bss[:, ti:ti + 1])
                    nc.scalar.activation(
                        ft_st[:, ti * d:(ti + 1) * d],
                        fblk[:, ti * d:(ti + 1) * d], AF.Copy,
                        scale=norm_all[:, b0 + ti:b0 + ti + 1])
                for tab, st in ((nhT, nh_st), (ftT, ft_st)):
                    nc.sync.dma_start(
                        tab[:]
                        .rearrange("(p t) x -> p t x", t=NT)[:, b0:b0 + bt, :],
                        st[:, :bt * d]
                        .rearrange("p (t x) -> p t x", x=d))
            bscope.__exit__(None, None, None)
            # ---- A/B-phase inputs + nhiT, emitted late so the build
            # loop's DMA stream starts immediately ----
            gidx_sb = spool.tile([128, cfg.slots // 16], I16, tag="gidx")
            nc.sync.dma_start(gidx_sb[:], gidx_e[:])
            dstloc_sb = spool.tile([128, NCH], F32, tag="dstloc")
            nc.sync.dma_start(dstloc_sb[:], dstloc_e[:])
            dstloc_p1 = spool.tile([128, NCH], F32, tag="dstloc_p1")
            nc.vector.tensor_scalar_add(dstloc_p1[:], dstloc_sb[:], 1.0)
            hp_sb = spool.tile([128, TPC * d], F32, tag="hp")
            nc.sync.dma_start(hp_sb[:], hpre[:])
            wT_sb = cpool.tile([d, d], F32, tag="wT")
            nc.sync.dma_start(wT_sb[:], wT[:])
            wT16 = cpool.tile([d, d], F16, tag="wT16")
            nc.vector.tensor_copy(wT16[:], wT_sb[:])
            degp_sb = spool.tile([128, TPC], F32, tag="degp")
            nc.sync.dma_start(degp_sb[:], degp[:])
            norm_own = spool.tile([128, TPC], F32, tag="norm_own")
            nc.vector.tensor_scalar_max(norm_own[:], degp_sb[:], 1.0)
            nc.scalar.activation(norm_own[:], norm_own[:], AF.Sqrt)
            nc.vector.reciprocal(norm_own[:], norm_own[:])
            # nhi_all: normalized own-tile rows (fp16), for phase-A rhs
            ssq = spool.tile([128, TPC], F32, tag="ssq")
            for t in range(TPC):
                scr = mpool.tile([128, d], F32, tag="sqscr")
                nc.scalar.activation(scr[:], hp_sb[:, t * d:(t + 1) * d],
                                     AF.Square, accum_out=ssq[:, t:t + 1])
            invl_own = spool.tile([128, TPC], F32, tag="invl_own")
            # floor 1e-8 (not 1e-24): invl <= 1e4 stays fp16-finite in dmat;
            # zero-feature pad rows still give nhi = 0 * 1e4 = 0 exactly
            nc.vector.tensor_scalar_max(ssq[:], ssq[:], 1e-8)
            nc.scalar.activation(invl_own[:], ssq[:], AF.Sqrt)
            nc.vector.reciprocal(invl_own[:], invl_own[:])
            # nhiT_all[:, t*d+j] = nhi_t[j, :] transposed, via hp^T @ D
            ident32 = cpool.tile([128, 128], F32, tag="ident32")
            nc.vector.tensor_scalar(ident32[:], iota_row[:], iota_col[:],
                                    None, op0=OP.is_equal)
            nhiT_all = spool.tile([128, TPC * d], F16, tag="nhiT_all")
            for t in range(TPC):
                dmat = mpool.tile([128, d], F32, tag="dmat")
                nc.vector.tensor_scalar_mul(dmat[:], ident32[:],
                                            invl_own[:, t:t + 1])
                ntp = apool.tile([128, d], F32, tag="ab")
                nc.tensor.matmul(ntp[:], hp_sb[:, t * d:(t + 1) * d],
                                 dmat[:], start=True, stop=True)
                nc.scalar.copy(nhiT_all[:, t * d:(t + 1) * d], ntp[:])


            def emit_b_gather(gi_):
                _, sbase, ltot, htot, _, _ = GM[gi_]
                xfl = gpool.tile([128, MAXL, d], F16, tag="xgl")
                xfh = gpool.tile([128, MAXH, d], F16, tag="xgh")
                i0 = sbase // 16
                nc.gpsimd.dma_gather(
                    out_ap=xfl[:, :ltot, :], in_ap=ftT[0:LROWS, :],
                    idxs_ap=gidx_sb[:, i0:i0 + ltot * 8],
                    num_idxs=ltot * 128, num_idxs_reg=ltot * 128,
                    elem_size=d, single_packet=False)
                nc.gpsimd.dma_gather(
                    out_ap=xfh[:, :htot, :], in_ap=ftT[LROWS:cfg.nodes, :],
                    idxs_ap=gidx_sb[:, i0 + ltot * 8:i0 + (ltot + htot) * 8],
                    num_idxs=htot * 128, num_idxs_reg=htot * 128,
                    elem_size=d, single_packet=False)
                return xfl, xfh

            run_a = cfg.phases in ("A", "T", "full")
            run_t = cfg.phases in ("T", "full")
            run_b = cfg.phases == "full"
            cos_sb = spool.tile([128, NCH], F32, tag="cos")
            if not run_a:
                nc.vector.memset(cos_sb[:, :1], 0.0)

            # ---------- Phase A: cos ----------
            if run_a:
                ascope = nc.named_scope("pcos")
                ascope.__enter__()
                for gi_, grp in enumerate(cfg.groups):
                    gbase, sbase, ltot, htot, lo, ho = GM[gi_]
                    xgl = gpool.tile([128, MAXL * 128], F16, tag="xgl")
                    xgh = gpool.tile([128, MAXH * 128], F16, tag="xgh")
                    i0 = sbase // 16
                    nc.gpsimd.dma_gather(
                        out_ap=xgl[:, :ltot * 128]
                        .rearrange("p (o n) -> p o n", o=1),
                        in_ap=nhT[0:LROWS, :],
                        idxs_ap=gidx_sb[:, i0:i0 + ltot * 8],
                        num_idxs=ltot * 128, num_idxs_reg=ltot * 128,
                        elem_size=d, transpose=True, single_packet=False)
                    nc.gpsimd.dma_gather(
                        out_ap=xgh[:, :htot * 128]
                        .rearrange("p (o n) -> p o n", o=1),
                        in_ap=nhT[LROWS:cfg.nodes, :],
                        idxs_ap=gidx_sb[:, i0 + ltot * 8:
                                        i0 + (ltot + htot) * 8],
                        num_idxs=htot * 128, num_idxs_reg=htot * 128,
                        elem_size=d, transpose=True, single_packet=False)
                    for i, t in enumerate(grp):
                        nlt, nht = cfg.nlk[t], cfg.nhk[t]
                        nct_k = nlt + nht
                        for s0 in range(0, nct_k, cfg.slab):
                            sn = min(cfg.slab, nct_k - s0)
                            c2_ps = ypool.tile([128, cfg.slab * d], F32,
                                               tag="y")
                            for j in range(sn):
                                k = s0 + j
                                if k < nlt:
                                    xT_ap = xgl[:, (lo[i] + k) * 128:
                                                (lo[i] + k) * 128 + 128]
                                else:
                                    hc = ho[i] + (k - nlt)
                                    xT_ap = xgh[:, hc * 128:hc * 128 + 128]
                                nc.tensor.matmul(
                                    c2_ps[:, j * d:(j + 1) * d],
                                    xT_ap,
                                    nhiT_all[:, t * d:(t + 1) * d],
                                    start=True, stop=True)
                            c2_sb = ypool_sb.tile([128, cfg.slab * d], F16,
                                                  tag="ysb")
                            nc.scalar.copy(c2_sb[:, :sn * d],
                                           c2_ps[:, :sn * d])
                            for j in range(sn):
                                k = s0 + j
                                ccg = (lo[i] + k if k < nlt
                                       else ltot + ho[i] + (k - nlt))
                                cc = gbase + ccg
                                scr = mpool.tile([128, d], F16, tag="cscr")
                                nc.vector.scalar_tensor_tensor(
                                    scr[:], iota16[:],
                                    dstloc_sb[:, cc:cc + 1],
                                    c2_sb[:, j * d:(j + 1) * d],
                                    op0=OP.is_equal, op1=OP.mult,
                                    accum_out=cos_sb[:, cc:cc + 1])
                ascope.__exit__(None, None, None)

            # prefetch first B gather groups (overlap with threshold phase)
            pf = []
            if run_b:
                for gi in range(min(2, len(cfg.groups))):
                    pf.append(emit_b_gather(gi))

            # ---------- Phase T: threshold ----------
            if run_t:
                tscope = nc.named_scope("pthr")
                tscope.__enter__()
                lo_t = tpool.tile([1, 1], F32, tag="lo")
                th_row = tpool.tile([1, 32], F32, tag="throw")
                th_bc = tpool.tile([128, 32], F32, tag="thbc")
                cnt128 = tpool.tile([128, 32], F32, tag="cnt128")
                gcnt = tpool.tile([1, 32], F32, tag="gcnt")
                srow = tpool.tile([1, 1], F32, tag="srow")
                cbase = tpool.tile([1, 1], F32, tag="cbase")
                iota32 = tpool.tile([1, 32], F32, tag="iota32")
                nc.vector.tensor_copy(iota32[:], iota_row[:1, :32])
                msk = tpool.tile([1, 32], F32, tag="msk")
                msct = tpool.tile([1, 32], F32, tag="msct")
                cscr2 = tpool.tile([128, NCH], F32, tag="cscr2")
                cand = tpool.tile([128, cfg.cand_cols], F32, tag="cand")
                nc.vector.memset(cbase[:], 0.0)
                nc.vector.memset(lo_t[:], -0.75)

                def emit_round(vals_ap, ncols, w_bin, mode, shift4):
                    nc.vector.tensor_scalar_mul(th_row[:], iota32[:], w_bin)
                    nc.vector.tensor_scalar(th_row[:], th_row[:], lo_t[:],
                                            None, op0=OP.add)
                    if shift4:
                        nc.vector.tensor_scalar_add(th_row[:], th_row[:], 4.0)
                    ps = tppool.tile([128, 32], F32, tag="tiny")
                    nc.tensor.matmul(ps[:], ones_row[:], th_row[:],
                                     start=True, stop=True)
                    nc.vector.tensor_copy(th_bc[:], ps[:])
                    for j in range(32):
                        nc.vector.tensor_scalar(
                            cscr2[:, :ncols], vals_ap, th_bc[:, j:j + 1],
                            None, op0=OP.is_lt, op1=OP.add,
                            accum_out=cnt128[:, j:j + 1])
                    cps = tppool.tile([1, 32], F32, tag="tiny")
                    nc.tensor.matmul(cps[:], ones_col[:], cnt128[:],
                                     start=True, stop=True)
                    nc.vector.tensor_copy(gcnt[:], cps[:])
                    if mode.startswith("global"):
                        if cfg.coll:
                            nc.sync.dma_start(cc_in[:], gcnt[:])
                            nc.gpsimd.collective_compute(
                                "AllReduce", OP.add,
                                replica_groups=groups_rep,
                                ins=[cc_in[:]], outs=[cc_out[:]])
                            nc.sync.dma_start(gcnt[:], cc_out[:])
                        else:
                            nc.sync.dma_start(cc_in[:], gcnt[:])
                            nc.sync.dma_start(gcnt[:], cc_in[:])
                    nc.vector.tensor_scalar(
                        msct[:], gcnt[:], cbase[:], KCUT - 0.5,
                        op0=OP.add, op1=OP.is_lt)
                    nc.vector.tensor_scalar(
                        msct[:], msct[:], 0.0, None,
                        op0=OP.add, op1=OP.add, accum_out=srow[:])
                    nc.vector.tensor_scalar(srow[:], srow[:], -1.0, 0.0,
                                            op0=OP.add, op1=OP.max)
                    if mode == "global2":
                        nc.vector.tensor_scalar(msk[:], iota32[:], srow[:],
                                                None, op0=OP.is_equal)
                        nc.vector.scalar_tensor_tensor(
                            msct[:], gcnt[:], 1.0, msk[:],
                            op0=OP.mult, op1=OP.mult, accum_out=cbase[:])
                    nc.vector.scalar_tensor_tensor(
                        lo_t[:], srow[:], w_bin, lo_t[:],
                        op0=OP.mult, op1=OP.add)

                W1 = 1.5 / 32
                W2 = 1.5 / 32 ** 2
                emit_round(cos_sb[:], NCH, W1, "global1", False)
                emit_round(cos_sb[:], NCH, W2, "global2", False)

                # compact in-bracket values, remapped to cos+4
                lo_bc = tpool.tile([128, 1], F32, tag="lobc")
                psb = tppool.tile([128, 1], F32, tag="tiny")
                nc.tensor.matmul(psb[:], ones_row[:], lo_t[:],
                                 start=True, stop=True)
                nc.vector.tensor_copy(lo_bc[:], psb[:])
                m1 = tpool.tile([128, NCH], F32, tag="m1")
                nc.vector.tensor_scalar(m1[:], cos_sb[:], lo_bc[:], None,
                                        op0=OP.is_ge)
                hi_bc = tpool.tile([128, 1], F32, tag="hibc")
                nc.vector.tensor_scalar_add(hi_bc[:], lo_bc[:], W2)
                m2 = tpool.tile([128, NCH], F32, tag="m2")
                nc.vector.tensor_scalar(m2[:], cos_sb[:], hi_bc[:], None,
                                        op0=OP.is_lt)
                nc.vector.tensor_mul(m1[:], m1[:], m2[:])
                c4 = tpool.tile([128, NCH], F32, tag="c4")
                nc.vector.tensor_scalar(c4[:], cos_sb[:], 5.0, None,
                                        op0=OP.add)
                nc.vector.tensor_mul(c4[:], c4[:], m1[:])
                nc.vector.tensor_scalar_add(c4[:], c4[:], -1.0)
                NSG = 8
                sg_out_cols = cfg.cand_cols // NSG
                sgc = tpool.tile([16, cfg.cand_cols], F32, tag="sgc")
                posi = tpool.tile([16, sg_out_cols], F32, tag="posi")
                nc.gpsimd.iota(posi[:], pattern=[[16, sg_out_cols]], base=0,
                               channel_multiplier=1,
                               allow_small_or_imprecise_dtypes=True)
                for sg_i in range(NSG):
                    y16s = y16pool.tile([16, NCH], F32, tag="y16s")
                    nc.sync.dma_start(y16s[:], c4[16 * sg_i:16 * (sg_i + 1), :])
                    sg_out = y16pool.tile([16, sg_out_cols], F32,
                                          tag="sgout")
                    nfound = y16pool.tile([1, 1], U32, tag="nfound")
                    nc.gpsimd.sparse_gather(sg_out[:], y16s[:],
                                            num_found=nfound[:])
                    nf_f = y16pool.tile([1, 1], F32, tag="nff")
                    nc.vector.tensor_copy(nf_f[:], nfound[:])
                    nf16 = y16pool.tile([16, 1], F32, tag="nf16")
                    ps16 = tppool.tile([16, 1], F32, tag="tiny")
                    nc.tensor.matmul(ps16[:], ones_row[:, :16], nf_f[:],
                                     start=True, stop=True)
                    nc.vector.tensor_copy(nf16[:], ps16[:])
                    mtail = y16pool.tile([16, sg_out_cols], F32,
                                         tag="mtail")
                    nc.vector.tensor_scalar(mtail[:], posi[:], nf16[:], None,
                                            op0=OP.is_lt)
                    big = y16pool.tile([16, sg_out_cols], F32, tag="big")
                    nc.vector.tensor_scalar(big[:], mtail[:], 0.5, 1e30,
                                            op0=OP.is_lt, op1=OP.mult)
                    nc.vector.tensor_mul(sg_out[:], sg_out[:], mtail[:])
                    nc.vector.tensor_add(
                        sgc[:, sg_i * sg_out_cols:(sg_i + 1) * sg_out_cols],
                        sg_out[:], big[:])
                nc.sync.dma_start(ag_in[:], sgc[:])
                if cfg.coll:
                    nc.gpsimd.collective_compute(
                        "AllGather", OP.bypass, replica_groups=groups_rep,
                        ins=[ag_in[:]], outs=[ag_out[:]])
                    for r in range(cfg.cores):
                        nc.sync.dma_start(cand[16 * r:16 * (r + 1), :],
                                          ag_out[r, :, :])
                else:
                    for r in range(cfg.cores):
                        nc.sync.dma_start(cand[16 * r:16 * (r + 1), :],
                                          ag_in[:])

                wr = W2
                for r in range(cfg.nrounds_local):
                    wr = wr / 32
                    emit_round(cand[:], cfg.cand_cols, wr, "local", True)
                nc.vector.tensor_scalar_add(lo_t[:], lo_t[:], wr)
                tstar = tpool.tile([128, 1], F32, tag="tstar")
                pst = tppool.tile([128, 1], F32, tag="tiny")
                nc.tensor.matmul(pst[:], ones_row[:], lo_t[:],
                                 start=True, stop=True)
                nc.vector.tensor_copy(tstar[:], pst[:])

                nc.sync.dma_start(ts_dbg[:, 0:1], lo_t[:])
                nc.sync.dma_start(ts_dbg[:, 1:2], cbase[:])
                nc.sync.dma_start(ts_dbg[:, 2:3], srow[:])
                nc.sync.dma_start(ts_dbg[:, 3:4], nf_f[:])
                # dm = (cos >= t*) * (dstloc+1) - 1
                dm = tpool.tile([128, NCH], F32, tag="c4")
                nc.vector.scalar_tensor_tensor(
                    dm[:], cos_sb[:], tstar[:], dstloc_p1[:],
                    op0=OP.is_ge, op1=OP.mult)
                nc.vector.tensor_scalar_add(dm[:], dm[:], -1.0)
                tscope.__exit__(None, None, None)

            # ---------- Phase B: aggregate + linear + tail ----------
            if run_b:
                bscope2 = nc.named_scope("pagg")
                bscope2.__enter__()
                hblk = 8
                hout_st = None
                for gi, grp in enumerate(cfg.groups):
                    gbase, sbase, ltot, htot, lo, ho = GM[gi]
                    if gi < len(pf):
                        xfl, xfh = pf[gi]
                    else:
                        xfl, xfh = emit_b_gather(gi)
                    for i, t in enumerate(grp):
                        nlt, nht = cfg.nlk[t], cfg.nhk[t]
                        nct_k = nlt + nht
                        if t % hblk == 0:
                            hout_st = hpool.tile([128, hblk * d], F32,
                                                 tag="hout")
                        at_ps = apool.tile([128, 128], F32, tag="ab")
                        for k in range(nct_k):
                            if k < nlt:
                                cc = gbase + lo[i] + k
                                x_ap = xfl[:, lo[i] + k, :]
                            else:
                                cc = gbase + ltot + ho[i] + (k - nlt)
                                x_ap = xfh[:, ho[i] + (k - nlt), :]
                            sa = mpool.tile([128, 128], F16, tag="sa")
                            nc.vector.tensor_scalar(sa[:], iota16[:],
                                                    dm[:, cc:cc + 1], None,
                                                    op0=OP.is_equal)
                            nc.tensor.matmul(at_ps[:], x_ap, sa[:],
                                             start=(k == 0),
                                             stop=(k == nct_k - 1))
                        at_sb = mpool.tile([128, 128], F16, tag="aggTsb")
                        nc.scalar.copy(at_sb[:], at_ps[:])
                        h_ps = apool.tile([128, d], F32, tag="ab")
                        nc.tensor.matmul(h_ps[:], at_sb[:], wT16[:],
                                         start=True, stop=True)
                        hre = mpool.tile([128, d], F32, tag="hre")
                        nc.scalar.activation(hre[:], h_ps[:], AF.Relu,
                                             scale=norm_own[:, t:t + 1])
                        nc.vector.tensor_add(
                            hout_st[:, (t % hblk) * d:(t % hblk + 1) * d],
                            hre[:], hp_sb[:, t * d:(t + 1) * d])
                        if t % hblk == hblk - 1 or t == TPC - 1:
                            hb0 = (t // hblk) * hblk
                            nbt = t - hb0 + 1
                            nc.sync.dma_start(
                                h_ext[:, hb0 * d:(hb0 + nbt) * d],
                                hout_st[:, :nbt * d])
                bscope2.__exit__(None, None, None)

    nc.finalize()
    return nc


def make_cfg(src, dst, kcut, cores=8):
    cfg = Cfg(kcut=kcut, cores=cores)
    rowid = (src.astype(np.int64) % 128) * NT + (src.astype(np.int64) // 128)
    is_h = rowid >= cfg.lsplit
    core_of = dst // cfg.own
    tpc = cfg.tpc
    cntL = np.zeros((cores, tpc), np.int64)
    cntH = np.zeros((cores, tpc), np.int64)
    for c in range(cores):
        sel = core_of == c
        ih, dd = is_h[sel], dst[sel]
        t_all = (dd.astype(np.int64) - c * cfg.own) // 128
        cntL[c] = np.bincount(t_all[~ih], minlength=tpc)
        cntH[c] = np.bincount(t_all[ih], minlength=tpc)
    perms = [np.argsort(-(cntL[c] + cntH[c]), kind="stable")
             for c in range(cores)]
    sL = np.stack([cntL[c][perms[c]] for c in range(cores)])
    sH = np.stack([cntH[c][perms[c]] for c in range(cores)])
    cfg.nlk = tuple(max(1, int(np.ceil(sL[:, k].max() / 128)))
                    for k in range(tpc))
    cfg.nhk = tuple(max(1, int(np.ceil(sH[:, k].max() / 128)))
                    for k in range(tpc))
    cfg.perms = tuple(perms)
    NSG = 8
    sg_in = (cfg.nchunk * 8 + NSG - 1) // NSG
    cfg.cand_cols = NSG * min(8, sg_in)
    return cfg


def make_inputs(cfg: Cfg, features, W, src, dst):
    nreal = features.shape[0]
    featp = np.zeros((cfg.nodes, cfg.d), np.float32)
    featp[:nreal] = features
    feat_t = np.ascontiguousarray(
        featp.reshape(NT, 128, cfg.d).transpose(1, 0, 2).reshape(128, -1))
    deg = np.bincount(dst, minlength=cfg.nodes).astype(np.float32)
    degall = np.ascontiguousarray(deg.reshape(NT, 128).T)
    wTc = np.ascontiguousarray(W.T).astype(np.float32)
    percore = host_prep(cfg, src, dst)
    in_maps = []
    for c in range(cfg.cores):
        base = c * cfg.own
        perm = np.asarray(cfg.perms[c])
        degp = np.ascontiguousarray(
            deg[base:base + cfg.own].reshape(cfg.tpc, 128)[perm].T)
        hpre_t = np.ascontiguousarray(
            featp[base:base + cfg.own].reshape(cfg.tpc, 128, cfg.d)[perm]
            .transpose(1, 0, 2).reshape(128, -1))
        pc = percore[c]
        in_maps.append(dict(
            feat=feat_t, wT=wTc, hpre=hpre_t, degp=degp, degall=degall,
            gidx=pc["gidx"], dstloc=pc["dstloc"]))
    return in_maps


def run(cfg: Cfg, features, W, src, dst):
    in_maps = make_inputs(cfg, features, W, src, dst)
    nc = build_nc(cfg)
    r = run_bass_kernel_spmd(nc, in_maps, core_ids=list(range(cfg.cores)))
    hs = []
    for c in range(cfg.cores):
        hp = r.results[c]["h"]
        tkp = hp.reshape(128, cfg.tpc, cfg.d).transpose(1, 0, 2)
        unp = np.empty_like(tkp)
        unp[np.asarray(cfg.perms[c])] = tkp
        hs.append(unp.reshape(cfg.own, cfg.d))
    h = np.concatenate(hs, axis=0)
    return h[:features.shape[0]]


# ---------------- harness entry point ----------------
def kernel(features, W, src, dst):
    """Full inputs in, full output out. Edges sharded by dst range across
    8 NeuronCores; cosine cut threshold found exactly on-device via
    multi-round counting + candidate compaction + allgather."""
    src = np.asarray(src).astype(np.int32)
    dst = np.asarray(dst).astype(np.int32)
    features = np.asarray(features, dtype=np.float32)
    W = np.asarray(W, dtype=np.float32)
    kcut = int(src.shape[0] * 0.1)
    cfg = make_cfg(src, dst, kcut)
    return run(cfg, features, W, src, dst).astype(np.float32)


# revision 68
# speedup vs baseline: 1.0817x; 1.0019x over previous
"""GCN layer kernel for trn2: host prep + bass kernel builder + runner.

v2: fp16 tables (p-major row numbering), batched DMAs, fp8 sexp fed
directly to PE, fp16 DVE fast modes, grouped multi-tile gathers.
"""
import sys
sys.path.insert(0, '/opt/trn_rl_repo')
import numpy as np
import ml_dtypes
from dataclasses import dataclass

import concourse.bacc as bacc
import concourse.mybir as mybir
import concourse.tile as tile
from concourse.bass_utils import run_bass_kernel_spmd

F32 = mybir.dt.float32
F16 = mybir.dt.float16
I16 = mybir.dt.int16
U32 = mybir.dt.uint32
FP8 = mybir.dt.float8e4
AF = mybir.ActivationFunctionType
OP = mybir.AluOpType

NT = 392          # total node tiles (50176/128)


@dataclass
class Cfg:
    nodes: int = 50176
    lsplit: int = 32536       # 83*392; p-major row split (<=32768 for int16)
    cores: int = 8
    d: int = 128
    kcut: int = 0
    nlk: tuple = ()           # L chunks per tile-rank (len tpc)
    nhk: tuple = ()           # H chunks per tile-rank
    perms: tuple = ()         # per-core tile permutation (host side only)
    G: int = 5                # tiles per gather group
    slab: int = 8             # y chunks per PSUM slab
    bblk: int = 20            # build tiles per DMA block
    cand_cols: int = 256
    nrounds_local: int = 2
    phases: str = "full"      # "build", "A", "T", "full"
    coll: bool = True         # False: stub collectives (tlsim)

    @property
    def tpc(self):
        return self.nodes // 128 // self.cores

    @property
    def own(self):
        return self.nodes // self.cores

    @property
    def nchunk(self):
        return int(sum(self.nlk) + sum(self.nhk))

    @property
    def slots(self):
        return self.nchunk * 128

    @property
    def groups(self):
        gs = []
        t = 0
        while t < self.tpc:
            gs.append(list(range(t, min(t + self.G, self.tpc))))
            t += self.G
        return gs


def group_meta(cfg):
    """Per group: (gbase_chunk, sbase_slot, ltot, htot, loffs, hoffs)."""
    out = []
    gb = sb = 0
    for grp in cfg.groups:
        lo, ho = [], []
        lt = ht = 0
        for k in grp:
            lo.append(lt); lt += cfg.nlk[k]
        for k in grp:
            ho.append(ht); ht += cfg.nhk[k]
        out.append((gb, sb, lt, ht, lo, ho))
        gb += lt + ht
        sb += (lt + ht) * 128
    return out


def host_prep(cfg: Cfg, src, dst):
    """Per-core gidx/dstloc/sexp in grouped-slot order, p-major row ids."""
    rowid = (src.astype(np.int64) % 128) * NT + (src.astype(np.int64) // 128)
    is_h_all = rowid >= cfg.lsplit
    rloc_all = rowid - np.where(is_h_all, cfg.lsplit, 0)
    core_of = dst // cfg.own
    gm = group_meta(cfg)
    out = []
    for c in range(cfg.cores):
        sel = np.nonzero(core_of == c)[0]
        rl = rloc_all[sel]
        ih = is_h_all[sel]
        dloc = dst[sel].astype(np.int64) - c * cfg.own
        t_all = dloc // 128
        loc = dloc % 128
        gidx = np.zeros(cfg.slots, np.int64)
        dstloc = np.full(cfg.slots, -1.0, np.float32)
        order = np.lexsort((loc, ih, t_all))
        rl, t_all, loc, ih = rl[order], t_all[order], loc[order], ih[order]
        perm = cfg.perms[c]
        for gi_, grp in enumerate(cfg.groups):
            _, sbase, ltot, htot, lo, ho = gm[gi_]
            for i, k in enumerate(grp):
                tt = perm[k]
                for hs, budget, coff in ((0, cfg.nlk[k], lo[i]),
                                         (1, cfg.nhk[k], ltot + ho[i])):
                    mm = (t_all == tt) & (ih == hs)
                    n = int(mm.sum())
                    assert n <= budget * 128, \
                        f"c{c} k{k} hs{hs}: {n}>{budget*128}"
                    gi = np.zeros(budget * 128, np.int64)
                    gi[:n] = rl[mm]
                    dl = np.full(budget * 128, -1.0, np.float32)
                    dl[:n] = loc[mm].astype(np.float32)
                    s0 = sbase + coff * 128
                    gidx[s0:s0 + budget * 128] = gi
                    dstloc[s0:s0 + budget * 128] = dl
        gidx_w = np.ascontiguousarray(
            np.tile(gidx.astype(np.int16).reshape(-1, 16).T, (8, 1)))
        dstloc_pc = np.ascontiguousarray(
            dstloc.reshape(cfg.nchunk, 128).T)
        out.append(dict(gidx=gidx_w, dstloc=dstloc_pc))
    return out


def build_nc(cfg: Cfg):
    nc = bacc.Bacc(None)
    d = cfg.d
    TPC, NCH = cfg.tpc, cfg.nchunk
    GM = group_meta(cfg)
    MAXL = max(m[2] for m in GM)
    MAXH = max(m[3] for m in GM)
    KCUT = float(cfg.kcut)
    LROWS, HROWS = cfg.lsplit, cfg.nodes - cfg.lsplit

    feat = nc.dram_tensor("feat", [128, NT * d], F32, kind="ExternalInput")
    wT = nc.dram_tensor("wT", [d, d], F32, kind="ExternalInput")
    hpre = nc.dram_tensor("hpre", [128, TPC * d], F32, kind="ExternalInput")
    degp = nc.dram_tensor("degp", [128, TPC], F32, kind="ExternalInput")
    degall = nc.dram_tensor("degall", [128, NT], F32, kind="ExternalInput")
    gidx_e = nc.dram_tensor("gidx", [128, cfg.slots // 16], I16,
                            kind="ExternalInput")
    dstloc_e = nc.dram_tensor("dstloc", [128, NCH], F32, kind="ExternalInput")
    h_ext = nc.dram_tensor("h", [128, TPC * d], F32, kind="ExternalOutput")
    ts_dbg = nc.dram_tensor("ts_dbg", [1, 4], F32, kind="ExternalOutput")

    cc_in = nc.dram_tensor("cc_in", [1, 32], F32)
    cc_out = nc.dram_tensor("cc_out", [1, 32], F32, addr_space="Shared")
    ag_in = nc.dram_tensor("ag_in", [16, cfg.cand_cols], F32)
    ag_out = nc.dram_tensor("ag_out", [cfg.cores, 16, cfg.cand_cols], F32,
                            addr_space="Shared")
    groups_rep = [list(range(cfg.cores))]

    with tile.TileContext(nc) as tc:
        with (tc.tile_pool(name="const", bufs=1) as cpool,
              tc.tile_pool(name="state", bufs=1) as spool,
              tc.tile_pool(name="dram", bufs=1, space="DRAM") as dpool,
              tc.tile_pool(name="bfeat", bufs=2) as bfpool,
              tc.tile_pool(name="bstage", bufs=2) as bspool,
              tc.tile_pool(name="gath", bufs=2) as gpool,
              tc.tile_pool(name="ysb", bufs=4) as ypool_sb,
              tc.tile_pool(name="hstage", bufs=2) as hpool,
              tc.tile_pool(name="ypsum", bufs=2, space="PSUM") as ypool,
              tc.tile_pool(name="apsum", bufs=2, space="PSUM") as apool,
              tc.tile_pool(name="tpsum", bufs=2, space="PSUM") as tppool,
              tc.tile_pool(name="misc", bufs=3) as mpool,
              tc.tile_pool(name="y16p", bufs=2) as y16pool,
              tc.tile_pool(name="thr", bufs=1) as tpool):

            # ---------- DRAM tables (fp16, contiguous p-major rows;
            # pad slots gather garbage row 0 -- harmless, sexp col is 0) ----
            nhT = dpool.tile([cfg.nodes, d], F16, tag="nhT")
            ftT = dpool.tile([cfg.nodes, d], F16, tag="ftT")

            # ---------- constants / inputs ----------
            iota_row = cpool.tile([128, 128], F32, tag="iota_row")
            nc.gpsimd.iota(iota_row[:], pattern=[[1, 128]], base=0,
                           channel_multiplier=0,
                           allow_small_or_imprecise_dtypes=True)
            iota16 = cpool.tile([128, 128], F16, tag="iota16")
            nc.vector.tensor_copy(iota16[:], iota_row[:])
            iota_col = cpool.tile([128, 1], F32, tag="iota_col")
            nc.gpsimd.iota(iota_col[:], pattern=[[1, 1]], base=0,
                           channel_multiplier=1,
                           allow_small_or_imprecise_dtypes=True)
            ones_col = cpool.tile([128, 1], F32, tag="ones_col")
            nc.vector.memset(ones_col[:], 1.0)
            ones_row = cpool.tile([1, 128], F32, tag="ones_row")
            nc.vector.memset(ones_row[:], 1.0)
            degall_sb = spool.tile([128, NT], F32, tag="degall")
            nc.sync.dma_start(degall_sb[:], degall[:])
            norm_all = spool.tile([128, NT], F32, tag="norm_all")
            nc.vector.tensor_scalar_max(norm_all[:], degall_sb[:], 1.0)
            nc.scalar.activation(norm_all[:], norm_all[:], AF.Sqrt)
            nc.vector.reciprocal(norm_all[:], norm_all[:])


            # ---------- table build (blocked) ----------
            bscope = nc.named_scope("pbuild")
            bscope.__enter__()
            nb = cfg.bblk
            for b0 in range(0, NT, nb):
                bt = min(nb, NT - b0)
                fblk = bfpool.tile([128, nb * d], F32, tag="fblk")
                nc.sync.dma_start(fblk[:, :bt * d],
                                  feat[:, b0 * d:(b0 + bt) * d])
                bss = mpool.tile([128, nb], F32, tag="bss")
                for ti in range(bt):
                    scr = mpool.tile([128, d], F32, tag="bsq")
                    nc.vector.scalar_tensor_tensor(
                        scr[:], fblk[:, ti * d:(ti + 1) * d], 1.0,
                        fblk[:, ti * d:(ti + 1) * d],
                        op0=OP.mult, op1=OP.mult,
                        accum_out=bss[:, ti:ti + 1])
                nc.vector.tensor_scalar_max(bss[:, :bt], bss[:, :bt], 1e-24)
                nc.scalar.activation(bss[:, :bt], bss[:, :bt], AF.Sqrt)
                nc.vector.reciprocal(bss[:, :bt], bss[:, :bt])
                nh_st = bspool.tile([128, nb * d], F16, tag="nh_st")
                ft_st = bspool.tile([128, nb * d], F16, tag="ft_st")
                for ti in range(bt):
                    nc.vector.tensor_scalar_mul(
                        nh_st[:, ti * d:(ti + 1) * d],
                        fblk[:, ti * d:(ti + 1) * d], bss[:, ti:ti + 1])
                    nc.scalar.activation(
                        ft_st[:, ti * d:(ti + 1) * d],
                        fblk[:, ti * d:(ti + 1) * d], AF.Copy,
                        scale=norm_all[:, b0 + ti:b0 + ti + 1])
                for tab, st in ((nhT, nh_st), (ftT, ft_st)):
                    nc.sync.dma_start(
                        tab[:]
                        .rearrange("(p t) x -> p t x", t=NT)[:, b0:b0 + bt, :],
                        st[:, :bt * d]
                        .rearrange("p (t x) -> p t x", x=d))
            bscope.__exit__(None, None, None)
            # ---- A/B-phase inputs + nhiT, emitted late so the build
            # loop's DMA stream starts immediately ----
            gidx_sb = spool.tile([128, cfg.slots // 16], I16, tag="gidx")
            nc.sync.dma_start(gidx_sb[:], gidx_e[:])
            dstloc_sb = spool.tile([128, NCH], F32, tag="dstloc")
            nc.sync.dma_start(dstloc_sb[:], dstloc_e[:])
            dstloc_p1 = spool.tile([128, NCH], F32, tag="dstloc_p1")
            nc.vector.tensor_scalar_add(dstloc_p1[:], dstloc_sb[:], 1.0)
            hp_sb = spool.tile([128, TPC * d], F32, tag="hp")
            nc.sync.dma_start(hp_sb[:], hpre[:])
            wT_sb = cpool.tile([d, d], F32, tag="wT")
            nc.sync.dma_start(wT_sb[:], wT[:])
            wT16 = cpool.tile([d, d], F16, tag="wT16")
            nc.vector.tensor_copy(wT16[:], wT_sb[:])
            degp_sb = spool.tile([128, TPC], F32, tag="degp")
            nc.sync.dma_start(degp_sb[:], degp[:])
            norm_own = spool.tile([128, TPC], F32, tag="norm_own")
            nc.vector.tensor_scalar_max(norm_own[:], degp_sb[:], 1.0)
            nc.scalar.activation(norm_own[:], norm_own[:], AF.Sqrt)
            nc.vector.reciprocal(norm_own[:], norm_own[:])
            # nhi_all: normalized own-tile rows (fp16), for phase-A rhs
            ssq = spool.tile([128, TPC], F32, tag="ssq")
            for t in range(TPC):
                scr = mpool.tile([128, d], F32, tag="sqscr")
                nc.scalar.activation(scr[:], hp_sb[:, t * d:(t + 1) * d],
                                     AF.Square, accum_out=ssq[:, t:t + 1])
            invl_own = spool.tile([128, TPC], F32, tag="invl_own")
            # floor 1e-8 (not 1e-24): invl <= 1e4 stays fp16-finite in dmat;
            # zero-feature pad rows still give nhi = 0 * 1e4 = 0 exactly
            nc.vector.tensor_scalar_max(ssq[:], ssq[:], 1e-8)
            nc.scalar.activation(invl_own[:], ssq[:], AF.Sqrt)
            nc.vector.reciprocal(invl_own[:], invl_own[:])
            # nhiT_all[:, t*d+j] = nhi_t[j, :] transposed, via hp^T @ D
            ident32 = cpool.tile([128, 128], F32, tag="ident32")
            nc.vector.tensor_scalar(ident32[:], iota_row[:], iota_col[:],
                                    None, op0=OP.is_equal)
            nhiT_all = spool.tile([128, TPC * d], F16, tag="nhiT_all")
            for t in range(TPC):
                dmat = mpool.tile([128, d], F32, tag="dmat")
                nc.vector.tensor_scalar_mul(dmat[:], ident32[:],
                                            invl_own[:, t:t + 1])
                ntp = apool.tile([128, d], F32, tag="ab")
                nc.tensor.matmul(ntp[:], hp_sb[:, t * d:(t + 1) * d],
                                 dmat[:], start=True, stop=True)
                nc.scalar.copy(nhiT_all[:, t * d:(t + 1) * d], ntp[:])


            def emit_b_gather(gi_):
                _, sbase, ltot, htot, _, _ = GM[gi_]
                xfl = gpool.tile([128, MAXL, d], F16, tag="xgl")
                xfh = gpool.tile([128, MAXH, d], F16, tag="xgh")
                i0 = sbase // 16
                nc.gpsimd.dma_gather(
                    out_ap=xfl[:, :ltot, :], in_ap=ftT[0:LROWS, :],
                    idxs_ap=gidx_sb[:, i0:i0 + ltot * 8],
                    num_idxs=ltot * 128, num_idxs_reg=ltot * 128,
                    elem_size=d, single_packet=False)
                nc.gpsimd.dma_gather(
                    out_ap=xfh[:, :htot, :], in_ap=ftT[LROWS:cfg.nodes, :],
                    idxs_ap=gidx_sb[:, i0 + ltot * 8:i0 + (ltot + htot) * 8],
                    num_idxs=htot * 128, num_idxs_reg=htot * 128,
                    elem_size=d, single_packet=False)
                return xfl, xfh

            run_a = cfg.phases in ("A", "T", "full")
            run_t = cfg.phases in ("T", "full")
            run_b = cfg.phases == "full"
            cos_sb = spool.tile([128, NCH], F32, tag="cos")
            if not run_a:
                nc.vector.memset(cos_sb[:, :1], 0.0)

            # ---------- Phase A: cos ----------
            if run_a:
                ascope = nc.named_scope("pcos")
                ascope.__enter__()
                for gi_, grp in enumerate(cfg.groups):
                    gbase, sbase, ltot, htot, lo, ho = GM[gi_]
                    xgl = gpool.tile([128, MAXL * 128], F16, tag="xgl")
                    xgh = gpool.tile([128, MAXH * 128], F16, tag="xgh")
                    i0 = sbase // 16
                    nc.gpsimd.dma_gather(
                        out_ap=xgl[:, :ltot * 128]
                        .rearrange("p (o n) -> p o n", o=1),
                        in_ap=nhT[0:LROWS, :],
                        idxs_ap=gidx_sb[:, i0:i0 + ltot * 8],
                        num_idxs=ltot * 128, num_idxs_reg=ltot * 128,
                        elem_size=d, transpose=True, single_packet=False)
                    nc.gpsimd.dma_gather(
                        out_ap=xgh[:, :htot * 128]
                        .rearrange("p (o n) -> p o n", o=1),
                        in_ap=nhT[LROWS:cfg.nodes, :],
                        idxs_ap=gidx_sb[:, i0 + ltot * 8:
                                        i0 + (ltot + htot) * 8],
                        num_idxs=htot * 128, num_idxs_reg=htot * 128,
                        elem_size=d, transpose=True, single_packet=False)
                    for i, t in enumerate(grp):
                        nlt, nht = cfg.nlk[t], cfg.nhk[t]
                        nct_k = nlt + nht
                        for s0 in range(0, nct_k, cfg.slab):
                            sn = min(cfg.slab, nct_k - s0)
                            c2_ps = ypool.tile([128, cfg.slab * d], F32,
                                               tag="y")
                            for j in range(sn):
                                k = s0 + j
                                if k < nlt:
                                    xT_ap = xgl[:, (lo[i] + k) * 128:
                                                (lo[i] + k) * 128 + 128]
                                else:
                                    hc = ho[i] + (k - nlt)
                                    xT_ap = xgh[:, hc * 128:hc * 128 + 128]
                                nc.tensor.matmul(
                                    c2_ps[:, j * d:(j + 1) * d],
                                    xT_ap,
                                    nhiT_all[:, t * d:(t + 1) * d],
                                    start=True, stop=True)
                            c2_sb = ypool_sb.tile([128, cfg.slab * d], F16,
                                                  tag="ysb")
                            nc.scalar.copy(c2_sb[:, :sn * d],
                                           c2_ps[:, :sn * d])
                            for j in range(sn):
                                k = s0 + j
                                ccg = (lo[i] + k if k < nlt
                                       else ltot + ho[i] + (k - nlt))
                                cc = gbase + ccg
                                scr = mpool.tile([128, d], F16, tag="cscr")
                                nc.vector.scalar_tensor_tensor(
                                    scr[:], iota16[:],
                                    dstloc_sb[:, cc:cc + 1],
                                    c2_sb[:, j * d:(j + 1) * d],
                                    op0=OP.is_equal, op1=OP.mult,
                                    accum_out=cos_sb[:, cc:cc + 1])
                ascope.__exit__(None, None, None)

            # prefetch first B gather groups (overlap with threshold phase)
            pf = []
            if run_b:
                for gi in range(min(2, len(cfg.groups))):
                    pf.append(emit_b_gather(gi))

            # ---------- Phase T: threshold ----------
            if run_t:
                tscope = nc.named_scope("pthr")
                tscope.__enter__()
                lo_t = tpool.tile([1, 1], F32, tag="lo")
                th_row = tpool.tile([1, 32], F32, tag="throw")
                th_bc = tpool.tile([128, 32], F32, tag="thbc")
                cnt128 = tpool.tile([128, 32], F32, tag="cnt128")
                gcnt = tpool.tile([1, 32], F32, tag="gcnt")
                srow = tpool.tile([1, 1], F32, tag="srow")
                cbase = tpool.tile([1, 1], F32, tag="cbase")
                iota32 = tpool.tile([1, 32], F32, tag="iota32")
                nc.vector.tensor_copy(iota32[:], iota_row[:1, :32])
                msk = tpool.tile([1, 32], F32, tag="msk")
                msct = tpool.tile([1, 32], F32, tag="msct")
                cscr2 = tpool.tile([128, NCH], F32, tag="cscr2")
                cand = tpool.tile([128, cfg.cand_cols], F32, tag="cand")
                nc.vector.memset(cbase[:], 0.0)
                nc.vector.memset(lo_t[:], -0.75)

                def emit_round(vals_ap, ncols, w_bin, mode, shift4):
                    nc.vector.tensor_scalar_mul(th_row[:], iota32[:], w_bin)
                    nc.vector.tensor_scalar(th_row[:], th_row[:], lo_t[:],
                                            None, op0=OP.add)
                    if shift4:
                        nc.vector.tensor_scalar_add(th_row[:], th_row[:], 4.0)
                    ps = tppool.tile([128, 32], F32, tag="tiny")
                    nc.tensor.matmul(ps[:], ones_row[:], th_row[:],
                                     start=True, stop=True)
                    nc.vector.tensor_copy(th_bc[:], ps[:])
                    for j in range(32):
                        nc.vector.tensor_scalar(
                            cscr2[:, :ncols], vals_ap, th_bc[:, j:j + 1],
                            None, op0=OP.is_lt, op1=OP.add,
                            accum_out=cnt128[:, j:j + 1])
                    cps = tppool.tile([1, 32], F32, tag="tiny")
                    nc.tensor.matmul(cps[:], ones_col[:], cnt128[:],
                                     start=True, stop=True)
                    nc.vector.tensor_copy(gcnt[:], cps[:])
                    if mode.startswith("global"):
                        if cfg.coll:
                            nc.sync.dma_start(cc_in[:], gcnt[:])
                            nc.gpsimd.collective_compute(
                                "AllReduce", OP.add,
                                replica_groups=groups_rep,
                                ins=[cc_in[:]], outs=[cc_out[:]])
                            nc.sync.dma_start(gcnt[:], cc_out[:])
                        else:
                            nc.sync.dma_start(cc_in[:], gcnt[:])
                            nc.sync.dma_start(gcnt[:], cc_in[:])
                    nc.vector.tensor_scalar(
                        msct[:], gcnt[:], cbase[:], KCUT - 0.5,
                        op0=OP.add, op1=OP.is_lt)
                    nc.vector.tensor_scalar(
                        msct[:], msct[:], 0.0, None,
                        op0=OP.add, op1=OP.add, accum_out=srow[:])
                    nc.vector.tensor_scalar(srow[:], srow[:], -1.0, 0.0,
                                            op0=OP.add, op1=OP.max)
                    if mode == "global2":
                        nc.vector.tensor_scalar(msk[:], iota32[:], srow[:],
                                                None, op0=OP.is_equal)
                        nc.vector.scalar_tensor_tensor(
                            msct[:], gcnt[:], 1.0, msk[:],
                            op0=OP.mult, op1=OP.mult, accum_out=cbase[:])
                    nc.vector.scalar_tensor_tensor(
                        lo_t[:], srow[:], w_bin, lo_t[:],
                        op0=OP.mult, op1=OP.add)

                W1 = 1.5 / 32
                W2 = 1.5 / 32 ** 2
                emit_round(cos_sb[:], NCH, W1, "global1", False)
                emit_round(cos_sb[:], NCH, W2, "global2", False)

                # compact in-bracket values, remapped to cos+4
                lo_bc = tpool.tile([128, 1], F32, tag="lobc")
                psb = tppool.tile([128, 1], F32, tag="tiny")
                nc.tensor.matmul(psb[:], ones_row[:], lo_t[:],
                                 start=True, stop=True)
                nc.vector.tensor_copy(lo_bc[:], psb[:])
                hi_bc = tpool.tile([128, 1], F32, tag="hibc")
                nc.vector.tensor_scalar_add(hi_bc[:], lo_bc[:], W2)
                c5 = tpool.tile([128, NCH], F32, tag="m2")
                nc.vector.tensor_scalar(c5[:], cos_sb[:], 5.0, None,
                                        op0=OP.add)
                t1 = tpool.tile([128, NCH], F32, tag="m1")
                nc.vector.scalar_tensor_tensor(
                    t1[:], cos_sb[:], lo_bc[:], c5[:],
                    op0=OP.is_ge, op1=OP.mult)
                c4 = tpool.tile([128, NCH], F32, tag="c4")
                nc.vector.scalar_tensor_tensor(
                    c4[:], cos_sb[:], hi_bc[:], t1[:],
                    op0=OP.is_lt, op1=OP.mult)
                nc.vector.tensor_scalar_add(c4[:], c4[:], -1.0)
                NSG = 8
                sg_out_cols = cfg.cand_cols // NSG
                sgc = tpool.tile([16, cfg.cand_cols], F32, tag="sgc")
                posi = tpool.tile([16, sg_out_cols], F32, tag="posi")
                nc.gpsimd.iota(posi[:], pattern=[[16, sg_out_cols]], base=0,
                               channel_multiplier=1,
                               allow_small_or_imprecise_dtypes=True)
                for sg_i in range(NSG):
                    y16s = y16pool.tile([16, NCH], F32, tag="y16s")
                    nc.sync.dma_start(y16s[:], c4[16 * sg_i:16 * (sg_i + 1), :])
                    sg_out = y16pool.tile([16, sg_out_cols], F32,
                                          tag="sgout")
                    nfound = y16pool.tile([1, 1], U32, tag="nfound")
                    nc.gpsimd.sparse_gather(sg_out[:], y16s[:],
                                            num_found=nfound[:])
                    nf_f = y16pool.tile([1, 1], F32, tag="nff")
                    nc.vector.tensor_copy(nf_f[:], nfound[:])
                    nf16 = y16pool.tile([16, 1], F32, tag="nf16")
                    ps16 = tppool.tile([16, 1], F32, tag="tiny")
                    nc.tensor.matmul(ps16[:], ones_row[:, :16], nf_f[:],
                                     start=True, stop=True)
                    nc.vector.tensor_copy(nf16[:], ps16[:])
                    mtail = y16pool.tile([16, sg_out_cols], F32,
                                         tag="mtail")
                    nc.vector.tensor_scalar(mtail[:], posi[:], nf16[:], None,
                                            op0=OP.is_lt)
                    big = y16pool.tile([16, sg_out_cols], F32, tag="big")
                    nc.vector.tensor_scalar(big[:], mtail[:], 0.5, 1e30,
                                            op0=OP.is_lt, op1=OP.mult)
                    nc.vector.tensor_mul(sg_out[:], sg_out[:], mtail[:])
                    nc.vector.tensor_add(
                        sgc[:, sg_i * sg_out_cols:(sg_i + 1) * sg_out_cols],
                        sg_out[:], big[:])
                nc.sync.dma_start(ag_in[:], sgc[:])
                if cfg.coll:
                    nc.gpsimd.collective_compute(
                        "AllGather", OP.bypass, replica_groups=groups_rep,
                        ins=[ag_in[:]], outs=[ag_out[:]])
                    for r in range(cfg.cores):
                        nc.sync.dma_start(cand[16 * r:16 * (r + 1), :],
                                          ag_out[r, :, :])
                else:
                    for r in range(cfg.cores):
                        nc.sync.dma_start(cand[16 * r:16 * (r + 1), :],
                                          ag_in[:])

                wr = W2
                for r in range(cfg.nrounds_local):
                    wr = wr / 32
                    emit_round(cand[:], cfg.cand_cols, wr, "local", True)
                nc.vector.tensor_scalar_add(lo_t[:], lo_t[:], wr)
                tstar = tpool.tile([128, 1], F32, tag="tstar")
                pst = tppool.tile([128, 1], F32, tag="tiny")
                nc.tensor.matmul(pst[:], ones_row[:], lo_t[:],
                                 start=True, stop=True)
                nc.vector.tensor_copy(tstar[:], pst[:])

                nc.sync.dma_start(ts_dbg[:, 0:1], lo_t[:])
                nc.sync.dma_start(ts_dbg[:, 1:2], cbase[:])
                nc.sync.dma_start(ts_dbg[:, 2:3], srow[:])
                nc.sync.dma_start(ts_dbg[:, 3:4], nf_f[:])
                # dm = (cos >= t*) * (dstloc+1) - 1
                dm = tpool.tile([128, NCH], F32, tag="c4")
                nc.vector.scalar_tensor_tensor(
                    dm[:], cos_sb[:], tstar[:], dstloc_p1[:],
                    op0=OP.is_ge, op1=OP.mult)
                nc.vector.tensor_scalar_add(dm[:], dm[:], -1.0)
                tscope.__exit__(None, None, None)

            # ---------- Phase B: aggregate + linear + tail ----------
            if run_b:
                bscope2 = nc.named_scope("pagg")
                bscope2.__enter__()
                hblk = 8
                hout_st = None
                for gi, grp in enumerate(cfg.groups):
                    gbase, sbase, ltot, htot, lo, ho = GM[gi]
                    if gi < len(pf):
                        xfl, xfh = pf[gi]
                    else:
                        xfl, xfh = emit_b_gather(gi)
                    for i, t in enumerate(grp):
                        nlt, nht = cfg.nlk[t], cfg.nhk[t]
                        nct_k = nlt + nht
                        if t % hblk == 0:
                            hout_st = hpool.tile([128, hblk * d], F32,
                                                 tag="hout")
                        at_ps = apool.tile([128, 128], F32, tag="ab")
                        for k in range(nct_k):
                            if k < nlt:
                                cc = gbase + lo[i] + k
                                x_ap = xfl[:, lo[i] + k, :]
                            else:
                                cc = gbase + ltot + ho[i] + (k - nlt)
                                x_ap = xfh[:, ho[i] + (k - nlt), :]
                            sa = mpool.tile([128, 128], F16, tag="sa")
                            nc.vector.tensor_scalar(sa[:], iota16[:],
                                                    dm[:, cc:cc + 1], None,
                                                    op0=OP.is_equal)
                            nc.tensor.matmul(at_ps[:], x_ap, sa[:],
                                             start=(k == 0),
                                             stop=(k == nct_k - 1))
                        at_sb = mpool.tile([128, 128], F16, tag="aggTsb")
                        nc.scalar.copy(at_sb[:], at_ps[:])
                        h_ps = apool.tile([128, d], F32, tag="ab")
                        nc.tensor.matmul(h_ps[:], at_sb[:], wT16[:],
                                         start=True, stop=True)
                        hre = mpool.tile([128, d], F32, tag="hre")
                        nc.scalar.activation(hre[:], h_ps[:], AF.Relu,
                                             scale=norm_own[:, t:t + 1])
                        nc.vector.tensor_add(
                            hout_st[:, (t % hblk) * d:(t % hblk + 1) * d],
                            hre[:], hp_sb[:, t * d:(t + 1) * d])
                        if t % hblk == hblk - 1 or t == TPC - 1:
                            hb0 = (t // hblk) * hblk
                            nbt = t - hb0 + 1
                            nc.sync.dma_start(
                                h_ext[:, hb0 * d:(hb0 + nbt) * d],
                                hout_st[:, :nbt * d])
                bscope2.__exit__(None, None, None)

    nc.finalize()
    return nc


def make_cfg(src, dst, kcut, cores=8):
    cfg = Cfg(kcut=kcut, cores=cores)
    rowid = (src.astype(np.int64) % 128) * NT + (src.astype(np.int64) // 128)
    is_h = rowid >= cfg.lsplit
    core_of = dst // cfg.own
    tpc = cfg.tpc
    cntL = np.zeros((cores, tpc), np.int64)
    cntH = np.zeros((cores, tpc), np.int64)
    for c in range(cores):
        sel = core_of == c
        ih, dd = is_h[sel], dst[sel]
        t_all = (dd.astype(np.int64) - c * cfg.own) // 128
        cntL[c] = np.bincount(t_all[~ih], minlength=tpc)
        cntH[c] = np.bincount(t_all[ih], minlength=tpc)
    perms = [np.argsort(-(cntL[c] + cntH[c]), kind="stable")
             for c in range(cores)]
    sL = np.stack([cntL[c][perms[c]] for c in range(cores)])
    sH = np.stack([cntH[c][perms[c]] for c in range(cores)])
    cfg.nlk = tuple(max(1, int(np.ceil(sL[:, k].max() / 128)))
                    for k in range(tpc))
    cfg.nhk = tuple(max(1, int(np.ceil(sH[:, k].max() / 128)))
                    for k in range(tpc))
    cfg.perms = tuple(perms)
    NSG = 8
    sg_in = (cfg.nchunk * 8 + NSG - 1) // NSG
    cfg.cand_cols = NSG * min(8, sg_in)
    return cfg


def make_inputs(cfg: Cfg, features, W, src, dst):
    nreal = features.shape[0]
    featp = np.zeros((cfg.nodes, cfg.d), np.float32)
    featp[:nreal] = features
    feat_t = np.ascontiguousarray(
        featp.reshape(NT, 128, cfg.d).transpose(1, 0, 2).reshape(128, -1))
    deg = np.bincount(dst, minlength=cfg.nodes).astype(np.float32)
    degall = np.ascontiguousarray(deg.reshape(NT, 128).T)
    wTc = np.ascontiguousarray(W.T).astype(np.float32)
    percore = host_prep(cfg, src, dst)
    in_maps = []
    for c in range(cfg.cores):
        base = c * cfg.own
        perm = np.asarray(cfg.perms[c])
        degp = np.ascontiguousarray(
            deg[base:base + cfg.own].reshape(cfg.tpc, 128)[perm].T)
        hpre_t = np.ascontiguousarray(
            featp[base:base + cfg.own].reshape(cfg.tpc, 128, cfg.d)[perm]
            .transpose(1, 0, 2).reshape(128, -1))
        pc = percore[c]
        in_maps.append(dict(
            feat=feat_t, wT=wTc, hpre=hpre_t, degp=degp, degall=degall,
            gidx=pc["gidx"], dstloc=pc["dstloc"]))
    return in_maps


def run(cfg: Cfg, features, W, src, dst):
    in_maps = make_inputs(cfg, features, W, src, dst)
    nc = build_nc(cfg)
    r = run_bass_kernel_spmd(nc, in_maps, core_ids=list(range(cfg.cores)))
    hs = []
    for c in range(cfg.cores):
        hp = r.results[c]["h"]
        tkp = hp.reshape(128, cfg.tpc, cfg.d).transpose(1, 0, 2)
        unp = np.empty_like(tkp)
        unp[np.asarray(cfg.perms[c])] = tkp
        hs.append(unp.reshape(cfg.own, cfg.d))
    h = np.concatenate(hs, axis=0)
    return h[:features.shape[0]]


# ---------------- harness entry point ----------------
def kernel(features, W, src, dst):
    """Full inputs in, full output out. Edges sharded by dst range across
    8 NeuronCores; cosine cut threshold found exactly on-device via
    multi-round counting + candidate compaction + allgather."""
    src = np.asarray(src).astype(np.int32)
    dst = np.asarray(dst).astype(np.int32)
    features = np.asarray(features, dtype=np.float32)
    W = np.asarray(W, dtype=np.float32)
    kcut = int(src.shape[0] * 0.1)
    cfg = make_cfg(src, dst, kcut)
    return run(cfg, features, W, src, dst).astype(np.float32)


# revision 69
# speedup vs baseline: 1.0833x; 1.0014x over previous
"""GCN layer kernel for trn2: host prep + bass kernel builder + runner.

v2: fp16 tables (p-major row numbering), batched DMAs, fp8 sexp fed
directly to PE, fp16 DVE fast modes, grouped multi-tile gathers.
"""
import sys
sys.path.insert(0, '/opt/trn_rl_repo')
import numpy as np
import ml_dtypes
from dataclasses import dataclass

import concourse.bacc as bacc
import concourse.mybir as mybir
import concourse.tile as tile
from concourse.bass_utils import run_bass_kernel_spmd

F32 = mybir.dt.float32
F16 = mybir.dt.float16
I16 = mybir.dt.int16
U32 = mybir.dt.uint32
FP8 = mybir.dt.float8e4
AF = mybir.ActivationFunctionType
OP = mybir.AluOpType

NT = 392          # total node tiles (50176/128)


@dataclass
class Cfg:
    nodes: int = 50176
    lsplit: int = 32536       # 83*392; p-major row split (<=32768 for int16)
    cores: int = 8
    d: int = 128
    kcut: int = 0
    nlk: tuple = ()           # L chunks per tile-rank (len tpc)
    nhk: tuple = ()           # H chunks per tile-rank
    perms: tuple = ()         # per-core tile permutation (host side only)
    G: int = 5                # tiles per gather group
    slab: int = 8             # y chunks per PSUM slab
    bblk: int = 20            # build tiles per DMA block
    cand_cols: int = 256
    nrounds_local: int = 2
    phases: str = "full"      # "build", "A", "T", "full"
    coll: bool = True         # False: stub collectives (tlsim)

    @property
    def tpc(self):
        return self.nodes // 128 // self.cores

    @property
    def own(self):
        return self.nodes // self.cores

    @property
    def nchunk(self):
        return int(sum(self.nlk) + sum(self.nhk))

    @property
    def slots(self):
        return self.nchunk * 128

    @property
    def groups(self):
        gs = []
        t = 0
        while t < self.tpc:
            gs.append(list(range(t, min(t + self.G, self.tpc))))
            t += self.G
        return gs


def group_meta(cfg):
    """Per group: (gbase_chunk, sbase_slot, ltot, htot, loffs, hoffs)."""
    out = []
    gb = sb = 0
    for grp in cfg.groups:
        lo, ho = [], []
        lt = ht = 0
        for k in grp:
            lo.append(lt); lt += cfg.nlk[k]
        for k in grp:
            ho.append(ht); ht += cfg.nhk[k]
        out.append((gb, sb, lt, ht, lo, ho))
        gb += lt + ht
        sb += (lt + ht) * 128
    return out


def host_prep(cfg: Cfg, src, dst):
    """Per-core gidx/dstloc/sexp in grouped-slot order, p-major row ids."""
    rowid = (src.astype(np.int64) % 128) * NT + (src.astype(np.int64) // 128)
    is_h_all = rowid >= cfg.lsplit
    rloc_all = rowid - np.where(is_h_all, cfg.lsplit, 0)
    core_of = dst // cfg.own
    gm = group_meta(cfg)
    out = []
    for c in range(cfg.cores):
        sel = np.nonzero(core_of == c)[0]
        rl = rloc_all[sel]
        ih = is_h_all[sel]
        dloc = dst[sel].astype(np.int64) - c * cfg.own
        t_all = dloc // 128
        loc = dloc % 128
        gidx = np.zeros(cfg.slots, np.int64)
        dstloc = np.full(cfg.slots, -1.0, np.float32)
        order = np.lexsort((loc, ih, t_all))
        rl, t_all, loc, ih = rl[order], t_all[order], loc[order], ih[order]
        perm = cfg.perms[c]
        for gi_, grp in enumerate(cfg.groups):
            _, sbase, ltot, htot, lo, ho = gm[gi_]
            for i, k in enumerate(grp):
                tt = perm[k]
                for hs, budget, coff in ((0, cfg.nlk[k], lo[i]),
                                         (1, cfg.nhk[k], ltot + ho[i])):
                    mm = (t_all == tt) & (ih == hs)
                    n = int(mm.sum())
                    assert n <= budget * 128, \
                        f"c{c} k{k} hs{hs}: {n}>{budget*128}"
                    gi = np.zeros(budget * 128, np.int64)
                    gi[:n] = rl[mm]
                    dl = np.full(budget * 128, -1.0, np.float32)
                    dl[:n] = loc[mm].astype(np.float32)
                    s0 = sbase + coff * 128
                    gidx[s0:s0 + budget * 128] = gi
                    dstloc[s0:s0 + budget * 128] = dl
        gidx_w = np.ascontiguousarray(
            np.tile(gidx.astype(np.int16).reshape(-1, 16).T, (8, 1)))
        dstloc_pc = np.ascontiguousarray(
            dstloc.reshape(cfg.nchunk, 128).T)
        out.append(dict(gidx=gidx_w, dstloc=dstloc_pc))
    return out


def build_nc(cfg: Cfg):
    nc = bacc.Bacc(None)
    d = cfg.d
    TPC, NCH = cfg.tpc, cfg.nchunk
    GM = group_meta(cfg)
    MAXL = max(m[2] for m in GM)
    MAXH = max(m[3] for m in GM)
    KCUT = float(cfg.kcut)
    LROWS, HROWS = cfg.lsplit, cfg.nodes - cfg.lsplit

    feat = nc.dram_tensor("feat", [128, NT * d], F32, kind="ExternalInput")
    wT = nc.dram_tensor("wT", [d, d], F32, kind="ExternalInput")
    hpre = nc.dram_tensor("hpre", [128, TPC * d], F32, kind="ExternalInput")
    degp = nc.dram_tensor("degp", [128, TPC], F32, kind="ExternalInput")
    degall = nc.dram_tensor("degall", [128, NT], F32, kind="ExternalInput")
    gidx_e = nc.dram_tensor("gidx", [128, cfg.slots // 16], I16,
                            kind="ExternalInput")
    dstloc_e = nc.dram_tensor("dstloc", [128, NCH], F32, kind="ExternalInput")
    h_ext = nc.dram_tensor("h", [128, TPC * d], F32, kind="ExternalOutput")
    ts_dbg = nc.dram_tensor("ts_dbg", [1, 4], F32, kind="ExternalOutput")

    cc_in = nc.dram_tensor("cc_in", [1, 32], F32)
    cc_out = nc.dram_tensor("cc_out", [1, 32], F32, addr_space="Shared")
    ag_in = nc.dram_tensor("ag_in", [16, cfg.cand_cols], F32)
    ag_out = nc.dram_tensor("ag_out", [cfg.cores, 16, cfg.cand_cols], F32,
                            addr_space="Shared")
    groups_rep = [list(range(cfg.cores))]

    with tile.TileContext(nc) as tc:
        with (tc.tile_pool(name="const", bufs=1) as cpool,
              tc.tile_pool(name="state", bufs=1) as spool,
              tc.tile_pool(name="dram", bufs=1, space="DRAM") as dpool,
              tc.tile_pool(name="bfeat", bufs=2) as bfpool,
              tc.tile_pool(name="bstage", bufs=2) as bspool,
              tc.tile_pool(name="gath", bufs=2) as gpool,
              tc.tile_pool(name="ysb", bufs=4) as ypool_sb,
              tc.tile_pool(name="hstage", bufs=2) as hpool,
              tc.tile_pool(name="ypsum", bufs=2, space="PSUM") as ypool,
              tc.tile_pool(name="apsum", bufs=2, space="PSUM") as apool,
              tc.tile_pool(name="tpsum", bufs=2, space="PSUM") as tppool,
              tc.tile_pool(name="misc", bufs=3) as mpool,
              tc.tile_pool(name="y16p", bufs=2) as y16pool,
              tc.tile_pool(name="thr", bufs=1) as tpool):

            # ---------- DRAM tables (fp16, contiguous p-major rows;
            # pad slots gather garbage row 0 -- harmless, sexp col is 0) ----
            nhT = dpool.tile([cfg.nodes, d], F16, tag="nhT")
            ftT = dpool.tile([cfg.nodes, d], F16, tag="ftT")

            # ---------- constants / inputs ----------
            iota_row = cpool.tile([128, 128], F32, tag="iota_row")
            nc.gpsimd.iota(iota_row[:], pattern=[[1, 128]], base=0,
                           channel_multiplier=0,
                           allow_small_or_imprecise_dtypes=True)
            iota16 = cpool.tile([128, 128], F16, tag="iota16")
            nc.vector.tensor_copy(iota16[:], iota_row[:])
            iota_col = cpool.tile([128, 1], F32, tag="iota_col")
            nc.gpsimd.iota(iota_col[:], pattern=[[1, 1]], base=0,
                           channel_multiplier=1,
                           allow_small_or_imprecise_dtypes=True)
            ones_col = cpool.tile([128, 1], F32, tag="ones_col")
            nc.vector.memset(ones_col[:], 1.0)
            ones_row = cpool.tile([1, 128], F32, tag="ones_row")
            nc.vector.memset(ones_row[:], 1.0)
            degall_sb = spool.tile([128, NT], F32, tag="degall")
            nc.sync.dma_start(degall_sb[:], degall[:])
            norm_all = spool.tile([128, NT], F32, tag="norm_all")
            nc.vector.tensor_scalar_max(norm_all[:], degall_sb[:], 1.0)
            nc.scalar.activation(norm_all[:], norm_all[:], AF.Sqrt)
            nc.vector.reciprocal(norm_all[:], norm_all[:])


            # ---------- table build (blocked) ----------
            bscope = nc.named_scope("pbuild")
            bscope.__enter__()
            nb = cfg.bblk
            for b0 in range(0, NT, nb):
                bt = min(nb, NT - b0)
                fblk = bfpool.tile([128, nb * d], F32, tag="fblk")
                nc.sync.dma_start(fblk[:, :bt * d],
                                  feat[:, b0 * d:(b0 + bt) * d])
                bss = mpool.tile([128, nb], F32, tag="bss")
                for ti in range(bt):
                    scr = mpool.tile([128, d], F32, tag="bsq")
                    nc.vector.scalar_tensor_tensor(
                        scr[:], fblk[:, ti * d:(ti + 1) * d], 1.0,
                        fblk[:, ti * d:(ti + 1) * d],
                        op0=OP.mult, op1=OP.mult,
                        accum_out=bss[:, ti:ti + 1])
                nc.vector.tensor_scalar_max(bss[:, :bt], bss[:, :bt], 1e-24)
                nc.scalar.activation(bss[:, :bt], bss[:, :bt], AF.Sqrt)
                nc.vector.reciprocal(bss[:, :bt], bss[:, :bt])
                nh_st = bspool.tile([128, nb * d], F16, tag="nh_st")
                ft_st = bspool.tile([128, nb * d], F16, tag="ft_st")
                for ti in range(bt):
                    nc.vector.tensor_scalar_mul(
                        nh_st[:, ti * d:(ti + 1) * d],
                        fblk[:, ti * d:(ti + 1) * d], bss[:, ti:ti + 1])
                    nc.scalar.activation(
                        ft_st[:, ti * d:(ti + 1) * d],
                        fblk[:, ti * d:(ti + 1) * d], AF.Copy,
                        scale=norm_all[:, b0 + ti:b0 + ti + 1])
                for tab, st in ((nhT, nh_st), (ftT, ft_st)):
                    nc.sync.dma_start(
                        tab[:]
                        .rearrange("(p t) x -> p t x", t=NT)[:, b0:b0 + bt, :],
                        st[:, :bt * d]
                        .rearrange("p (t x) -> p t x", x=d))
            bscope.__exit__(None, None, None)
            # ---- A/B-phase inputs + nhiT, emitted late so the build
            # loop's DMA stream starts immediately ----
            gidx_sb = spool.tile([128, cfg.slots // 16], I16, tag="gidx")
            nc.sync.dma_start(gidx_sb[:], gidx_e[:])
            dstloc_sb = spool.tile([128, NCH], F32, tag="dstloc")
            nc.sync.dma_start(dstloc_sb[:], dstloc_e[:])
            dstloc_p1 = spool.tile([128, NCH], F32, tag="dstloc_p1")
            nc.vector.tensor_scalar_add(dstloc_p1[:], dstloc_sb[:], 1.0)
            hp_sb = spool.tile([128, TPC * d], F32, tag="hp")
            nc.sync.dma_start(hp_sb[:], hpre[:])
            wT_sb = cpool.tile([d, d], F32, tag="wT")
            nc.sync.dma_start(wT_sb[:], wT[:])
            wT16 = cpool.tile([d, d], F16, tag="wT16")
            nc.vector.tensor_copy(wT16[:], wT_sb[:])
            degp_sb = spool.tile([128, TPC], F32, tag="degp")
            nc.sync.dma_start(degp_sb[:], degp[:])
            norm_own = spool.tile([128, TPC], F32, tag="norm_own")
            nc.vector.tensor_scalar_max(norm_own[:], degp_sb[:], 1.0)
            nc.scalar.activation(norm_own[:], norm_own[:], AF.Sqrt)
            nc.vector.reciprocal(norm_own[:], norm_own[:])
            # nhi_all: normalized own-tile rows (fp16), for phase-A rhs
            ssq = spool.tile([128, TPC], F32, tag="ssq")
            for t in range(TPC):
                scr = mpool.tile([128, d], F32, tag="sqscr")
                nc.scalar.activation(scr[:], hp_sb[:, t * d:(t + 1) * d],
                                     AF.Square, accum_out=ssq[:, t:t + 1])
            invl_own = spool.tile([128, TPC], F32, tag="invl_own")
            # floor 1e-8 (not 1e-24): invl <= 1e4 stays fp16-finite in dmat;
            # zero-feature pad rows still give nhi = 0 * 1e4 = 0 exactly
            nc.vector.tensor_scalar_max(ssq[:], ssq[:], 1e-8)
            nc.scalar.activation(invl_own[:], ssq[:], AF.Sqrt)
            nc.vector.reciprocal(invl_own[:], invl_own[:])
            # nhiT_all[:, t*d+j] = nhi_t[j, :] transposed, via hp^T @ D
            ident32 = cpool.tile([128, 128], F32, tag="ident32")
            nc.vector.tensor_scalar(ident32[:], iota_row[:], iota_col[:],
                                    None, op0=OP.is_equal)
            nhiT_all = spool.tile([128, TPC * d], F16, tag="nhiT_all")
            for t in range(TPC):
                dmat = mpool.tile([128, d], F32, tag="dmat")
                nc.vector.tensor_scalar_mul(dmat[:], ident32[:],
                                            invl_own[:, t:t + 1])
                ntp = apool.tile([128, d], F32, tag="ab")
                nc.tensor.matmul(ntp[:], hp_sb[:, t * d:(t + 1) * d],
                                 dmat[:], start=True, stop=True)
                nc.scalar.copy(nhiT_all[:, t * d:(t + 1) * d], ntp[:])


            def emit_b_gather(gi_):
                _, sbase, ltot, htot, _, _ = GM[gi_]
                xfl = gpool.tile([128, MAXL, d], F16, tag="xgl")
                xfh = gpool.tile([128, MAXH, d], F16, tag="xgh")
                i0 = sbase // 16
                nc.gpsimd.dma_gather(
                    out_ap=xfl[:, :ltot, :], in_ap=ftT[0:LROWS, :],
                    idxs_ap=gidx_sb[:, i0:i0 + ltot * 8],
                    num_idxs=ltot * 128, num_idxs_reg=ltot * 128,
                    elem_size=d, single_packet=False)
                nc.gpsimd.dma_gather(
                    out_ap=xfh[:, :htot, :], in_ap=ftT[LROWS:cfg.nodes, :],
                    idxs_ap=gidx_sb[:, i0 + ltot * 8:i0 + (ltot + htot) * 8],
                    num_idxs=htot * 128, num_idxs_reg=htot * 128,
                    elem_size=d, single_packet=False)
                return xfl, xfh

            _W1 = 1.5 / 32
            _W2 = 1.5 / 32 ** 2
            iw_r1 = cpool.tile([128, 32], F32, tag="iw_r1")
            nc.vector.tensor_scalar(iw_r1[:], iota_row[:, :32], _W1, None,
                                    op0=OP.mult)
            iw_r2 = cpool.tile([128, 32], F32, tag="iw_r2")
            nc.vector.tensor_scalar(iw_r2[:], iota_row[:, :32], _W2, None,
                                    op0=OP.mult)
            iw_l1 = cpool.tile([128, 32], F32, tag="iw_l1")
            nc.vector.tensor_scalar(iw_l1[:], iota_row[:, :32], _W2 / 32,
                                    4.0, op0=OP.mult, op1=OP.add)
            iw_l2 = cpool.tile([128, 32], F32, tag="iw_l2")
            nc.vector.tensor_scalar(iw_l2[:], iota_row[:, :32], _W2 / 1024,
                                    4.0, op0=OP.mult, op1=OP.add)

            run_a = cfg.phases in ("A", "T", "full")
            run_t = cfg.phases in ("T", "full")
            run_b = cfg.phases == "full"
            cos_sb = spool.tile([128, NCH], F32, tag="cos")
            if not run_a:
                nc.vector.memset(cos_sb[:, :1], 0.0)

            # ---------- Phase A: cos ----------
            if run_a:
                ascope = nc.named_scope("pcos")
                ascope.__enter__()
                for gi_, grp in enumerate(cfg.groups):
                    gbase, sbase, ltot, htot, lo, ho = GM[gi_]
                    xgl = gpool.tile([128, MAXL * 128], F16, tag="xgl")
                    xgh = gpool.tile([128, MAXH * 128], F16, tag="xgh")
                    i0 = sbase // 16
                    nc.gpsimd.dma_gather(
                        out_ap=xgl[:, :ltot * 128]
                        .rearrange("p (o n) -> p o n", o=1),
                        in_ap=nhT[0:LROWS, :],
                        idxs_ap=gidx_sb[:, i0:i0 + ltot * 8],
                        num_idxs=ltot * 128, num_idxs_reg=ltot * 128,
                        elem_size=d, transpose=True, single_packet=False)
                    nc.gpsimd.dma_gather(
                        out_ap=xgh[:, :htot * 128]
                        .rearrange("p (o n) -> p o n", o=1),
                        in_ap=nhT[LROWS:cfg.nodes, :],
                        idxs_ap=gidx_sb[:, i0 + ltot * 8:
                                        i0 + (ltot + htot) * 8],
                        num_idxs=htot * 128, num_idxs_reg=htot * 128,
                        elem_size=d, transpose=True, single_packet=False)
                    for i, t in enumerate(grp):
                        nlt, nht = cfg.nlk[t], cfg.nhk[t]
                        nct_k = nlt + nht
                        for s0 in range(0, nct_k, cfg.slab):
                            sn = min(cfg.slab, nct_k - s0)
                            c2_ps = ypool.tile([128, cfg.slab * d], F32,
                                               tag="y")
                            for j in range(sn):
                                k = s0 + j
                                if k < nlt:
                                    xT_ap = xgl[:, (lo[i] + k) * 128:
                                                (lo[i] + k) * 128 + 128]
                                else:
                                    hc = ho[i] + (k - nlt)
                                    xT_ap = xgh[:, hc * 128:hc * 128 + 128]
                                nc.tensor.matmul(
                                    c2_ps[:, j * d:(j + 1) * d],
                                    xT_ap,
                                    nhiT_all[:, t * d:(t + 1) * d],
                                    start=True, stop=True)
                            c2_sb = ypool_sb.tile([128, cfg.slab * d], F16,
                                                  tag="ysb")
                            nc.scalar.copy(c2_sb[:, :sn * d],
                                           c2_ps[:, :sn * d])
                            for j in range(sn):
                                k = s0 + j
                                ccg = (lo[i] + k if k < nlt
                                       else ltot + ho[i] + (k - nlt))
                                cc = gbase + ccg
                                scr = mpool.tile([128, d], F16, tag="cscr")
                                nc.vector.scalar_tensor_tensor(
                                    scr[:], iota16[:],
                                    dstloc_sb[:, cc:cc + 1],
                                    c2_sb[:, j * d:(j + 1) * d],
                                    op0=OP.is_equal, op1=OP.mult,
                                    accum_out=cos_sb[:, cc:cc + 1])
                ascope.__exit__(None, None, None)

            # prefetch first B gather groups (overlap with threshold phase)
            pf = []
            if run_b:
                for gi in range(min(2, len(cfg.groups))):
                    pf.append(emit_b_gather(gi))

            # ---------- Phase T: threshold ----------
            if run_t:
                tscope = nc.named_scope("pthr")
                tscope.__enter__()
                lo_t = tpool.tile([1, 1], F32, tag="lo")
                th_row = tpool.tile([1, 32], F32, tag="throw")
                th_bc = tpool.tile([128, 32], F32, tag="thbc")
                cnt128 = tpool.tile([128, 32], F32, tag="cnt128")
                gcnt = tpool.tile([1, 32], F32, tag="gcnt")
                srow = tpool.tile([1, 1], F32, tag="srow")
                cbase = tpool.tile([1, 1], F32, tag="cbase")
                iota32 = tpool.tile([1, 32], F32, tag="iota32")
                nc.vector.tensor_copy(iota32[:], iota_row[:1, :32])
                msk = tpool.tile([1, 32], F32, tag="msk")
                msct = tpool.tile([1, 32], F32, tag="msct")
                cscr2 = tpool.tile([128, NCH], F32, tag="cscr2")
                cand = tpool.tile([128, cfg.cand_cols], F32, tag="cand")
                nc.vector.memset(cbase[:], 0.0)
                nc.vector.memset(lo_t[:], -0.75)

                def emit_round(vals_ap, ncols, w_bin, mode, shift4,
                               pre_iw=None):
                    if pre_iw is not None:
                        psb2 = tppool.tile([128, 1], F32, tag="tiny")
                        nc.tensor.matmul(psb2[:], ones_row[:], lo_t[:],
                                         start=True, stop=True)
                        nc.vector.tensor_scalar(th_bc[:], pre_iw[:],
                                                psb2[:, 0:1], None,
                                                op0=OP.add)
                    else:
                        nc.vector.tensor_scalar_mul(th_row[:], iota32[:],
                                                    w_bin)
                        nc.vector.tensor_scalar(th_row[:], th_row[:],
                                                lo_t[:], None, op0=OP.add)
                        if shift4:
                            nc.vector.tensor_scalar_add(th_row[:], th_row[:],
                                                        4.0)
                        ps = tppool.tile([128, 32], F32, tag="tiny")
                        nc.tensor.matmul(ps[:], ones_row[:], th_row[:],
                                         start=True, stop=True)
                        nc.vector.tensor_copy(th_bc[:], ps[:])
                    for j in range(32):
                        nc.vector.tensor_scalar(
                            cscr2[:, :ncols], vals_ap, th_bc[:, j:j + 1],
                            None, op0=OP.is_lt, op1=OP.add,
                            accum_out=cnt128[:, j:j + 1])
                    cps = tppool.tile([1, 32], F32, tag="tiny")
                    nc.tensor.matmul(cps[:], ones_col[:], cnt128[:],
                                     start=True, stop=True)
                    nc.vector.tensor_copy(gcnt[:], cps[:])
                    if mode.startswith("global"):
                        if cfg.coll:
                            nc.sync.dma_start(cc_in[:], gcnt[:])
                            nc.gpsimd.collective_compute(
                                "AllReduce", OP.add,
                                replica_groups=groups_rep,
                                ins=[cc_in[:]], outs=[cc_out[:]])
                            nc.sync.dma_start(gcnt[:], cc_out[:])
                        else:
                            nc.sync.dma_start(cc_in[:], gcnt[:])
                            nc.sync.dma_start(gcnt[:], cc_in[:])
                    nc.vector.tensor_scalar(
                        msct[:], gcnt[:], cbase[:], KCUT - 0.5,
                        op0=OP.add, op1=OP.is_lt)
                    nc.vector.tensor_scalar(
                        msct[:], msct[:], 0.0, None,
                        op0=OP.add, op1=OP.add, accum_out=srow[:])
                    nc.vector.tensor_scalar(srow[:], srow[:], -1.0, 0.0,
                                            op0=OP.add, op1=OP.max)
                    if mode == "global2":
                        nc.vector.tensor_scalar(msk[:], iota32[:], srow[:],
                                                None, op0=OP.is_equal)
                        nc.vector.scalar_tensor_tensor(
                            msct[:], gcnt[:], 1.0, msk[:],
                            op0=OP.mult, op1=OP.mult, accum_out=cbase[:])
                    nc.vector.scalar_tensor_tensor(
                        lo_t[:], srow[:], w_bin, lo_t[:],
                        op0=OP.mult, op1=OP.add)

                W1 = 1.5 / 32
                W2 = 1.5 / 32 ** 2
                emit_round(cos_sb[:], NCH, W1, "global1", False,
                           pre_iw=iw_r1)
                emit_round(cos_sb[:], NCH, W2, "global2", False,
                           pre_iw=iw_r2)

                # compact in-bracket values, remapped to cos+4
                lo_bc = tpool.tile([128, 1], F32, tag="lobc")
                psb = tppool.tile([128, 1], F32, tag="tiny")
                nc.tensor.matmul(psb[:], ones_row[:], lo_t[:],
                                 start=True, stop=True)
                nc.vector.tensor_copy(lo_bc[:], psb[:])
                hi_bc = tpool.tile([128, 1], F32, tag="hibc")
                nc.vector.tensor_scalar_add(hi_bc[:], lo_bc[:], W2)
                c5 = tpool.tile([128, NCH], F32, tag="m2")
                nc.vector.tensor_scalar(c5[:], cos_sb[:], 5.0, None,
                                        op0=OP.add)
                t1 = tpool.tile([128, NCH], F32, tag="m1")
                nc.vector.scalar_tensor_tensor(
                    t1[:], cos_sb[:], lo_bc[:], c5[:],
                    op0=OP.is_ge, op1=OP.mult)
                c4 = tpool.tile([128, NCH], F32, tag="c4")
                nc.vector.scalar_tensor_tensor(
                    c4[:], cos_sb[:], hi_bc[:], t1[:],
                    op0=OP.is_lt, op1=OP.mult)
                nc.vector.tensor_scalar_add(c4[:], c4[:], -1.0)
                NSG = 8
                sg_out_cols = cfg.cand_cols // NSG
                sgc = tpool.tile([16, cfg.cand_cols], F32, tag="sgc")
                posi = tpool.tile([16, sg_out_cols], F32, tag="posi")
                nc.gpsimd.iota(posi[:], pattern=[[16, sg_out_cols]], base=0,
                               channel_multiplier=1,
                               allow_small_or_imprecise_dtypes=True)
                for sg_i in range(NSG):
                    y16s = y16pool.tile([16, NCH], F32, tag="y16s")
                    nc.sync.dma_start(y16s[:], c4[16 * sg_i:16 * (sg_i + 1), :])
                    sg_out = y16pool.tile([16, sg_out_cols], F32,
                                          tag="sgout")
                    nfound = y16pool.tile([1, 1], U32, tag="nfound")
                    nc.gpsimd.sparse_gather(sg_out[:], y16s[:],
                                            num_found=nfound[:])
                    nf_f = y16pool.tile([1, 1], F32, tag="nff")
                    nc.vector.tensor_copy(nf_f[:], nfound[:])
                    nf16 = y16pool.tile([16, 1], F32, tag="nf16")
                    ps16 = tppool.tile([16, 1], F32, tag="tiny")
                    nc.tensor.matmul(ps16[:], ones_row[:, :16], nf_f[:],
                                     start=True, stop=True)
                    nc.vector.tensor_copy(nf16[:], ps16[:])
                    mtail = y16pool.tile([16, sg_out_cols], F32,
                                         tag="mtail")
                    nc.vector.tensor_scalar(mtail[:], posi[:], nf16[:], None,
                                            op0=OP.is_lt)
                    big = y16pool.tile([16, sg_out_cols], F32, tag="big")
                    nc.vector.tensor_scalar(big[:], mtail[:], 0.5, 1e30,
                                            op0=OP.is_lt, op1=OP.mult)
                    nc.vector.tensor_mul(sg_out[:], sg_out[:], mtail[:])
                    nc.vector.tensor_add(
                        sgc[:, sg_i * sg_out_cols:(sg_i + 1) * sg_out_cols],
                        sg_out[:], big[:])
                nc.sync.dma_start(ag_in[:], sgc[:])
                if cfg.coll:
                    nc.gpsimd.collective_compute(
                        "AllGather", OP.bypass, replica_groups=groups_rep,
                        ins=[ag_in[:]], outs=[ag_out[:]])
                    for r in range(cfg.cores):
                        nc.sync.dma_start(cand[16 * r:16 * (r + 1), :],
                                          ag_out[r, :, :])
                else:
                    for r in range(cfg.cores):
                        nc.sync.dma_start(cand[16 * r:16 * (r + 1), :],
                                          ag_in[:])

                wr = W2
                iw_loc = [iw_l1, iw_l2]
                for r in range(cfg.nrounds_local):
                    wr = wr / 32
                    emit_round(cand[:], cfg.cand_cols, wr, "local", True,
                               pre_iw=iw_loc[r])
                nc.vector.tensor_scalar_add(lo_t[:], lo_t[:], wr)
                tstar = tpool.tile([128, 1], F32, tag="tstar")
                pst = tppool.tile([128, 1], F32, tag="tiny")
                nc.tensor.matmul(pst[:], ones_row[:], lo_t[:],
                                 start=True, stop=True)
                nc.vector.tensor_copy(tstar[:], pst[:])

                nc.sync.dma_start(ts_dbg[:, 0:1], lo_t[:])
                nc.sync.dma_start(ts_dbg[:, 1:2], cbase[:])
                nc.sync.dma_start(ts_dbg[:, 2:3], srow[:])
                nc.sync.dma_start(ts_dbg[:, 3:4], nf_f[:])
                # dm = (cos >= t*) * (dstloc+1) - 1
                dm = tpool.tile([128, NCH], F32, tag="c4")
                nc.vector.scalar_tensor_tensor(
                    dm[:], cos_sb[:], tstar[:], dstloc_p1[:],
                    op0=OP.is_ge, op1=OP.mult)
                nc.vector.tensor_scalar_add(dm[:], dm[:], -1.0)
                tscope.__exit__(None, None, None)

            # ---------- Phase B: aggregate + linear + tail ----------
            if run_b:
                bscope2 = nc.named_scope("pagg")
                bscope2.__enter__()
                hblk = 8
                hout_st = None
                for gi, grp in enumerate(cfg.groups):
                    gbase, sbase, ltot, htot, lo, ho = GM[gi]
                    if gi < len(pf):
                        xfl, xfh = pf[gi]
                    else:
                        xfl, xfh = emit_b_gather(gi)
                    for i, t in enumerate(grp):
                        nlt, nht = cfg.nlk[t], cfg.nhk[t]
                        nct_k = nlt + nht
                        if t % hblk == 0:
                            hout_st = hpool.tile([128, hblk * d], F32,
                                                 tag="hout")
                        at_ps = apool.tile([128, 128], F32, tag="ab")
                        for k in range(nct_k):
                            if k < nlt:
                                cc = gbase + lo[i] + k
                                x_ap = xfl[:, lo[i] + k, :]
                            else:
                                cc = gbase + ltot + ho[i] + (k - nlt)
                                x_ap = xfh[:, ho[i] + (k - nlt), :]
                            sa = mpool.tile([128, 128], F16, tag="sa")
                            nc.vector.tensor_scalar(sa[:], iota16[:],
                                                    dm[:, cc:cc + 1], None,
                                                    op0=OP.is_equal)
                            nc.tensor.matmul(at_ps[:], x_ap, sa[:],
                                             start=(k == 0),
                                             stop=(k == nct_k - 1))
                        at_sb = mpool.tile([128, 128], F16, tag="aggTsb")
                        nc.scalar.copy(at_sb[:], at_ps[:])
                        h_ps = apool.tile([128, d], F32, tag="ab")
                        nc.tensor.matmul(h_ps[:], at_sb[:], wT16[:],
                                         start=True, stop=True)
                        hre = mpool.tile([128, d], F32, tag="hre")
                        nc.scalar.activation(hre[:], h_ps[:], AF.Relu,
                                             scale=norm_own[:, t:t + 1])
                        nc.vector.tensor_add(
                            hout_st[:, (t % hblk) * d:(t % hblk + 1) * d],
                            hre[:], hp_sb[:, t * d:(t + 1) * d])
                        if t % hblk == hblk - 1 or t == TPC - 1:
                            hb0 = (t // hblk) * hblk
                            nbt = t - hb0 + 1
                            nc.sync.dma_start(
                                h_ext[:, hb0 * d:(hb0 + nbt) * d],
                                hout_st[:, :nbt * d])
                bscope2.__exit__(None, None, None)

    nc.finalize()
    return nc


def make_cfg(src, dst, kcut, cores=8):
    cfg = Cfg(kcut=kcut, cores=cores)
    rowid = (src.astype(np.int64) % 128) * NT + (src.astype(np.int64) // 128)
    is_h = rowid >= cfg.lsplit
    core_of = dst // cfg.own
    tpc = cfg.tpc
    cntL = np.zeros((cores, tpc), np.int64)
    cntH = np.zeros((cores, tpc), np.int64)
    for c in range(cores):
        sel = core_of == c
        ih, dd = is_h[sel], dst[sel]
        t_all = (dd.astype(np.int64) - c * cfg.own) // 128
        cntL[c] = np.bincount(t_all[~ih], minlength=tpc)
        cntH[c] = np.bincount(t_all[ih], minlength=tpc)
    perms = [np.argsort(-(cntL[c] + cntH[c]), kind="stable")
             for c in range(cores)]
    sL = np.stack([cntL[c][perms[c]] for c in range(cores)])
    sH = np.stack([cntH[c][perms[c]] for c in range(cores)])
    cfg.nlk = tuple(max(1, int(np.ceil(sL[:, k].max() / 128)))
                    for k in range(tpc))
    cfg.nhk = tuple(max(1, int(np.ceil(sH[:, k].max() / 128)))
                    for k in range(tpc))
    cfg.perms = tuple(perms)
    NSG = 8
    sg_in = (cfg.nchunk * 8 + NSG - 1) // NSG
    cfg.cand_cols = NSG * min(8, sg_in)
    return cfg


def make_inputs(cfg: Cfg, features, W, src, dst):
    nreal = features.shape[0]
    featp = np.zeros((cfg.nodes, cfg.d), np.float32)
    featp[:nreal] = features
    feat_t = np.ascontiguousarray(
        featp.reshape(NT, 128, cfg.d).transpose(1, 0, 2).reshape(128, -1))
    deg = np.bincount(dst, minlength=cfg.nodes).astype(np.float32)
    degall = np.ascontiguousarray(deg.reshape(NT, 128).T)
    wTc = np.ascontiguousarray(W.T).astype(np.float32)
    percore = host_prep(cfg, src, dst)
    in_maps = []
    for c in range(cfg.cores):
        base = c * cfg.own
        perm = np.asarray(cfg.perms[c])
        degp = np.ascontiguousarray(
            deg[base:base + cfg.own].reshape(cfg.tpc, 128)[perm].T)
        hpre_t = np.ascontiguousarray(
            featp[base:base + cfg.own].reshape(cfg.tpc, 128, cfg.d)[perm]
            .transpose(1, 0, 2).reshape(128, -1))
        pc = percore[c]
        in_maps.append(dict(
            feat=feat_t, wT=wTc, hpre=hpre_t, degp=degp, degall=degall,
            gidx=pc["gidx"], dstloc=pc["dstloc"]))
    return in_maps


def run(cfg: Cfg, features, W, src, dst):
    in_maps = make_inputs(cfg, features, W, src, dst)
    nc = build_nc(cfg)
    r = run_bass_kernel_spmd(nc, in_maps, core_ids=list(range(cfg.cores)))
    hs = []
    for c in range(cfg.cores):
        hp = r.results[c]["h"]
        tkp = hp.reshape(128, cfg.tpc, cfg.d).transpose(1, 0, 2)
        unp = np.empty_like(tkp)
        unp[np.asarray(cfg.perms[c])] = tkp
        hs.append(unp.reshape(cfg.own, cfg.d))
    h = np.concatenate(hs, axis=0)
    return h[:features.shape[0]]


# ---------------- harness entry point ----------------
def kernel(features, W, src, dst):
    """Full inputs in, full output out. Edges sharded by dst range across
    8 NeuronCores; cosine cut threshold found exactly on-device via
    multi-round counting + candidate compaction + allgather."""
    src = np.asarray(src).astype(np.int32)
    dst = np.asarray(dst).astype(np.int32)
    features = np.asarray(features, dtype=np.float32)
    W = np.asarray(W, dtype=np.float32)
    kcut = int(src.shape[0] * 0.1)
    cfg = make_cfg(src, dst, kcut)
    return run(cfg, features, W, src, dst).astype(np.float32)
